# revision 1
# baseline (speedup 1.0000x reference)
import sys
import numpy as np
import ml_dtypes

sys.path.insert(0, "/opt/trn_rl_repo")

import concourse.bass as bass
import concourse.tile as tile
from concourse import mybir
from concourse.bass_utils import run_bass_kernel_spmd

F32 = mybir.dt.float32
F32R = mybir.dt.float32r
BF16 = mybir.dt.bfloat16
AF = mybir.ActivationFunctionType
ALU = mybir.AluOpType

HID = 128
NT = 128       # tokens per image
NAH = 512      # atoms per core (half of 1024)
NG = 64        # ligand graphs
NI = 4         # images
NCORES = 8

TRACE = False
TRACE_KW = {}
LAST = None


_COMPUTE_INSTS = (
    "InstActivation", "InstTensorCopy", "InstTensorScalar", "InstTensorScalarPtr",
    "InstTensorTensor", "InstTensorTensorReduce", "InstTensorReduce", "InstMemSet",
    "InstMatmult", "InstScalarTensorTensor", "InstTensorTensorScan", "InstLdweights",
    "InstDMACopy", "InstDMATransposeAnt", "InstTriggeredCopy", "InstDrain",
    "InstEventSemaphoreOp", "InstSemaphoreOp", "InstCopy", "InstIota", "InstSelect",
)


def _legalize_waits(nc):
    # walrus in this toolchain accepts at most ONE sync wait on TPB compute
    # instructions; hoist extras into same-engine NoOps placed just before.
    k = 0
    for f in nc.m.functions:
        for blk in f.blocks:
            insts = blk.instructions
            out = []
            for ins in insts:
                si = getattr(ins, "sync_info", None)
                if (si is not None and len(si.on_wait) > 1
                        and type(ins).__name__ in _COMPUTE_INSTS):
                    waits = list(si.on_wait)
                    for w in waits[:-1]:
                        nop = mybir.InstNoOp(
                            name=f"WNOP-{k}", engine=ins.engine,
                            sync_info=mybir.SyncInfo(on_wait=[w], on_update=[]))
                        k += 1
                        out.append(nop)
                    ins.sync_info = mybir.SyncInfo(on_wait=[waits[-1]],
                                                   on_update=list(si.on_update))
                out.append(ins)
            blk.instructions = out
    return k


def build_program(bpe: float, bpg: float, bb2: float, bint_zero: bool = True, sim_trace: bool = False) -> bass.Bass:
    nc = bass.Bass()

    # ---- DRAM inputs (per-core views; same names across SPMD cores) ----
    d_tfT = nc.dram_tensor("tfT", [2, 128, 128], F32, kind="ExternalInput")
    d_laT = nc.dram_tensor("laT", [64, NAH], F32, kind="ExternalInput")
    d_lgT = nc.dram_tensor("lgT", [64, NG], F32, kind="ExternalInput")
    d_msf0 = nc.dram_tensor("msf0", [96, 4096], F32, kind="ExternalInput")
    d_msf1 = nc.dram_tensor("msf1", [64, 512], F32, kind="ExternalInput")
    d_S = nc.dram_tensor("Sh", [4, 128, NG], F32, kind="ExternalInput")

    d_wtok = nc.dram_tensor("W_token", [2, 128, HID], F32, kind="ExternalInput")
    d_w96 = nc.dram_tensor("W96", [9, 96, HID], F32, kind="ExternalInput")
    d_w0 = nc.dram_tensor("W0t", [27, 64, HID], F32, kind="ExternalInput")
    d_wpk = nc.dram_tensor("W_pocket", [2, 128, HID], F32, kind="ExternalInput")
    d_wcat = nc.dram_tensor("W_cat", [3, 128, HID], F32, kind="ExternalInput")
    d_wgate = nc.dram_tensor("W_gate", [3, 128, HID], F32, kind="ExternalInput")
    d_watom = nc.dram_tensor("W_atom", [64, HID], F32, kind="ExternalInput")
    d_wgraph = nc.dram_tensor("W_graph", [64, HID], F32, kind="ExternalInput")
    d_wb1 = nc.dram_tensor("W_bias1", [2, 128, HID], F32, kind="ExternalInput")
    d_wb2 = nc.dram_tensor("W_bias2", [128, 1], F32, kind="ExternalInput")
    d_wint = nc.dram_tensor("W_int", [128, HID], BF16, kind="ExternalInput")
    d_wpeg = nc.dram_tensor("W_peg", [128, 2], F32, kind="ExternalInput")

    d_btok = nc.dram_tensor("b_token", [128, 1], F32, kind="ExternalInput")
    d_bpk = nc.dram_tensor("b_pocket", [128, 1], F32, kind="ExternalInput")
    d_bcat = nc.dram_tensor("b_cat", [128, 1], F32, kind="ExternalInput")
    d_bgate = nc.dram_tensor("b_gate", [128, 1], F32, kind="ExternalInput")
    d_bgateh = nc.dram_tensor("b_gate_h", [128, 1], F32, kind="ExternalInput")
    d_batom = nc.dram_tensor("b_atom", [128, 1], F32, kind="ExternalInput")
    d_bgraph = nc.dram_tensor("b_graph", [128, 1], F32, kind="ExternalInput")
    d_bb1 = nc.dram_tensor("b_bias1", [128, 1], F32, kind="ExternalInput")
    d_bint = nc.dram_tensor("b_int", [128, 1], F32, kind="ExternalInput")

    d_seg = nc.dram_tensor("seg_out", [1, NG], F32, kind="ExternalOutput")
    d_bias = nc.dram_tensor("bias_out", [1, NG], F32, kind="ExternalOutput")

    tc_ref = tile.TileContext(nc, trace_sim=sim_trace)
    with tc_ref as tc:
        with (
            tc.tile_pool(name="const", bufs=1) as cpool,
            tc.tile_pool(name="pre_sb", bufs=2) as prepool,
            tc.tile_pool(name="big", bufs=1) as bigpool,
            tc.tile_pool(name="x", bufs=6) as xpool,
            tc.tile_pool(name="h", bufs=4) as hpool,
            tc.tile_pool(name="gate", bufs=4) as gpool,
            tc.tile_pool(name="junk", bufs=2) as jpool,
            tc.tile_pool(name="ps_pre", bufs=2, space="PSUM") as pspre,
            tc.tile_pool(name="ps_y", bufs=2, space="PSUM") as psy,
            tc.tile_pool(name="ps_z", bufs=2, space="PSUM") as psz,
        ):
            # ---------- loads ----------
            def load(pool, dram_ap, shape, tag, dt=F32):
                t = pool.tile(shape, dt, tag=tag)
                nc.sync.dma_start(t[:], dram_ap)
                return t

            def load_bias(pool, dram_ap, tag):
                tf_ = pool.tile([128, 1], F32, tag=tag + "_f")
                nc.sync.dma_start(tf_[:], dram_ap)
                t = pool.tile([128, 1], F32, tag=tag)
                nc.scalar.activation(t[:], tf_[:], AF.Copy)
                return t

            def cast_r(pool, src, shape, tag):
                t = pool.tile(shape, F32R, tag=tag)
                nc.scalar.activation(t[:], src[:], AF.Copy)
                return t

            wint = cpool.tile([128, HID], BF16, tag="wint")
            nc.sync.dma_start(wint[:], d_wint[:])
            wpeg_f = load(cpool, d_wpeg[:], [128, 2], "wpegf")
            wpeg = cast_r(cpool, wpeg_f, [128, 2], "wpeg")
            bint = load_bias(cpool, d_bint[:], "bint")
            watom_f = load(cpool, d_watom[:], [64, HID], "watomf")
            watom = cast_r(cpool, watom_f, [64, HID], "watom")
            batom = load_bias(cpool, d_batom[:], "batom")
            btok = load_bias(cpool, d_btok[:], "btok")
            wtok_f = cpool.tile([128, 2 * HID], F32, tag="wtokf")
            nc.sync.dma_start(wtok_f[:, 0:HID], d_wtok[0])
            nc.sync.dma_start(wtok_f[:, HID:2 * HID], d_wtok[1])
            wtok = cpool.tile([128, 2 * HID], F32R, tag="wtok")
            nc.scalar.activation(wtok[:, 0:HID], wtok_f[:, 0:HID], AF.Copy)
            nc.scalar.activation(wtok[:, HID:2 * HID], wtok_f[:, HID:2 * HID], AF.Copy)
            St = cpool.tile([128, 4 * NG], F32, tag="St")
            for q in range(4):
                nc.sync.dma_start(St[:, q * NG:(q + 1) * NG], d_S[q])

            # ---------- preamble: tok / atoms ----------
            tf = prepool.tile([128, 256], F32, tag="tf")
            nc.sync.dma_start(tf[:, 0:128], d_tfT[0])
            nc.sync.dma_start(tf[:, 128:256], d_tfT[1])
            # 2*silu(x) = x*(1+tanh(x/2)); factor 0.5 folded into W_token on host
            tft = prepool.tile([128, 256], F32, tag="tft")
            nc.scalar.activation(tft[:, 0:128], tf[:, 0:128], AF.Tanh, scale=0.5)
            nc.scalar.activation(tft[:, 128:256], tf[:, 128:256], AF.Tanh, scale=0.5)
            tfr = prepool.tile([128, 256], F32R, tag="tfr")
            nc.vector.scalar_tensor_tensor(tfr[:], tft[:], 1.0, tf[:],
                                           op0=ALU.add, op1=ALU.mult)
            ps_tok = pspre.tile([128, NT], F32, tag="ps")
            nc.tensor.matmul(ps_tok[:], wtok[:, 0:HID], tfr[:, 0:128], start=True, stop=False)
            nc.tensor.matmul(ps_tok[:], wtok[:, HID:2 * HID], tfr[:, 128:256], start=False, stop=True)
            tokT = cpool.tile([128, NT], F32, tag="tokT")
            nc.scalar.activation(tokT[:], ps_tok[:], AF.Identity, bias=btok[:])

            la_f = prepool.tile([64, NAH], F32, tag="laf")
            nc.sync.dma_start(la_f[:], d_laT[:])
            la = cast_r(prepool, la_f, [64, NAH], "la")
            ps_at = psy.tile([128, NAH], F32, tag="y")
            nc.tensor.matmul(ps_at[:], watom[:], la[:], start=True, stop=True)
            atomsT = cpool.tile([128, NAH], BF16, tag="atomsT")
            nc.scalar.activation(atomsT[:], ps_at[:], AF.Identity, bias=batom[:])

            # ---------- preamble: convs / pocket / pf / bias head ----------
            wpk = cpool.tile([128, 2 * HID], F32, tag="wpk")
            nc.sync.dma_start(wpk[:, 0:HID], d_wpk[0])
            nc.sync.dma_start(wpk[:, HID:2 * HID], d_wpk[1])
            wcat = cpool.tile([128, 3 * HID], F32, tag="wcat")
            wgate = cpool.tile([128, 3 * HID], F32, tag="wgate")
            for q in range(3):
                nc.sync.dma_start(wcat[:, q * HID:(q + 1) * HID], d_wcat[q])
                nc.sync.dma_start(wgate[:, q * HID:(q + 1) * HID], d_wgate[q])
            wgraph = load(cpool, d_wgraph[:], [64, HID], "wgraph")
            wb1 = cpool.tile([128, 2 * HID], F32, tag="wb1")
            nc.sync.dma_start(wb1[:, 0:HID], d_wb1[0])
            nc.sync.dma_start(wb1[:, HID:2 * HID], d_wb1[1])
            wb2 = load(cpool, d_wb2[:], [128, 1], "wb2")
            bpk = load_bias(cpool, d_bpk[:], "bpk")
            bcat = load_bias(cpool, d_bcat[:], "bcat")
            bgate = load_bias(cpool, d_bgate[:], "bgate")
            bgateh = load_bias(cpool, d_bgateh[:], "bgateh")
            bgraph = load_bias(cpool, d_bgraph[:], "bgraph")
            bb1 = load_bias(cpool, d_bb1[:], "bb1")
            w96_f = cpool.tile([96, 9 * HID], F32, tag="w96f")
            nc.sync.dma_start(w96_f[:, :].rearrange("p (t o) -> p t o", t=9),
                              d_w96[:, :, :].rearrange("t c o -> c t o"))
            w96 = cast_r(cpool, w96_f, [96, 9 * HID], "w96")
            w0_f = cpool.tile([64, 27 * HID], F32, tag="w0f")
            nc.sync.dma_start(w0_f[:, :].rearrange("p (t o) -> p t o", t=27),
                              d_w0[:, :, :].rearrange("t c o -> c t o"))
            w0 = cast_r(cpool, w0_f, [64, 27 * HID], "w0")

            # conv1 (ms_feat_0) — host sends 3x dx-shifted copies stacked on partitions
            x1f = bigpool.tile([96, 4096], F32, tag="x1f")
            nc.sync.dma_start(x1f[:], d_msf0[:])
            x1t = bigpool.tile([96, 4096], F32, tag="x1t")
            nc.scalar.activation(x1t[:], x1f[:], AF.Tanh, scale=0.5)
            x3 = bigpool.tile([96, 4096], F32R, tag="x3")
            nc.vector.scalar_tensor_tensor(x3[:], x1t[:], 1.0, x1f[:],
                                           op0=ALU.add, op1=ALU.mult)
            x3v = x3[:, :].rearrange("p (z q) -> p z q", z=16)
            x3v = x3v.rearrange("p z (b d) -> p z b d", b=16)  # [96, 16, 16, 16]

            x0f = prepool.tile([64, 512], F32, tag="x0f")
            nc.sync.dma_start(x0f[:], d_msf1[:])
            x0t = prepool.tile([64, 512], F32, tag="x0t")
            nc.scalar.activation(x0t[:], x0f[:], AF.Tanh, scale=0.5)
            x0 = prepool.tile([64, 512], F32R, tag="x0")
            nc.vector.scalar_tensor_tensor(x0[:], x0t[:], 1.0, x0f[:],
                                           op0=ALU.add, op1=ALU.mult)

            pre_tasks = []
            p1parts = prepool.tile([128, 7], F32, tag="p1p")
            def mk_conv1(c):
                def run():
                    ps_c1 = pspre.tile([128, 392], F32, tag="ps")
                    out_ap = ps_c1[:, :].rearrange("p (a b c) -> p a b c", a=2, b=14)
                    for dz in range(3):
                        for dy in range(3):
                            rhs = x3v[:, dz + 2 * c:dz + 2 * c + 2, dy:dy + 14, 0:14]
                            ti = dz * 3 + dy
                            nc.tensor.matmul(out_ap, w96[:, ti * HID:(ti + 1) * HID], rhs,
                                             start=(ti == 0), stop=(ti == 8))
                    junk = jpool.tile([128, 392], F32, tag="junk")
                    nc.vector.tensor_scalar(junk[:], ps_c1[:], 1.0, 0.0, op0=ALU.mult, op1=ALU.add,
                                            accum_out=p1parts[:, c:c + 1])
                return run
            for c in range(7):
                pre_tasks.append(mk_conv1(c))

            def task_conv0():
                ps_c0 = pspre.tile([128, 216], F32, tag="ps")
                out0_ap = ps_c0[:, :].rearrange("p (a b c) -> p a b c", a=6, b=6)
                x0v = x0[:, :].rearrange("p (z q) -> p z q", z=8)
                x0v = x0v.rearrange("p z (b d) -> p z b d", b=8)
                for dz in range(3):
                    for dy in range(3):
                        for dx in range(3):
                            rhs = x0v[:, dz:dz + 6, dy:dy + 6, dx:dx + 6]
                            ti = dz * 9 + dy * 3 + dx
                            nc.tensor.matmul(out0_ap, w0[:, ti * HID:(ti + 1) * HID], rhs,
                                             start=(ti == 0), stop=(ti == 26))
                p0 = prepool.tile([128, 1], F32, tag="p0")
                junk0 = jpool.tile([128, 216], F32, tag="junk")
                nc.vector.tensor_scalar(junk0[:], ps_c0[:], 1.0, 0.0, op0=ALU.mult, op1=ALU.add,
                                        accum_out=p0[:])
                p0m = prepool.tile([128, 1], F32, tag="p0m")
                nc.vector.tensor_scalar_mul(p0m[:], p0[:], 1.0 / 216.0)
                state["p0"] = p0; state["p0m"] = p0m
            pre_tasks.append(task_conv0)

            def task_pocket():
                p0, p0m = state["p0"], state["p0m"]
                p1 = prepool.tile([128, 1], F32, tag="p1")
                junk7 = jpool.tile([128, 7], F32, tag="junk7")
                nc.vector.tensor_scalar(junk7[:], p1parts[:], 1.0, 0.0, op0=ALU.mult, op1=ALU.add,
                                        accum_out=p1[:])
                p1m = prepool.tile([128, 1], F32, tag="p1m")
                nc.vector.tensor_scalar_mul(p1m[:], p1[:], 1.0 / 2744.0)
                tp0 = prepool.tile([128, 1], F32, tag="tp0")
                nc.scalar.activation(tp0[:], p0[:], AF.Tanh, scale=0.5 / 216.0)
                sp0 = prepool.tile([128, 1], F32, tag="sp0")
                nc.vector.scalar_tensor_tensor(sp0[:], tp0[:], 1.0, p0m[:],
                                               op0=ALU.add, op1=ALU.mult)
                tp1 = prepool.tile([128, 1], F32, tag="tp1")
                nc.scalar.activation(tp1[:], p1[:], AF.Tanh, scale=0.5 / 2744.0)
                sp1 = prepool.tile([128, 1], F32, tag="sp1")
                nc.vector.scalar_tensor_tensor(sp1[:], tp1[:], 1.0, p1m[:],
                                               op0=ALU.add, op1=ALU.mult)
                ps_pk = pspre.tile([128, 1], F32, tag="ps")
                nc.tensor.matmul(ps_pk[:], wpk[:, 0:HID], sp0[:], start=True, stop=False)
                nc.tensor.matmul(ps_pk[:], wpk[:, HID:2 * HID], sp1[:], start=False, stop=True)
                pocket = prepool.tile([128, 1], F32, tag="pocket")
                nc.scalar.activation(pocket[:], ps_pk[:], AF.Identity, bias=bpk[:])
                state["pocket"] = pocket
            pre_tasks.append(task_pocket)

            def task_pf():
                pocket = state["pocket"]
                tok_sum = prepool.tile([128, 1], F32, tag="toksum")
                junkt = jpool.tile([128, NT], F32, tag="junk")
                nc.vector.tensor_scalar(junkt[:], tokT[:], 1.0, 0.0, op0=ALU.mult, op1=ALU.add,
                                        accum_out=tok_sum[:])
                ps_pf = pspre.tile([128, 2], F32, tag="ps")
                chunks = [pocket, tok_sum, tok_sum]
                for q in range(3):
                    nc.tensor.matmul(ps_pf[:, 0:1], wcat[:, q * HID:(q + 1) * HID], chunks[q][:],
                                     start=(q == 0), stop=(q == 2))
                for q in range(3):
                    nc.tensor.matmul(ps_pf[:, 1:2], wgate[:, q * HID:(q + 1) * HID], chunks[q][:],
                                     start=(q == 0), stop=(q == 2))
                pf_t = prepool.tile([128, 1], F32, tag="pft")
                nc.scalar.activation(pf_t[:], ps_pf[:, 1:2], AF.Tanh, bias=bgateh[:], scale=0.5)
                pf_sig = prepool.tile([128, 1], F32, tag="pfsig")
                nc.vector.tensor_scalar(pf_sig[:], pf_t[:], 0.5, 0.5, op0=ALU.mult, op1=ALU.add)
                pf_lin = prepool.tile([128, 1], F32, tag="pflin")
                nc.scalar.activation(pf_lin[:], ps_pf[:, 0:1], AF.Identity, bias=bcat[:])
                pf = prepool.tile([128, 1], F32, tag="pf")
                nc.vector.tensor_mul(pf[:], pf_lin[:], pf_sig[:])
                state["pf"] = pf
            pre_tasks.append(task_pf)

            def task_bias():
                pf = state["pf"]
                lg = prepool.tile([64, NG], F32, tag="lg")
                nc.sync.dma_start(lg[:], d_lgT[:])
                ps_gf = pspre.tile([128, NG], F32, tag="ps")
                nc.tensor.matmul(ps_gf[:], wgraph[:], lg[:], start=True, stop=True)
                gfT = prepool.tile([128, NG], F32, tag="gfT")
                nc.scalar.activation(gfT[:], ps_gf[:], AF.Identity, bias=bgraph[:])
                ps_u = pspre.tile([128, 1], F32, tag="ps")
                nc.tensor.matmul(ps_u[:], wb1[:, 0:HID], pf[:], start=True, stop=True)
                ub = prepool.tile([128, 1], F32, tag="ub")
                nc.scalar.activation(ub[:], ps_u[:], AF.Identity, bias=bb1[:])
                ps_hb = pspre.tile([128, NG], F32, tag="ps")
                nc.tensor.matmul(ps_hb[:], wb1[:, HID:2 * HID], gfT[:], start=True, stop=True)
                hb = prepool.tile([128, NG], F32, tag="hb")
                nc.scalar.activation(hb[:], ps_hb[:], AF.Lrelu, bias=ub[:], alpha=0.01)
                ps_b2 = pspre.tile([1, NG], F32, tag="ps")
                nc.tensor.matmul(ps_b2[:], wb2[:], hb[:], start=True, stop=True)
                bias_sb = prepool.tile([1, NG], F32, tag="bias")
                nc.scalar.activation(bias_sb[:], ps_b2[:], AF.Identity, bias=bb2)
                nc.sync.dma_start(d_bias[:], bias_sb[:])
            pre_tasks.append(task_bias)
            state = {}

            # ---------- main loop ----------
            # acc[p, 8a + jt] accumulates pe for atom (128a + p), token group jt
            acc = cpool.tile([128, 32], F32, tag="acc")
            nc.vector.memset(acc[:], 0.0)
            zq4 = None
            for g in range(16):  # 8 tokens per group
                if g % 4 == 0:
                    zq4 = psz.tile([128, 256], F32, tag="z")  # 4 groups per bank
                zq = zq4[:, 64 * (g % 4):64 * (g % 4) + 64]  # col = 16a + 2jt + r
                for u in range(4):  # 2 tokens per u
                    y2 = psy.tile([128, 1024], F32, tag="y")
                    h2 = hpool.tile([128, 1024], F32R, tag="h")
                    for v in range(2):
                        j = 8 * g + 2 * u + v
                        x = xpool.tile([128, NAH], BF16, tag="x")
                        nc.vector.tensor_scalar_mul(x[:], atomsT[:], tokT[:, j:j + 1])
                        nc.tensor.matmul(y2[:, 512 * v:512 * (v + 1)], wint[:], x[:],
                                         start=True, stop=True)
                    if ((4 * g + u) % 6 != 5) or not bint_zero:
                        nc.scalar.activation(h2[:], y2[:], AF.Lrelu, bias=bint[:], alpha=0.01)
                    else:
                        # DVE leaky-relu (valid for b_int == 0): max(y, 0.01*y)
                        hscaled = hpool.tile([128, 1024], F32, tag="hs")
                        nc.vector.tensor_scalar_mul(hscaled[:], y2[:], 0.01)
                        nc.vector.tensor_max(h2[:], y2[:], hscaled[:])
                    for v in range(2):
                        jt = 2 * u + v
                        for a in range(4):
                            nc.tensor.matmul(zq[:, 16 * a + 2 * jt:16 * a + 2 * jt + 2],
                                             h2[:, 512 * v + 128 * a:512 * v + 128 * (a + 1)],
                                             wpeg[:], start=True, stop=True)
                # sigmoid(z1+bpg) = 0.5 + 0.5*tanh((z1+bpg)/2) -- tanh shares the
                # ACT table set with leaky_relu, so no table reloads in the loop
                s = gpool.tile([128, 32], F32, tag="s")
                nc.scalar.activation(s[:], zq[:, 1::2], AF.Tanh, bias=bpg * 0.5, scale=0.5)
                w = gpool.tile([128, 32], F32, tag="w")
                nc.vector.tensor_scalar(w[:], s[:], 0.5, 0.5, op0=ALU.mult, op1=ALU.add)
                t = gpool.tile([128, 32], F32, tag="t")
                nc.vector.scalar_tensor_tensor(t[:], zq[:, 0::2], bpe, w[:],
                                               op0=ALU.add, op1=ALU.mult)
                nc.vector.tensor_add(acc[:], acc[:], t[:])
                if g < len(pre_tasks):
                    pre_tasks[g]()

            # reduce over the 8 token-groups -> atom_e [128, 4] (atom chunks as cols)
            ae4 = prepool.tile([128, 4], F32, tag="ae4")
            junka = jpool.tile([128, 8], F32, tag="junk8")
            for a in range(4):
                junka = jpool.tile([128, 8], F32, tag="junk8")
                nc.vector.tensor_scalar(junka[:], acc[:, 8 * a:8 * (a + 1)], 1.0, 0.0,
                                        op0=ALU.mult, op1=ALU.add, accum_out=ae4[:, a:a + 1])
            ps_seg = pspre.tile([1, NG], F32, tag="ps")
            for q in range(4):
                nc.tensor.matmul(ps_seg[:], ae4[:, q:q + 1], St[:, q * NG:(q + 1) * NG],
                                 start=(q == 0), stop=(q == 3))
            seg_sb = prepool.tile([1, NG], F32, tag="seg")
            nc.scalar.activation(seg_sb[:], ps_seg[:], AF.Copy)
            nc.sync.dma_start(d_seg[:], seg_sb[:])


    _legalize_waits(nc)
    nc._tile_ctx = tc_ref
    return nc


def kernel(**inputs) -> np.ndarray:
    f = lambda a: np.ascontiguousarray(np.asarray(a), dtype=np.float32)
    tf = f(inputs["token_features"])
    la = f(inputs["lig_atom"])
    lg = f(inputs["lig_graph"])
    m0 = f(inputs["ms_feat_0"])
    m1 = f(inputs["ms_feat_1"])
    lb = np.asarray(inputs["ligand_batch"])
    S = (lb[:, None] == np.arange(NG)[None, :]).astype(np.float32)

    Wc1 = f(inputs["Wc1"])
    Wc0 = f(inputs["Wc0"])
    W96 = np.ascontiguousarray(Wc1.transpose(2, 3, 4, 1, 0).reshape(9, 96, HID))
    W0t = np.ascontiguousarray(Wc0.transpose(2, 3, 4, 1, 0).reshape(27, 64, HID))
    wcat = f(inputs["W_cat"]).copy()
    wgate = f(inputs["W_gate"]).copy()
    wcat[2 * HID:] /= float(NT)
    wgate[2 * HID:] /= float(NT)
    wpeg = np.concatenate([f(inputs["W_pe"]), f(inputs["W_pg"])], axis=1)

    bpe = float(np.asarray(inputs["b_pe"]).reshape(-1)[0])
    bpg = float(np.asarray(inputs["b_pg"]).reshape(-1)[0])
    bb2 = float(np.asarray(inputs["b_bias2"]).reshape(-1)[0])

    col = lambda a: f(a).reshape(128, 1)
    shared = {
        "W_token": (f(inputs["W_token"]) * 0.5).reshape(2, 128, HID),
        "W96": W96 * 0.5, "W0t": W0t * 0.5,
        "W_pocket": (f(inputs["W_pocket"]) * 0.5).reshape(2, 128, HID),
        "W_cat": wcat.reshape(3, 128, HID),
        "W_gate": wgate.reshape(3, 128, HID),
        "W_atom": f(inputs["W_atom"]),
        "W_graph": f(inputs["W_graph"]),
        "W_bias1": f(inputs["W_bias1"]).reshape(2, 128, HID),
        "W_bias2": f(inputs["W_bias2"]),
        "W_int": f(inputs["W_int"]).astype(ml_dtypes.bfloat16),
        "W_peg": wpeg,
        "b_token": col(inputs["b_token"]), "b_pocket": col(inputs["b_pocket"]),
        "b_cat": col(inputs["b_cat"]), "b_gate": col(inputs["b_gate"]),
        "b_atom": col(inputs["b_atom"]), "b_graph": col(inputs["b_graph"]),
        "b_bias1": col(inputs["b_bias1"]), "b_int": col(inputs["b_int"]),
        "b_gate_h": col(inputs["b_gate"]) * 0.5,
    }

    in_maps = []
    for c in range(NCORES):
        n, h = c // 2, c % 2
        m = dict(shared)
        m["tfT"] = np.ascontiguousarray(tf[n].T.reshape(2, 128, 128))
        m["laT"] = np.ascontiguousarray(la[n, 512 * h:512 * (h + 1)].T)
        m["lgT"] = np.ascontiguousarray(lg[n].T)
        m0f = m0[n].reshape(32, 4096)
        x3h = np.zeros((96, 4096), dtype=np.float32)
        for dd in range(3):
            x3h[32 * dd:32 * (dd + 1), 0:4096 - dd] = m0f[:, dd:]
        m["msf0"] = x3h
        m["msf1"] = m1[n].reshape(64, 512)
        m["Sh"] = np.ascontiguousarray(S[512 * h:512 * (h + 1)].reshape(4, 128, NG))
        in_maps.append(m)

    bint_zero = bool(np.all(np.asarray(inputs['b_int']) == 0.0))
    nc = build_program(bpe, bpg, bb2, bint_zero)
    r = run_bass_kernel_spmd(nc, in_maps, core_ids=list(range(NCORES)),
                             trace=TRACE, **(TRACE_KW if TRACE else {}))
    global LAST
    LAST = r
    res = r.results

    out = np.zeros((NI, NG), dtype=np.float32)
    for n in range(NI):
        out[n] = (res[2 * n]["seg_out"][0] + res[2 * n + 1]["seg_out"][0]
                  + res[2 * n]["bias_out"][0])
    return out



# revision 5
# speedup vs baseline: 1.9723x; 1.9723x over previous
import sys
import numpy as np
import ml_dtypes

sys.path.insert(0, "/opt/trn_rl_repo")

import concourse.bass as bass
import concourse.tile as tile
from concourse import mybir
from concourse.bass_utils import run_bass_kernel_spmd

F32 = mybir.dt.float32
BF16 = mybir.dt.bfloat16
AF = mybir.ActivationFunctionType
ALU = mybir.AluOpType

HID = 128
NT = 128       # tokens per image
NAH = 512      # atoms per core (half of 1024)
NG = 64        # ligand graphs
NI = 4         # images
NCORES = 8

# WB (128-partition weight concat, bf16) column offsets
OFF_WINT = 0
OFF_WTOK = 128
OFF_WPK = 384
OFF_WCAT = 640
OFF_WGATE = 1024
OFF_WB1 = 1408
OFF_WB2 = 1664
OFF_WPEG = 1665
OFF_UPEG = 1667
NWB = 1669

# BI (f32 bias concat) columns
BI_TOK, BI_ATOM, BI_INT, BI_PK, BI_CAT, BI_GH, BI_GR, BI_B1, BI_C0, BI_C1 = range(10)
NBI = 10

# lrelu unit assignment: 'A' = ACT Prelu, 'B' = DVE relu99 + linear-fold
N_A_UNITS = 33

TRACE = False
TRACE_KW = {}
LAST = None


_COMPUTE_INSTS = (
    "InstActivation", "InstTensorCopy", "InstTensorScalar", "InstTensorScalarPtr",
    "InstTensorTensor", "InstTensorTensorReduce", "InstTensorReduce", "InstMemSet",
    "InstMatmult", "InstScalarTensorTensor", "InstTensorTensorScan", "InstLdweights",
    "InstDMACopy", "InstDMATransposeAnt", "InstTriggeredCopy", "InstDrain",
    "InstEventSemaphoreOp", "InstSemaphoreOp", "InstCopy", "InstIota", "InstSelect",
)


def _legalize_waits(nc):
    # walrus in this toolchain accepts at most ONE sync wait on TPB compute
    # instructions; hoist extras into same-engine NoOps placed just before.
    k = 0
    for f in nc.m.functions:
        for blk in f.blocks:
            insts = blk.instructions
            out = []
            for ins in insts:
                si = getattr(ins, "sync_info", None)
                if (si is not None and len(si.on_wait) > 1
                        and type(ins).__name__ in _COMPUTE_INSTS):
                    waits = list(si.on_wait)
                    for w in waits[:-1]:
                        nop = mybir.InstNoOp(
                            name=f"WNOP-{k}", engine=ins.engine,
                            sync_info=mybir.SyncInfo(on_wait=[w], on_update=[]))
                        k += 1
                        out.append(nop)
                    ins.sync_info = mybir.SyncInfo(on_wait=[waits[-1]],
                                                   on_update=list(si.on_update))
                out.append(ins)
            blk.instructions = out
    return k


def _register_const(nc, val, dtype=F32):
    if (dtype, float(val)) in nc.const_aps.aps:
        return
    t = nc.alloc_sbuf_tensor(f"uconst-{dtype.name}-{val}", [128, 1], dtype)
    nc.gpsimd.memset(t.ap(), float(val))
    nc.const_aps.aps[(dtype, float(val))] = t.ap()


def _unit_engines():
    # interleave N_A_UNITS 'A' units among 64 as evenly as possible
    eng = []
    for u in range(64):
        if (u + 1) * N_A_UNITS // 64 > u * N_A_UNITS // 64:
            eng.append('A')
        else:
            eng.append('B')
    return eng


def build_program(bpe: float, bpg: float, bb2: float, bint_zero: bool = True,
                  sim_trace: bool = False) -> bass.Bass:
    nc = bass.Bass()
    _register_const(nc, 0.5 * bpg)
    _register_const(nc, bb2)
    nc.all_engine_barrier()

    # ---- DRAM inputs (per-core views; same names across SPMD cores) ----
    d_WB = nc.dram_tensor("WB", [128, NWB], BF16, kind="ExternalInput")
    d_BI = nc.dram_tensor("BI", [128, NBI], F32, kind="ExternalInput")
    d_tfT = nc.dram_tensor("tfT", [2, 128, 128], F32, kind="ExternalInput")
    d_laT = nc.dram_tensor("laT", [64, NAH], BF16, kind="ExternalInput")
    d_W64 = nc.dram_tensor("W64", [64, 256], BF16, kind="ExternalInput")
    d_m0T = nc.dram_tensor("m0T", [32, 128, 32], BF16, kind="ExternalInput")
    d_M1 = nc.dram_tensor("M1m", [32, 128, 27], BF16, kind="ExternalInput")
    d_m1T = nc.dram_tensor("m1T", [4, 128, 64], BF16, kind="ExternalInput")
    d_M0 = nc.dram_tensor("M0m", [4, 128, 27], BF16, kind="ExternalInput")
    d_W0T = nc.dram_tensor("W0T", [64, 27 * 128], BF16, kind="ExternalInput")
    d_W32 = nc.dram_tensor("W32", [32, 27 * 128], BF16, kind="ExternalInput")
    d_lgT = nc.dram_tensor("lgT", [64, NG], BF16, kind="ExternalInput")
    d_Sh = nc.dram_tensor("Sh", [4, 128, NG], BF16, kind="ExternalInput")

    d_res = nc.dram_tensor("res_out", [1, 128], F32, kind="ExternalOutput")

    ENG = _unit_engines()
    if not bint_zero:
        ENG[:] = ['A'] * 64

    tc_ref = tile.TileContext(nc, trace_sim=sim_trace)
    with tc_ref as tc:
        with (
            tc.tile_pool(name="const", bufs=1) as cpool,
            tc.tile_pool(name="pre", bufs=1) as prepool,
            tc.tile_pool(name="x", bufs=6) as xpool,
            tc.tile_pool(name="u", bufs=4) as upool,
            tc.tile_pool(name="h", bufs=4) as hpool,
            tc.tile_pool(name="g", bufs=2) as gpool,
            tc.tile_pool(name="j", bufs=2) as jpool,
            tc.tile_pool(name="ps_y", bufs=3, space="PSUM") as psy,
            tc.tile_pool(name="ps_z", bufs=1, space="PSUM") as psz,
            tc.tile_pool(name="ps_p", bufs=1, space="PSUM") as pspre,
        ):
            # ---------- input DMAs (order = DMA device service priority) ----
            WBsb = cpool.tile([128, NWB], BF16, tag="WB")
            nc.sync.dma_start(WBsb[:], d_WB[:])
            BIsb = cpool.tile([128, NBI], F32, tag="BI")
            nc.sync.dma_start(BIsb[:], d_BI[:])
            tf = prepool.tile([128, 256], F32, tag="tf")
            nc.sync.dma_start(tf[:, :].rearrange("p (c j) -> p c j", c=2),
                              d_tfT[:, :, :].rearrange("c p j -> p c j"))
            la = prepool.tile([64, NAH], BF16, tag="la")
            nc.sync.dma_start(la[:], d_laT[:])
            W64sb = cpool.tile([64, 256], BF16, tag="W64")
            nc.sync.dma_start(W64sb[:], d_W64[:])
            m0sb = cpool.tile([128, 1024], BF16, tag="m0")
            nc.sync.dma_start(m0sb[:, :].rearrange("p (u c) -> p u c", u=32),
                              d_m0T[:, :, :].rearrange("u p c -> p u c"))
            M1sb = cpool.tile([128, 864], BF16, tag="M1")
            nc.sync.dma_start(M1sb[:, :].rearrange("p (u o) -> p u o", u=32),
                              d_M1[:, :, :].rearrange("u p o -> p u o"))
            m1sb = cpool.tile([128, 256], BF16, tag="m1")
            nc.sync.dma_start(m1sb[:, :].rearrange("p (u c) -> p u c", u=4),
                              d_m1T[:, :, :].rearrange("u p c -> p u c"))
            M0sb = cpool.tile([128, 108], BF16, tag="M0")
            nc.sync.dma_start(M0sb[:, :].rearrange("p (u o) -> p u o", u=4),
                              d_M0[:, :, :].rearrange("u p o -> p u o"))
            W0Tsb = cpool.tile([64, 27 * 128], BF16, tag="W0T")
            nc.sync.dma_start(W0Tsb[:], d_W0T[:])
            W32sb = cpool.tile([32, 27 * 128], BF16, tag="W32")
            nc.sync.dma_start(W32sb[:], d_W32[:])
            lg = cpool.tile([64, NG], BF16, tag="lg")
            nc.sync.dma_start(lg[:], d_lgT[:])
            Stsb = cpool.tile([128, 4 * NG], BF16, tag="St")
            nc.sync.dma_start(Stsb[:, :].rearrange("p (q g) -> p q g", q=4),
                              d_Sh[:, :, :].rearrange("q p g -> p q g"))

            bias = lambda i: BIsb[:, i:i + 1]

            # ---------- preamble: tok / atoms (needed before main loop) -----
            tfr = prepool.tile([128, 256], BF16, tag="tfr")
            nc.scalar.activation(tfr[:], tf[:], AF.Silu)
            ps_tok = pspre.tile([128, 128], F32, tag="pre")
            nc.tensor.matmul(ps_tok[:], WBsb[:, OFF_WTOK:OFF_WTOK + 128],
                             tfr[:, 0:128], start=True, stop=False)
            nc.tensor.matmul(ps_tok[:], WBsb[:, OFF_WTOK + 128:OFF_WTOK + 256],
                             tfr[:, 128:256], start=False, stop=True)
            tokT = cpool.tile([128, NT], F32, tag="tokT")
            nc.scalar.activation(tokT[:], ps_tok[:], AF.Identity, bias=bias(BI_TOK))

            ps_at = pspre.tile([128, NAH], F32, tag="pre")
            nc.tensor.matmul(ps_at[:], W64sb[:, 0:128], la[:], start=True, stop=True)
            atomsT = cpool.tile([128, NAH], BF16, tag="atomsT")
            nc.scalar.activation(atomsT[:], ps_at[:], AF.Identity, bias=bias(BI_ATOM))

            # ---------- deferred preamble tasks (interleaved into loop) ----
            state = {}

            def task_silu1():
                s0 = cpool.tile([128, 1024], BF16, tag="s0")
                nc.scalar.activation(s0[:], m0sb[:], AF.Silu)
                state["s0"] = s0

            def task_S1():
                S1 = pspre.tile([32, 27], F32, tag="pre")
                for u in range(32):
                    nc.tensor.matmul(S1[:], state["s0"][:, 32 * u:32 * u + 32],
                                     M1sb[:, 27 * u:27 * u + 27],
                                     start=(u == 0), stop=(u == 31))
                S1b = prepool.tile([32, 27], BF16, tag="S1b")
                nc.scalar.activation(S1b[:], S1[:], AF.Copy)
                state["S1b"] = S1b

            def task_p1():
                pp = pspre.tile([128, 1], F32, tag="pre")
                for o in range(27):
                    nc.tensor.matmul(pp[:], W32sb[:, 128 * o:128 * o + 128],
                                     state["S1b"][:, o:o + 1],
                                     start=(o == 0), stop=(o == 26))
                sp1 = prepool.tile([128, 1], BF16, tag="sp1")
                nc.scalar.activation(sp1[:], pp[:], AF.Silu, bias=bias(BI_C1))
                state["sp1"] = sp1

            def task_silu0():
                s1 = prepool.tile([128, 256], BF16, tag="s1")
                nc.scalar.activation(s1[:], m1sb[:], AF.Silu)
                state["s1"] = s1

            def task_S0():
                S0 = pspre.tile([64, 27], F32, tag="pre")
                for u in range(4):
                    nc.tensor.matmul(S0[:], state["s1"][:, 64 * u:64 * u + 64],
                                     M0sb[:, 27 * u:27 * u + 27],
                                     start=(u == 0), stop=(u == 3))
                S0b = prepool.tile([64, 27], BF16, tag="S0b")
                nc.scalar.activation(S0b[:], S0[:], AF.Copy)
                state["S0b"] = S0b

            def task_p0():
                pp = pspre.tile([128, 1], F32, tag="pre")
                for o in range(27):
                    nc.tensor.matmul(pp[:], W0Tsb[:, 128 * o:128 * o + 128],
                                     state["S0b"][:, o:o + 1],
                                     start=(o == 0), stop=(o == 26))
                sp0 = prepool.tile([128, 1], BF16, tag="sp0")
                nc.scalar.activation(sp0[:], pp[:], AF.Silu, bias=bias(BI_C0))
                state["sp0"] = sp0

            def task_pocket():
                ps_pk = pspre.tile([128, 1], F32, tag="pre")
                nc.tensor.matmul(ps_pk[:], WBsb[:, OFF_WPK:OFF_WPK + 128],
                                 state["sp0"][:], start=True, stop=False)
                nc.tensor.matmul(ps_pk[:], WBsb[:, OFF_WPK + 128:OFF_WPK + 256],
                                 state["sp1"][:], start=False, stop=True)
                pocket = prepool.tile([128, 1], BF16, tag="pocket")
                nc.scalar.activation(pocket[:], ps_pk[:], AF.Identity, bias=bias(BI_PK))
                state["pocket"] = pocket

            def task_pf():
                tok_sum = prepool.tile([128, 1], F32, tag="toksum")
                junkt = jpool.tile([128, NT], F32, tag="junk")
                nc.vector.tensor_scalar(junkt[:], tokT[:], 1.0, 0.0, op0=ALU.mult,
                                        op1=ALU.add, accum_out=tok_sum[:])
                tok_sum_b = prepool.tile([128, 1], BF16, tag="toksumb")
                nc.scalar.activation(tok_sum_b[:], tok_sum[:], AF.Copy)
                ps_pf = pspre.tile([128, 2], F32, tag="pre")
                chunks = [state["pocket"], tok_sum_b, tok_sum_b]
                for q in range(3):
                    nc.tensor.matmul(ps_pf[:, 0:1],
                                     WBsb[:, OFF_WCAT + 128 * q:OFF_WCAT + 128 * (q + 1)],
                                     chunks[q][:], start=(q == 0), stop=(q == 2))
                for q in range(3):
                    nc.tensor.matmul(ps_pf[:, 1:2],
                                     WBsb[:, OFF_WGATE + 128 * q:OFF_WGATE + 128 * (q + 1)],
                                     chunks[q][:], start=(q == 0), stop=(q == 2))
                # sigmoid(z + bg) = 0.5 + 0.5*tanh(0.5z + 0.5bg)
                gt = prepool.tile([128, 1], F32, tag="gt")
                nc.scalar.activation(gt[:], ps_pf[:, 1:2], AF.Tanh,
                                     bias=bias(BI_GH), scale=0.5)
                pf_sig = prepool.tile([128, 1], F32, tag="pfsig")
                nc.gpsimd.tensor_scalar(pf_sig[:], gt[:], 0.5, 0.5, op0=ALU.mult, op1=ALU.add)
                pf_lin = prepool.tile([128, 1], F32, tag="pflin")
                nc.scalar.activation(pf_lin[:], ps_pf[:, 0:1], AF.Identity, bias=bias(BI_CAT))
                pf = prepool.tile([128, 1], BF16, tag="pf")
                nc.gpsimd.tensor_tensor(pf[:], pf_lin[:], pf_sig[:], op=ALU.mult)
                state["pf"] = pf

            def task_gf():
                ps_gf = pspre.tile([128, NG], F32, tag="pre")
                nc.tensor.matmul(ps_gf[:], W64sb[:, 128:256], lg[:], start=True, stop=True)
                gfT = prepool.tile([128, NG], BF16, tag="gfT")
                nc.scalar.activation(gfT[:], ps_gf[:], AF.Identity, bias=bias(BI_GR))
                state["gfT"] = gfT

            def task_bias1():
                ps_u = pspre.tile([128, 1], F32, tag="pre")
                nc.tensor.matmul(ps_u[:], WBsb[:, OFF_WB1:OFF_WB1 + 128],
                                 state["pf"][:], start=True, stop=True)
                ub = prepool.tile([128, 1], F32, tag="ub")
                nc.scalar.activation(ub[:], ps_u[:], AF.Identity, bias=bias(BI_B1))
                ps_hb = pspre.tile([128, NG], F32, tag="pre")
                nc.tensor.matmul(ps_hb[:], WBsb[:, OFF_WB1 + 128:OFF_WB1 + 256],
                                 state["gfT"][:], start=True, stop=True)
                hb = prepool.tile([128, NG], BF16, tag="hb")
                nc.scalar.activation(hb[:], ps_hb[:], AF.Prelu, bias=ub[:], alpha=0.01)
                state["hb"] = hb

            def task_bias2():
                ps_b2 = pspre.tile([1, NG], F32, tag="pre")
                nc.tensor.matmul(ps_b2[:], WBsb[:, OFF_WB2:OFF_WB2 + 1],
                                 state["hb"][:], start=True, stop=True)
                nc.scalar.activation(res[:, NG:2 * NG], ps_b2[:], AF.Identity, bias=bb2)

            pre_tasks = [task_silu1, task_S1, task_p1, task_silu0, task_S0,
                         task_p0, task_pocket, task_pf, task_gf, task_bias1,
                         task_bias2]
            TASK_AT = {12 + 5 * i: t for i, t in enumerate(pre_tasks)}

            res = cpool.tile([1, 128], F32, tag="res")

            # ---------- main loop ----------
            # 64 units u of 2 tokens; y2[o, 512v + a] for token j = 2u+v.
            # zq8 (per 64-token block) col layout: 8*(j%64) + 2*a_chunk + {pe,pg}
            wpeg_ap = WBsb[:, OFF_WPEG:OFF_WPEG + 2]
            upeg_ap = WBsb[:, OFF_UPEG:OFF_UPEG + 2]
            wint_ap = WBsb[:, OFF_WINT:OFF_WINT + 128]
            zq_tiles = [None, None]
            ae_parts = cpool.tile([128, 8], F32, tag="aeparts")
            pending = []

            def emit_unit(u):
                y2 = psy.tile([128, 1024], F32, tag="y")
                ujs = []
                for v in range(2):
                    j = 2 * u + v
                    Wj = xpool.tile([128, 128], BF16, tag="x")
                    nc.gpsimd.tensor_scalar_mul(Wj[:], wint_ap, tokT[:, j:j + 1])
                    nc.tensor.matmul(y2[:, 512 * v:512 * (v + 1)], Wj[:], atomsT[:],
                                     start=True, stop=True)
                    if ENG[u] == 'B':
                        uj = upool.tile([128, 2], BF16, tag="u")
                        nc.gpsimd.tensor_scalar_mul(uj[:], upeg_ap, tokT[:, j:j + 1])
                        ujs.append(uj)
                return (u, y2, ujs)

            def flush_unit(ent):
                u, y2, ujs = ent
                h = hpool.tile([128, 1024], BF16, tag="h")
                if ENG[u] == 'A':
                    nc.scalar.activation(h[:], y2[:], AF.Prelu, bias=bias(BI_INT),
                                         alpha=0.01)
                else:
                    # h = 0.99*relu(y); the 0.01*y linear part of lrelu is
                    # folded into the zq accumulation via upeg below
                    nc.vector.tensor_scalar(h[:], y2[:], 0.0, 0.99,
                                            op0=ALU.max, op1=ALU.mult)
                for v in range(2):
                    j = 2 * u + v
                    b, jj = j // 64, j % 64
                    if zq_tiles[b] is None:
                        zq_tiles[b] = psz.tile([128, 512], F32, tag="z", name=f"zq{b}")
                    zq = zq_tiles[b]
                    for a in range(4):
                        cols = zq[:, 8 * jj + 2 * a:8 * jj + 2 * a + 2]
                        if ENG[u] == 'A':
                            nc.tensor.matmul(cols, h[:, 512 * v + 128 * a:512 * v + 128 * (a + 1)],
                                             wpeg_ap, start=True, stop=True)
                        else:
                            nc.tensor.matmul(cols, h[:, 512 * v + 128 * a:512 * v + 128 * (a + 1)],
                                             wpeg_ap, start=True, stop=False)
                            nc.tensor.matmul(cols, atomsT[:, 128 * a:128 * (a + 1)],
                                             ujs[v][:], start=False, stop=True)

            def gates(b):
                zq = zq_tiles[b]
                s = gpool.tile([128, 256], F32, tag="s")
                nc.scalar.activation(s[:], zq[:, 1::2], AF.Tanh, bias=0.5 * bpg, scale=0.5)
                w = gpool.tile([128, 256], F32, tag="w")
                nc.gpsimd.tensor_scalar(w[:], s[:], 0.5, 0.5, op0=ALU.mult, op1=ALU.add)
                t = gpool.tile([128, 256], F32, tag="t")
                nc.vector.scalar_tensor_tensor(t[:], zq[:, 0::2], bpe, w[:],
                                               op0=ALU.add, op1=ALU.mult)
                for a in range(4):
                    junka = jpool.tile([128, 64], F32, tag="junka")
                    nc.vector.tensor_scalar(junka[:], t[:, a::4], 1.0, 0.0,
                                            op0=ALU.mult, op1=ALU.add,
                                            accum_out=ae_parts[:, 4 * b + a:4 * b + a + 1])

            for u in range(64):
                pending.append(emit_unit(u))
                if len(pending) > 1:
                    flush_unit(pending.pop(0))
                fu = u - 1  # unit just flushed
                if fu == 31:
                    gates(0)
                if fu in TASK_AT:
                    TASK_AT[fu]()

            flush_unit(pending.pop(0))
            gates(1)

            # atom_e reduce -> seg matmul -> out
            ae4 = prepool.tile([128, 4], F32, tag="ae4")
            nc.vector.tensor_tensor(ae4[:], ae_parts[:, 0:4], ae_parts[:, 4:8], op=ALU.add)
            ae4b = prepool.tile([128, 4], BF16, tag="ae4b")
            nc.scalar.activation(ae4b[:], ae4[:], AF.Copy)
            ps_seg = pspre.tile([1, NG], F32, tag="pre")
            for q in range(4):
                nc.tensor.matmul(ps_seg[:], ae4b[:, q:q + 1], Stsb[:, q * NG:(q + 1) * NG],
                                 start=(q == 0), stop=(q == 3))
            nc.scalar.activation(res[:, 0:NG], ps_seg[:], AF.Copy)
            nc.sync.dma_start(d_res[:], res[:])

    _legalize_waits(nc)
    nc._tile_ctx = tc_ref
    return nc


def kernel(**inputs) -> np.ndarray:
    f = lambda a: np.ascontiguousarray(np.asarray(a), dtype=np.float32)
    bf = lambda a: np.ascontiguousarray(np.asarray(a, dtype=np.float32)).astype(ml_dtypes.bfloat16)
    tf = f(inputs["token_features"])
    la = f(inputs["lig_atom"])
    lgr = f(inputs["lig_graph"])
    m0 = f(inputs["ms_feat_0"])
    m1 = f(inputs["ms_feat_1"])
    lb = np.asarray(inputs["ligand_batch"])
    S = (lb[:, None] == np.arange(NG)[None, :]).astype(np.float32)

    # ---- weight prep (host-side layout/scale transforms only) ----
    wint_bf = bf(inputs["W_int"])                       # [128,128]
    wpe = f(inputs["W_pe"]); wpg = f(inputs["W_pg"])    # [128,1]
    wpeg = np.concatenate([wpe, wpg], axis=1)           # [128,2]
    u_pe = wint_bf.astype(np.float64) @ wpe.astype(np.float64)
    u_pg = wint_bf.astype(np.float64) @ wpg.astype(np.float64)
    upeg = 0.01 * np.concatenate([u_pe, u_pg], axis=1)  # [128,2]

    wcat = f(inputs["W_cat"]).copy()                    # [384,128]
    wgate = f(inputs["W_gate"]).copy()
    wcat[2 * HID:] /= float(NT)
    wgate[2 * HID:] /= float(NT)

    WB = np.zeros((128, NWB), dtype=np.float32)
    WB[:, OFF_WINT:OFF_WINT + 128] = wint_bf.astype(np.float32)
    WB[:, OFF_WTOK:OFF_WTOK + 256] = f(inputs["W_token"]).reshape(2, 128, HID).transpose(1, 0, 2).reshape(128, 256)
    WB[:, OFF_WPK:OFF_WPK + 256] = f(inputs["W_pocket"]).reshape(2, 128, HID).transpose(1, 0, 2).reshape(128, 256)
    WB[:, OFF_WCAT:OFF_WCAT + 384] = wcat.reshape(3, 128, HID).transpose(1, 0, 2).reshape(128, 384)
    WB[:, OFF_WGATE:OFF_WGATE + 384] = wgate.reshape(3, 128, HID).transpose(1, 0, 2).reshape(128, 384)
    WB[:, OFF_WB1:OFF_WB1 + 256] = f(inputs["W_bias1"]).reshape(2, 128, HID).transpose(1, 0, 2).reshape(128, 256)
    WB[:, OFF_WB2:OFF_WB2 + 1] = f(inputs["W_bias2"])
    WB[:, OFF_WPEG:OFF_WPEG + 2] = wpeg
    WB[:, OFF_UPEG:OFF_UPEG + 2] = upeg
    WB_bf = WB.astype(ml_dtypes.bfloat16)

    # conv weights as [c, off*128 + o], scaled by 1/num_output_positions
    Wc0 = f(inputs["Wc0"])  # [128,64,3,3,3] applied to ms_feat_1
    Wc1 = f(inputs["Wc1"])  # [128,32,3,3,3] applied to ms_feat_0
    W0T = np.ascontiguousarray(Wc0.reshape(128, 64, 27).transpose(1, 2, 0)).reshape(64, 27 * 128) / 216.0
    W32 = np.ascontiguousarray(Wc1.reshape(128, 32, 27).transpose(1, 2, 0)).reshape(32, 27 * 128) / 2744.0

    W64 = np.zeros((64, 256), dtype=np.float32)
    W64[:, 0:128] = f(inputs["W_atom"])
    W64[:, 128:256] = f(inputs["W_graph"])

    col = lambda a: f(a).reshape(128, 1)
    BI = np.zeros((128, NBI), dtype=np.float32)
    BI[:, BI_TOK] = f(inputs["b_token"])
    BI[:, BI_ATOM] = f(inputs["b_atom"])
    BI[:, BI_INT] = f(inputs["b_int"])
    BI[:, BI_PK] = f(inputs["b_pocket"])
    BI[:, BI_CAT] = f(inputs["b_cat"])
    BI[:, BI_GH] = 0.5 * f(inputs["b_gate"])
    BI[:, BI_GR] = f(inputs["b_graph"])
    BI[:, BI_B1] = f(inputs["b_bias1"])
    BI[:, BI_C0] = f(inputs["bc0"])
    BI[:, BI_C1] = f(inputs["bc1"])

    # window-membership masks: M[pos, off] = 1 iff pos-off in valid out range
    def win_mask(D, O):
        g = np.arange(D)
        z, y, x = np.meshgrid(g, g, g, indexing="ij")
        pos = np.stack([z.ravel(), y.ravel(), x.ravel()], 1)  # [D^3, 3]
        d = np.arange(3)
        dz, dy, dx = np.meshgrid(d, d, d, indexing="ij")
        off = np.stack([dz.ravel(), dy.ravel(), dx.ravel()], 1)  # [27, 3]
        r = pos[:, None, :] - off[None, :, :]
        return np.all((r >= 0) & (r < O), axis=2).astype(np.float32)  # [D^3, 27]

    M1 = win_mask(16, 14).reshape(32, 128, 27)
    M0 = win_mask(8, 6).reshape(4, 128, 27)

    bpe = float(np.asarray(inputs["b_pe"]).reshape(-1)[0])
    bpg = float(np.asarray(inputs["b_pg"]).reshape(-1)[0])
    bb2 = float(np.asarray(inputs["b_bias2"]).reshape(-1)[0])

    shared = {
        "WB": WB_bf, "BI": BI,
        "W64": W64.astype(ml_dtypes.bfloat16),
        "W0T": W0T.astype(ml_dtypes.bfloat16),
        "W32": W32.astype(ml_dtypes.bfloat16),
        "M1m": M1.astype(ml_dtypes.bfloat16),
        "M0m": M0.astype(ml_dtypes.bfloat16),
    }

    in_maps = []
    for c in range(NCORES):
        n, h = c // 2, c % 2
        m = dict(shared)
        m["tfT"] = np.ascontiguousarray(tf[n].T.reshape(2, 128, 128))
        m["laT"] = bf(la[n, 512 * h:512 * (h + 1)].T)
        m["lgT"] = bf(lgr[n].T)
        m["m0T"] = bf(m0[n].reshape(32, 4096).T.reshape(32, 128, 32))
        m["m1T"] = bf(m1[n].reshape(64, 512).T.reshape(4, 128, 64))
        m["Sh"] = bf(S[512 * h:512 * (h + 1)].reshape(4, 128, NG))
        in_maps.append(m)

    bint_zero = bool(np.all(np.asarray(inputs["b_int"]) == 0.0))
    nc = build_program(bpe, bpg, bb2, bint_zero)
    r = run_bass_kernel_spmd(nc, in_maps, core_ids=list(range(NCORES)),
                             trace=TRACE, **(TRACE_KW if TRACE else {}))
    global LAST
    LAST = r
    res = r.results

    out = np.zeros((NI, NG), dtype=np.float32)
    for n in range(NI):
        out[n] = (res[2 * n]["res_out"][0, 0:NG] + res[2 * n + 1]["res_out"][0, 0:NG]
                  + res[2 * n]["res_out"][0, NG:2 * NG])
    return out


# revision 7
# speedup vs baseline: 2.0527x; 1.0408x over previous
import sys
import numpy as np
import ml_dtypes

sys.path.insert(0, "/opt/trn_rl_repo")

import concourse.bass as bass
import concourse.tile as tile
from concourse import mybir
from concourse.bass_utils import run_bass_kernel_spmd

F32 = mybir.dt.float32
BF16 = mybir.dt.bfloat16
AF = mybir.ActivationFunctionType
ALU = mybir.AluOpType

HID = 128
NT = 128       # tokens per image
NAH = 512      # atoms per core (half of 1024)
NG = 64        # ligand graphs
NI = 4         # images
NCORES = 8

# WB (128-partition weight concat, bf16) column offsets
OFF_WINT = 0
OFF_WTOK = 128
OFF_WPK = 384
OFF_WCAT = 640
OFF_WGATE = 1024
OFF_WB1 = 1408
OFF_WB2 = 1664
OFF_WPEG = 1665
OFF_UPEG = 1667
NWB = 1669

# BI (f32 bias concat) columns
BI_TOK, BI_ATOM, BI_INT, BI_PK, BI_CAT, BI_GH, BI_GR, BI_B1, BI_C0, BI_C1 = range(10)
BI_WPEG = 10   # cols 10:12 = [W_pe, W_pg] f32
NBI = 12

# lrelu unit assignment: 'A' = ACT Prelu, 'B' = DVE relu99 + linear-fold
N_A_UNITS = 33

TRACE = False
TRACE_KW = {}
LAST = None


_COMPUTE_INSTS = (
    "InstActivation", "InstTensorCopy", "InstTensorScalar", "InstTensorScalarPtr",
    "InstTensorTensor", "InstTensorTensorReduce", "InstTensorReduce", "InstMemSet",
    "InstMatmult", "InstScalarTensorTensor", "InstTensorTensorScan", "InstLdweights",
    "InstDMACopy", "InstDMATransposeAnt", "InstTriggeredCopy", "InstDrain",
    "InstEventSemaphoreOp", "InstSemaphoreOp", "InstCopy", "InstIota", "InstSelect",
)


def _legalize_waits(nc):
    # walrus in this toolchain accepts at most ONE sync wait on TPB compute
    # instructions; hoist extras into same-engine NoOps placed just before.
    k = 0
    for f in nc.m.functions:
        for blk in f.blocks:
            insts = blk.instructions
            out = []
            for ins in insts:
                si = getattr(ins, "sync_info", None)
                if (si is not None and len(si.on_wait) > 1
                        and type(ins).__name__ in _COMPUTE_INSTS):
                    waits = list(si.on_wait)
                    for w in waits[:-1]:
                        nop = mybir.InstNoOp(
                            name=f"WNOP-{k}", engine=ins.engine,
                            sync_info=mybir.SyncInfo(on_wait=[w], on_update=[]))
                        k += 1
                        out.append(nop)
                    ins.sync_info = mybir.SyncInfo(on_wait=[waits[-1]],
                                                   on_update=list(si.on_update))
                out.append(ins)
            blk.instructions = out
    return k


def _register_const(nc, val, dtype=F32):
    if (dtype, float(val)) in nc.const_aps.aps:
        return
    t = nc.alloc_sbuf_tensor(f"uconst-{dtype.name}-{val}", [128, 1], dtype)
    nc.gpsimd.memset(t.ap(), float(val))
    nc.const_aps.aps[(dtype, float(val))] = t.ap()


def _unit_engines():
    # interleave N_A_UNITS 'A' units among 64 as evenly as possible
    eng = []
    for u in range(64):
        if (u + 1) * N_A_UNITS // 64 > u * N_A_UNITS // 64:
            eng.append('A')
        else:
            eng.append('B')
    return eng


def build_program(bpe: float, bpg: float, bb2: float, bint_zero: bool = True,
                  sim_trace: bool = False) -> bass.Bass:
    nc = bass.Bass()
    _register_const(nc, 0.5 * bpg)
    _register_const(nc, bb2)
    nc.all_engine_barrier()

    # ---- DRAM inputs (per-core views; same names across SPMD cores) ----
    d_WB = nc.dram_tensor("WB", [128, NWB], BF16, kind="ExternalInput")
    d_BI = nc.dram_tensor("BI", [128, NBI], F32, kind="ExternalInput")
    d_tfX = nc.dram_tensor("tfX", [2, 128, 256], F32, kind="ExternalInput")
    d_laT = nc.dram_tensor("laT", [64, NAH], BF16, kind="ExternalInput")
    d_W64 = nc.dram_tensor("W64", [64, 256], BF16, kind="ExternalInput")
    d_m0T = nc.dram_tensor("m0T", [32, 128, 32], BF16, kind="ExternalInput")
    d_M1 = nc.dram_tensor("M1m", [32, 128, 27], BF16, kind="ExternalInput")
    d_m1T = nc.dram_tensor("m1T", [4, 128, 64], BF16, kind="ExternalInput")
    d_M0 = nc.dram_tensor("M0m", [4, 128, 27], BF16, kind="ExternalInput")
    d_W0T = nc.dram_tensor("W0T", [64, 27 * 128], BF16, kind="ExternalInput")
    d_W32 = nc.dram_tensor("W32", [32, 27 * 128], BF16, kind="ExternalInput")
    d_lgT = nc.dram_tensor("lgT", [64, NG], BF16, kind="ExternalInput")
    d_Sh = nc.dram_tensor("Sh", [4, 128, NG], BF16, kind="ExternalInput")

    d_res = nc.dram_tensor("res_out", [1, 128], F32, kind="ExternalOutput")

    ENG = _unit_engines()
    if not bint_zero:
        ENG[:] = ['A'] * 64

    tc_ref = tile.TileContext(nc, trace_sim=sim_trace)
    with tc_ref as tc:
        with (
            tc.tile_pool(name="const", bufs=1) as cpool,
            tc.tile_pool(name="pre", bufs=1) as prepool,
            tc.tile_pool(name="x", bufs=6) as xpool,
            tc.tile_pool(name="u", bufs=4) as upool,
            tc.tile_pool(name="h", bufs=4) as hpool,
            tc.tile_pool(name="g", bufs=2) as gpool,
            tc.tile_pool(name="j", bufs=2) as jpool,
            tc.tile_pool(name="ps_y", bufs=3, space="PSUM") as psy,
            tc.tile_pool(name="ps_z", bufs=1, space="PSUM") as psz,
            tc.tile_pool(name="ps_p", bufs=1, space="PSUM") as pspre,
        ):
            # ---------- engine warmups (hide ACT table load + start PE pstate clock)
            warm = cpool.tile([128, 1], F32, tag="warm")
            nc.gpsimd.memset(warm[:], 0.0)
            warma = cpool.tile([128, 1], F32, tag="warma")
            nc.scalar.activation(warma[:], warm[:], AF.Silu)
            ps_warm = pspre.tile([1, 1], F32, tag="pre")
            nc.tensor.matmul(ps_warm[:], warm[:], warm[:], start=True, stop=True)
            warmb = cpool.tile([1, 1], F32, tag="warmb")
            nc.scalar.activation(warmb[:], ps_warm[:], AF.Copy)

            # ---------- input DMAs (order = DMA device service priority) ----
            tfx = prepool.tile([128, 512], F32, tag="tfx")
            nc.sync.dma_start(tfx[:, :].rearrange("p (c k) -> p c k", c=2),
                              d_tfX[:, :, :].rearrange("c p k -> p c k"))
            la = prepool.tile([64, NAH], BF16, tag="la")
            nc.sync.dma_start(la[:], d_laT[:])
            W64sb = cpool.tile([64, 256], BF16, tag="W64")
            nc.sync.dma_start(W64sb[:], d_W64[:])
            BIsb = cpool.tile([128, NBI], F32, tag="BI")
            nc.sync.dma_start(BIsb[:], d_BI[:])
            WBsb = cpool.tile([128, NWB], BF16, tag="WB")
            nc.sync.dma_start(WBsb[:], d_WB[:])
            m0sb = cpool.tile([128, 1024], BF16, tag="m0")
            nc.sync.dma_start(m0sb[:, :].rearrange("p (u c) -> p u c", u=32),
                              d_m0T[:, :, :].rearrange("u p c -> p u c"))
            M1sb = cpool.tile([128, 864], BF16, tag="M1")
            nc.sync.dma_start(M1sb[:, :].rearrange("p (u o) -> p u o", u=32),
                              d_M1[:, :, :].rearrange("u p o -> p u o"))
            m1sb = cpool.tile([128, 256], BF16, tag="m1")
            nc.sync.dma_start(m1sb[:, :].rearrange("p (u c) -> p u c", u=4),
                              d_m1T[:, :, :].rearrange("u p c -> p u c"))
            M0sb = cpool.tile([128, 108], BF16, tag="M0")
            nc.sync.dma_start(M0sb[:, :].rearrange("p (u o) -> p u o", u=4),
                              d_M0[:, :, :].rearrange("u p o -> p u o"))
            W0Tsb = cpool.tile([64, 27 * 128], BF16, tag="W0T")
            nc.sync.dma_start(W0Tsb[:], d_W0T[:])
            W32sb = cpool.tile([32, 27 * 128], BF16, tag="W32")
            nc.sync.dma_start(W32sb[:], d_W32[:])
            lg = cpool.tile([64, NG], BF16, tag="lg")
            nc.sync.dma_start(lg[:], d_lgT[:])
            Stsb = cpool.tile([128, 4 * NG], BF16, tag="St")
            nc.sync.dma_start(Stsb[:, :].rearrange("p (q g) -> p q g", q=4),
                              d_Sh[:, :, :].rearrange("q p g -> p q g"))

            bias = lambda i: BIsb[:, i:i + 1]

            # ---------- preamble: tok / atoms (needed before main loop) -----
            F32R = mybir.dt.float32r
            tfxv = tfx[:, :].rearrange("p (c k) -> p c k", c=2)
            tfr = prepool.tile([128, 256], F32R, tag="tfr")
            nc.scalar.activation(tfr[:, :].rearrange("p (c j) -> p c j", c=2),
                                 tfxv[:, :, 0:128], AF.Silu)
            wtokr = prepool.tile([128, 256], F32R, tag="wtokr")
            nc.scalar.activation(wtokr[:, :].rearrange("p (c o) -> p c o", c=2),
                                 tfxv[:, :, 128:256], AF.Copy)
            ps_tok = pspre.tile([128, 128], F32, tag="pre")
            nc.tensor.matmul(ps_tok[:], wtokr[:, 0:128],
                             tfr[:, 0:128], start=True, stop=False)
            nc.tensor.matmul(ps_tok[:], wtokr[:, 128:256],
                             tfr[:, 128:256], start=False, stop=True)
            tokT = cpool.tile([128, NT], F32, tag="tokT")
            nc.scalar.activation(tokT[:], ps_tok[:], AF.Identity, bias=bias(BI_TOK))

            ps_at = pspre.tile([128, NAH], F32, tag="pre")
            nc.tensor.matmul(ps_at[:], W64sb[:, 0:128], la[:], start=True, stop=True)
            atomsT = cpool.tile([128, NAH], BF16, tag="atomsT")
            nc.scalar.activation(atomsT[:], ps_at[:], AF.Identity, bias=bias(BI_ATOM))

            # ---------- deferred preamble tasks (interleaved into loop) ----
            state = {}

            def task_silu1():
                s0 = cpool.tile([128, 1024], BF16, tag="s0")
                nc.scalar.activation(s0[:], m0sb[:], AF.Silu)
                state["s0"] = s0

            def task_S1():
                S1 = pspre.tile([32, 27], F32, tag="pre")
                for u in range(32):
                    nc.tensor.matmul(S1[:], state["s0"][:, 32 * u:32 * u + 32],
                                     M1sb[:, 27 * u:27 * u + 27],
                                     start=(u == 0), stop=(u == 31))
                S1b = prepool.tile([32, 27], BF16, tag="S1b")
                nc.scalar.activation(S1b[:], S1[:], AF.Copy)
                state["S1b"] = S1b

            def task_p1():
                pp = pspre.tile([128, 1], F32, tag="pre")
                for o in range(27):
                    nc.tensor.matmul(pp[:], W32sb[:, 128 * o:128 * o + 128],
                                     state["S1b"][:, o:o + 1],
                                     start=(o == 0), stop=(o == 26))
                sp1 = prepool.tile([128, 1], BF16, tag="sp1")
                nc.scalar.activation(sp1[:], pp[:], AF.Silu, bias=bias(BI_C1))
                state["sp1"] = sp1

            def task_silu0():
                s1 = prepool.tile([128, 256], BF16, tag="s1")
                nc.scalar.activation(s1[:], m1sb[:], AF.Silu)
                state["s1"] = s1

            def task_S0():
                S0 = pspre.tile([64, 27], F32, tag="pre")
                for u in range(4):
                    nc.tensor.matmul(S0[:], state["s1"][:, 64 * u:64 * u + 64],
                                     M0sb[:, 27 * u:27 * u + 27],
                                     start=(u == 0), stop=(u == 3))
                S0b = prepool.tile([64, 27], BF16, tag="S0b")
                nc.scalar.activation(S0b[:], S0[:], AF.Copy)
                state["S0b"] = S0b

            def task_p0():
                pp = pspre.tile([128, 1], F32, tag="pre")
                for o in range(27):
                    nc.tensor.matmul(pp[:], W0Tsb[:, 128 * o:128 * o + 128],
                                     state["S0b"][:, o:o + 1],
                                     start=(o == 0), stop=(o == 26))
                sp0 = prepool.tile([128, 1], BF16, tag="sp0")
                nc.scalar.activation(sp0[:], pp[:], AF.Silu, bias=bias(BI_C0))
                state["sp0"] = sp0

            def task_pocket():
                ps_pk = pspre.tile([128, 1], F32, tag="pre")
                nc.tensor.matmul(ps_pk[:], WBsb[:, OFF_WPK:OFF_WPK + 128],
                                 state["sp0"][:], start=True, stop=False)
                nc.tensor.matmul(ps_pk[:], WBsb[:, OFF_WPK + 128:OFF_WPK + 256],
                                 state["sp1"][:], start=False, stop=True)
                pocket = prepool.tile([128, 1], BF16, tag="pocket")
                nc.scalar.activation(pocket[:], ps_pk[:], AF.Identity, bias=bias(BI_PK))
                state["pocket"] = pocket

            def task_pf():
                tok_sum = prepool.tile([128, 1], F32, tag="toksum")
                junkt = jpool.tile([128, NT], F32, tag="junk")
                nc.vector.tensor_scalar(junkt[:], tokT[:], 1.0, 0.0, op0=ALU.mult,
                                        op1=ALU.add, accum_out=tok_sum[:])
                tok_sum_b = prepool.tile([128, 1], BF16, tag="toksumb")
                nc.scalar.activation(tok_sum_b[:], tok_sum[:], AF.Copy)
                ps_pf = pspre.tile([128, 2], F32, tag="pre")
                chunks = [state["pocket"], tok_sum_b, tok_sum_b]
                for q in range(3):
                    nc.tensor.matmul(ps_pf[:, 0:1],
                                     WBsb[:, OFF_WCAT + 128 * q:OFF_WCAT + 128 * (q + 1)],
                                     chunks[q][:], start=(q == 0), stop=(q == 2))
                for q in range(3):
                    nc.tensor.matmul(ps_pf[:, 1:2],
                                     WBsb[:, OFF_WGATE + 128 * q:OFF_WGATE + 128 * (q + 1)],
                                     chunks[q][:], start=(q == 0), stop=(q == 2))
                # sigmoid(z + bg) = 0.5 + 0.5*tanh(0.5z + 0.5bg)
                gt = prepool.tile([128, 1], F32, tag="gt")
                nc.scalar.activation(gt[:], ps_pf[:, 1:2], AF.Tanh,
                                     bias=bias(BI_GH), scale=0.5)
                pf_sig = prepool.tile([128, 1], F32, tag="pfsig")
                nc.gpsimd.tensor_scalar(pf_sig[:], gt[:], 0.5, 0.5, op0=ALU.mult, op1=ALU.add)
                pf_lin = prepool.tile([128, 1], F32, tag="pflin")
                nc.scalar.activation(pf_lin[:], ps_pf[:, 0:1], AF.Identity, bias=bias(BI_CAT))
                pf = prepool.tile([128, 1], BF16, tag="pf")
                nc.gpsimd.tensor_tensor(pf[:], pf_lin[:], pf_sig[:], op=ALU.mult)
                state["pf"] = pf

            def task_gf():
                ps_gf = pspre.tile([128, NG], F32, tag="pre")
                nc.tensor.matmul(ps_gf[:], W64sb[:, 128:256], lg[:], start=True, stop=True)
                gfT = prepool.tile([128, NG], BF16, tag="gfT")
                nc.scalar.activation(gfT[:], ps_gf[:], AF.Identity, bias=bias(BI_GR))
                state["gfT"] = gfT

            def task_bias1():
                ps_u = pspre.tile([128, 1], F32, tag="pre")
                nc.tensor.matmul(ps_u[:], WBsb[:, OFF_WB1:OFF_WB1 + 128],
                                 state["pf"][:], start=True, stop=True)
                ub = prepool.tile([128, 1], F32, tag="ub")
                nc.scalar.activation(ub[:], ps_u[:], AF.Identity, bias=bias(BI_B1))
                ps_hb = pspre.tile([128, NG], F32, tag="pre")
                nc.tensor.matmul(ps_hb[:], WBsb[:, OFF_WB1 + 128:OFF_WB1 + 256],
                                 state["gfT"][:], start=True, stop=True)
                hb = prepool.tile([128, NG], BF16, tag="hb")
                nc.scalar.activation(hb[:], ps_hb[:], AF.Prelu, bias=ub[:], alpha=0.01)
                state["hb"] = hb

            def task_bias2():
                ps_b2 = pspre.tile([1, NG], F32, tag="pre")
                nc.tensor.matmul(ps_b2[:], WBsb[:, OFF_WB2:OFF_WB2 + 1],
                                 state["hb"][:], start=True, stop=True)
                nc.scalar.activation(res[:, NG:2 * NG], ps_b2[:], AF.Identity, bias=bb2)

            pre_tasks = [task_silu1, task_S1, task_p1, task_silu0, task_S0,
                         task_p0, task_pocket, task_pf, task_gf, task_bias1,
                         task_bias2]
            TASK_AT = {12 + 5 * i: t for i, t in enumerate(pre_tasks)}

            res = cpool.tile([1, 128], F32, tag="res")

            # ---------- main loop ----------
            # 64 units u of 2 tokens; y2[o, 512v + a] for token j = 2u+v.
            # zq8 (per 64-token block) col layout: 8*(j%64) + 2*a_chunk + {pe,pg}
            wpegr = cpool.tile([128, 2], F32R, tag="wpegr")
            nc.scalar.activation(wpegr[:], BIsb[:, BI_WPEG:BI_WPEG + 2], AF.Copy)
            wpeg_ap = wpegr[:]
            upeg_ap = WBsb[:, OFF_UPEG:OFF_UPEG + 2]
            wint_ap = WBsb[:, OFF_WINT:OFF_WINT + 128]
            zq_tiles = [None, None]
            ae_parts = cpool.tile([128, 8], F32, tag="aeparts")
            pending = []

            def emit_unit(u):
                y2 = psy.tile([128, 1024], F32, tag="y")
                ujs = []
                for v in range(2):
                    j = 2 * u + v
                    Wj = xpool.tile([128, 128], BF16, tag="x")
                    nc.gpsimd.tensor_scalar_mul(Wj[:], wint_ap, tokT[:, j:j + 1])
                    nc.tensor.matmul(y2[:, 512 * v:512 * (v + 1)], Wj[:], atomsT[:],
                                     start=True, stop=True)
                    if ENG[u] == 'B':
                        uj = upool.tile([128, 2], BF16, tag="u")
                        nc.gpsimd.tensor_scalar_mul(uj[:], upeg_ap, tokT[:, j:j + 1])
                        ujs.append(uj)
                return (u, y2, ujs)

            def flush_unit(ent):
                u, y2, ujs = ent
                h = hpool.tile([128, 1024], F32R, tag="h")
                if ENG[u] == 'A':
                    nc.scalar.activation(h[:], y2[:], AF.Prelu, bias=bias(BI_INT),
                                         alpha=0.01)
                else:
                    # h = 0.99*relu(y); the 0.01*y linear part of lrelu is
                    # folded into the zq accumulation via upeg below
                    nc.vector.tensor_scalar(h[:], y2[:], 0.0, 0.99,
                                            op0=ALU.max, op1=ALU.mult)
                for v in range(2):
                    j = 2 * u + v
                    b, jj = j // 64, j % 64
                    if zq_tiles[b] is None:
                        zq_tiles[b] = psz.tile([128, 512], F32, tag="z", name=f"zq{b}")
                    zq = zq_tiles[b]
                    for a in range(4):
                        cols = zq[:, 8 * jj + 2 * a:8 * jj + 2 * a + 2]
                        if ENG[u] == 'A':
                            nc.tensor.matmul(cols, h[:, 512 * v + 128 * a:512 * v + 128 * (a + 1)],
                                             wpeg_ap, start=True, stop=True)
                        else:
                            nc.tensor.matmul(cols, h[:, 512 * v + 128 * a:512 * v + 128 * (a + 1)],
                                             wpeg_ap, start=True, stop=False)
                            nc.tensor.matmul(cols, atomsT[:, 128 * a:128 * (a + 1)],
                                             ujs[v][:], start=False, stop=True)

            def gates(b):
                zq = zq_tiles[b]
                s = gpool.tile([128, 256], F32, tag="s")
                nc.scalar.activation(s[:], zq[:, 1::2], AF.Tanh, bias=0.5 * bpg, scale=0.5)
                w = gpool.tile([128, 256], F32, tag="w")
                nc.gpsimd.tensor_scalar(w[:], s[:], 0.5, 0.5, op0=ALU.mult, op1=ALU.add)
                t = gpool.tile([128, 256], F32, tag="t")
                nc.vector.scalar_tensor_tensor(t[:], zq[:, 0::2], bpe, w[:],
                                               op0=ALU.add, op1=ALU.mult)
                for a in range(4):
                    junka = jpool.tile([128, 64], F32, tag="junka")
                    nc.vector.tensor_scalar(junka[:], t[:, a::4], 1.0, 0.0,
                                            op0=ALU.mult, op1=ALU.add,
                                            accum_out=ae_parts[:, 4 * b + a:4 * b + a + 1])

            for u in range(64):
                pending.append(emit_unit(u))
                if len(pending) > 1:
                    flush_unit(pending.pop(0))
                fu = u - 1  # unit just flushed
                if fu == 31:
                    gates(0)
                if fu in TASK_AT:
                    TASK_AT[fu]()

            flush_unit(pending.pop(0))
            gates(1)

            # atom_e reduce -> seg matmul -> out
            ae4r = prepool.tile([128, 4], F32R, tag="ae4r")
            nc.vector.tensor_tensor(ae4r[:], ae_parts[:, 0:4], ae_parts[:, 4:8], op=ALU.add)
            Str = prepool.tile([128, 4 * NG], F32R, tag="Str")
            nc.scalar.activation(Str[:], Stsb[:], AF.Copy)
            ps_seg = pspre.tile([1, NG], F32, tag="pre")
            for q in range(4):
                nc.tensor.matmul(ps_seg[:], ae4r[:, q:q + 1], Str[:, q * NG:(q + 1) * NG],
                                 start=(q == 0), stop=(q == 3))
            nc.scalar.activation(res[:, 0:NG], ps_seg[:], AF.Copy)
            nc.sync.dma_start(d_res[:], res[:])

    _legalize_waits(nc)
    nc._tile_ctx = tc_ref
    return nc


def kernel(**inputs) -> np.ndarray:
    f = lambda a: np.ascontiguousarray(np.asarray(a), dtype=np.float32)
    bf = lambda a: np.ascontiguousarray(np.asarray(a, dtype=np.float32)).astype(ml_dtypes.bfloat16)
    tf = f(inputs["token_features"])
    la = f(inputs["lig_atom"])
    lgr = f(inputs["lig_graph"])
    m0 = f(inputs["ms_feat_0"])
    m1 = f(inputs["ms_feat_1"])
    lb = np.asarray(inputs["ligand_batch"])
    S = (lb[:, None] == np.arange(NG)[None, :]).astype(np.float32)

    # ---- weight prep (host-side layout/scale transforms only) ----
    wint_bf = bf(inputs["W_int"])                       # [128,128]
    wpe = f(inputs["W_pe"]); wpg = f(inputs["W_pg"])    # [128,1]
    wpeg = np.concatenate([wpe, wpg], axis=1)           # [128,2]
    u_pe = wint_bf.astype(np.float64) @ wpe.astype(np.float64)
    u_pg = wint_bf.astype(np.float64) @ wpg.astype(np.float64)
    upeg = 0.01 * np.concatenate([u_pe, u_pg], axis=1)  # [128,2]

    wcat = f(inputs["W_cat"]).copy()                    # [384,128]
    wgate = f(inputs["W_gate"]).copy()
    wcat[2 * HID:] /= float(NT)
    wgate[2 * HID:] /= float(NT)

    WB = np.zeros((128, NWB), dtype=np.float32)
    WB[:, OFF_WINT:OFF_WINT + 128] = wint_bf.astype(np.float32)
    WB[:, OFF_WTOK:OFF_WTOK + 256] = f(inputs["W_token"]).reshape(2, 128, HID).transpose(1, 0, 2).reshape(128, 256)
    WB[:, OFF_WPK:OFF_WPK + 256] = f(inputs["W_pocket"]).reshape(2, 128, HID).transpose(1, 0, 2).reshape(128, 256)
    WB[:, OFF_WCAT:OFF_WCAT + 384] = wcat.reshape(3, 128, HID).transpose(1, 0, 2).reshape(128, 384)
    WB[:, OFF_WGATE:OFF_WGATE + 384] = wgate.reshape(3, 128, HID).transpose(1, 0, 2).reshape(128, 384)
    WB[:, OFF_WB1:OFF_WB1 + 256] = f(inputs["W_bias1"]).reshape(2, 128, HID).transpose(1, 0, 2).reshape(128, 256)
    WB[:, OFF_WB2:OFF_WB2 + 1] = f(inputs["W_bias2"])
    WB[:, OFF_WPEG:OFF_WPEG + 2] = wpeg
    WB[:, OFF_UPEG:OFF_UPEG + 2] = upeg
    WB_bf = WB.astype(ml_dtypes.bfloat16)

    # conv weights as [c, off*128 + o], scaled by 1/num_output_positions
    Wc0 = f(inputs["Wc0"])  # [128,64,3,3,3] applied to ms_feat_1
    Wc1 = f(inputs["Wc1"])  # [128,32,3,3,3] applied to ms_feat_0
    W0T = np.ascontiguousarray(Wc0.reshape(128, 64, 27).transpose(1, 2, 0)).reshape(64, 27 * 128) / 216.0
    W32 = np.ascontiguousarray(Wc1.reshape(128, 32, 27).transpose(1, 2, 0)).reshape(32, 27 * 128) / 2744.0

    W64 = np.zeros((64, 256), dtype=np.float32)
    W64[:, 0:128] = f(inputs["W_atom"])
    W64[:, 128:256] = f(inputs["W_graph"])

    col = lambda a: f(a).reshape(128, 1)
    BI = np.zeros((128, NBI), dtype=np.float32)
    BI[:, BI_TOK] = f(inputs["b_token"])
    BI[:, BI_ATOM] = f(inputs["b_atom"])
    BI[:, BI_INT] = f(inputs["b_int"])
    BI[:, BI_PK] = f(inputs["b_pocket"])
    BI[:, BI_CAT] = f(inputs["b_cat"])
    BI[:, BI_GH] = 0.5 * f(inputs["b_gate"])
    BI[:, BI_GR] = f(inputs["b_graph"])
    BI[:, BI_B1] = f(inputs["b_bias1"])
    BI[:, BI_C0] = f(inputs["bc0"])
    BI[:, BI_C1] = f(inputs["bc1"])
    BI[:, BI_WPEG:BI_WPEG + 2] = wpeg

    # window-membership masks: M[pos, off] = 1 iff pos-off in valid out range
    def win_mask(D, O):
        g = np.arange(D)
        z, y, x = np.meshgrid(g, g, g, indexing="ij")
        pos = np.stack([z.ravel(), y.ravel(), x.ravel()], 1)  # [D^3, 3]
        d = np.arange(3)
        dz, dy, dx = np.meshgrid(d, d, d, indexing="ij")
        off = np.stack([dz.ravel(), dy.ravel(), dx.ravel()], 1)  # [27, 3]
        r = pos[:, None, :] - off[None, :, :]
        return np.all((r >= 0) & (r < O), axis=2).astype(np.float32)  # [D^3, 27]

    M1 = win_mask(16, 14).reshape(32, 128, 27)
    M0 = win_mask(8, 6).reshape(4, 128, 27)

    bpe = float(np.asarray(inputs["b_pe"]).reshape(-1)[0])
    bpg = float(np.asarray(inputs["b_pg"]).reshape(-1)[0])
    bb2 = float(np.asarray(inputs["b_bias2"]).reshape(-1)[0])

    shared = {
        "WB": WB_bf, "BI": BI,
        "W64": W64.astype(ml_dtypes.bfloat16),
        "W0T": W0T.astype(ml_dtypes.bfloat16),
        "W32": W32.astype(ml_dtypes.bfloat16),
        "M1m": M1.astype(ml_dtypes.bfloat16),
        "M0m": M0.astype(ml_dtypes.bfloat16),
    }

    in_maps = []
    for c in range(NCORES):
        n, h = c // 2, c % 2
        m = dict(shared)
        tfX = np.zeros((2, 128, 256), dtype=np.float32)
        tfX[:, :, 0:128] = tf[n].T.reshape(2, 128, 128)
        tfX[:, :, 128:256] = f(inputs["W_token"]).reshape(2, 128, HID)
        m["tfX"] = tfX
        m["laT"] = bf(la[n, 512 * h:512 * (h + 1)].T)
        m["lgT"] = bf(lgr[n].T)
        m["m0T"] = bf(m0[n].reshape(32, 4096).T.reshape(32, 128, 32))
        m["m1T"] = bf(m1[n].reshape(64, 512).T.reshape(4, 128, 64))
        m["Sh"] = bf(S[512 * h:512 * (h + 1)].reshape(4, 128, NG))
        in_maps.append(m)

    bint_zero = bool(np.all(np.asarray(inputs["b_int"]) == 0.0))
    nc = build_program(bpe, bpg, bb2, bint_zero)
    r = run_bass_kernel_spmd(nc, in_maps, core_ids=list(range(NCORES)),
                             trace=TRACE, **(TRACE_KW if TRACE else {}))
    global LAST
    LAST = r
    res = r.results

    out = np.zeros((NI, NG), dtype=np.float32)
    for n in range(NI):
        out[n] = (res[2 * n]["res_out"][0, 0:NG] + res[2 * n + 1]["res_out"][0, 0:NG]
                  + res[2 * n]["res_out"][0, NG:2 * NG])
    return out


# revision 17
# speedup vs baseline: 2.0939x; 1.0201x over previous
import sys
import numpy as np
import ml_dtypes

sys.path.insert(0, "/opt/trn_rl_repo")

import concourse.bass as bass
import concourse.tile as tile
from concourse import mybir
from concourse.bass_utils import run_bass_kernel_spmd

F32 = mybir.dt.float32
BF16 = mybir.dt.bfloat16
AF = mybir.ActivationFunctionType
ALU = mybir.AluOpType

HID = 128
NT = 128       # tokens per image
NAH = 512      # atoms per core (half of 1024)
NG = 64        # ligand graphs
NI = 4         # images
NCORES = 8

# WB (128-partition weight concat, bf16) column offsets
OFF_WINT = 0
OFF_WTOK = 128
OFF_WPK = 384
OFF_WCAT = 640
OFF_WGATE = 1024
OFF_WB1 = 1408
OFF_WB2 = 1664
OFF_WPEG = 1665
OFF_UPEG = 1667
NWB = 1669

# BI (f32 bias concat) columns
BI_TOK, BI_ATOM, BI_INT, BI_PK, BI_CAT, BI_GH, BI_GR, BI_B1, BI_C0, BI_C1 = range(10)
BI_WPEG = 10   # cols 10:12 = [W_pe, W_pg] f32
NBI = 12

# lrelu unit assignment: 'A' = ACT Prelu, 'B' = DVE relu99 + linear-fold
N_A_UNITS = 32

TRACE = False
TRACE_KW = {}
LAST = None


_COMPUTE_INSTS = (
    "InstActivation", "InstTensorCopy", "InstTensorScalar", "InstTensorScalarPtr",
    "InstTensorTensor", "InstTensorTensorReduce", "InstTensorReduce", "InstMemSet",
    "InstMatmult", "InstScalarTensorTensor", "InstTensorTensorScan", "InstLdweights",
    "InstDMACopy", "InstDMATransposeAnt", "InstTriggeredCopy", "InstDrain",
    "InstEventSemaphoreOp", "InstSemaphoreOp", "InstCopy", "InstIota", "InstSelect",
)


def _legalize_waits(nc):
    # walrus in this toolchain accepts at most ONE sync wait on TPB compute
    # instructions; hoist extras into same-engine NoOps placed just before.
    k = 0
    for f in nc.m.functions:
        for blk in f.blocks:
            insts = blk.instructions
            out = []
            for ins in insts:
                si = getattr(ins, "sync_info", None)
                if (si is not None and len(si.on_wait) > 1
                        and type(ins).__name__ in _COMPUTE_INSTS):
                    waits = list(si.on_wait)
                    for w in waits[:-1]:
                        nop = mybir.InstNoOp(
                            name=f"WNOP-{k}", engine=ins.engine,
                            sync_info=mybir.SyncInfo(on_wait=[w], on_update=[]))
                        k += 1
                        out.append(nop)
                    ins.sync_info = mybir.SyncInfo(on_wait=[waits[-1]],
                                                   on_update=list(si.on_update))
                out.append(ins)
            blk.instructions = out
    return k


def _register_const(nc, val, dtype=F32):
    if (dtype, float(val)) in nc.const_aps.aps:
        return
    t = nc.alloc_sbuf_tensor(f"uconst-{dtype.name}-{val}", [128, 1], dtype)
    nc.gpsimd.memset(t.ap(), float(val))
    nc.const_aps.aps[(dtype, float(val))] = t.ap()


def _unit_engines():
    # interleave N_A_UNITS 'A' units among 64 as evenly as possible
    eng = []
    for u in range(64):
        if (u + 1) * N_A_UNITS // 64 > u * N_A_UNITS // 64:
            eng.append('B')
        else:
            eng.append('A')
    return eng


def build_program(bpe: float, bpg: float, bb2: float, bint_zero: bool = True,
                  sim_trace: bool = False) -> bass.Bass:
    nc = bass.Bass()
    _register_const(nc, 0.5 * bpg)
    _register_const(nc, bb2)
    nc.all_engine_barrier()

    # ---- DRAM inputs (per-core views; same names across SPMD cores) ----
    d_WB = nc.dram_tensor("WB", [128, NWB], BF16, kind="ExternalInput")
    d_BI = nc.dram_tensor("BI", [128, NBI], F32, kind="ExternalInput")
    d_tfX = nc.dram_tensor("tfX", [2, 128, 128], F32, kind="ExternalInput")
    d_laT = nc.dram_tensor("laT", [64, NAH], BF16, kind="ExternalInput")
    d_W64 = nc.dram_tensor("W64", [64, 256], BF16, kind="ExternalInput")
    d_m0T = nc.dram_tensor("m0T", [32, 128, 32], BF16, kind="ExternalInput")
    d_M1 = nc.dram_tensor("M1m", [32, 128, 27], BF16, kind="ExternalInput")
    d_m1T = nc.dram_tensor("m1T", [4, 128, 64], BF16, kind="ExternalInput")
    d_M0 = nc.dram_tensor("M0m", [4, 128, 27], BF16, kind="ExternalInput")
    d_W0T = nc.dram_tensor("W0T", [64, 27 * 128], BF16, kind="ExternalInput")
    d_W32 = nc.dram_tensor("W32", [32, 27 * 128], BF16, kind="ExternalInput")
    d_lgT = nc.dram_tensor("lgT", [64, NG], BF16, kind="ExternalInput")
    d_Sh = nc.dram_tensor("Sh", [4, 128, NG], BF16, kind="ExternalInput")

    d_res = nc.dram_tensor("res_out", [1, 128], F32, kind="ExternalOutput")

    ENG = _unit_engines()
    if not bint_zero:
        ENG[:] = ['A'] * 64

    tc_ref = tile.TileContext(nc, trace_sim=sim_trace)
    with tc_ref as tc:
        with (
            tc.tile_pool(name="const", bufs=1) as cpool,
            tc.tile_pool(name="pre", bufs=1) as prepool,
            tc.tile_pool(name="x", bufs=8) as xpool,
            tc.tile_pool(name="u", bufs=4) as upool,
            tc.tile_pool(name="h", bufs=6) as hpool,
            tc.tile_pool(name="g", bufs=2) as gpool,
            tc.tile_pool(name="j", bufs=2) as jpool,
            tc.tile_pool(name="ps_y", bufs=3, space="PSUM") as psy,
            tc.tile_pool(name="ps_z", bufs=1, space="PSUM") as psz,
            tc.tile_pool(name="ps_p", bufs=1, space="PSUM") as pspre,
        ):
            # ---------- engine warmups (hide ACT table load + start PE pstate clock)
            warm = cpool.tile([128, 1], F32, tag="warm")
            nc.gpsimd.memset(warm[:], 0.0)
            warma = cpool.tile([128, 1], F32, tag="warma")
            nc.scalar.activation(warma[:], warm[:], AF.Silu)
            ps_warm = pspre.tile([1, 1], F32, tag="pre")
            nc.tensor.matmul(ps_warm[:], warm[:], warm[:], start=True, stop=True)
            warmb = cpool.tile([1, 1], F32, tag="warmb")
            nc.scalar.activation(warmb[:], ps_warm[:], AF.Copy)

            # ---------- input DMAs (order = DMA device service priority) ----
            tfx = prepool.tile([128, 256], F32, tag="tfx")
            nc.sync.dma_start(tfx[:, :].rearrange("p (c k) -> p c k", c=2),
                              d_tfX[:, :, :].rearrange("c p k -> p c k"))
            la = prepool.tile([64, NAH], BF16, tag="la")
            nc.sync.dma_start(la[:], d_laT[:])
            W64sb = cpool.tile([64, 256], BF16, tag="W64")
            nc.sync.dma_start(W64sb[:], d_W64[:])
            BIsb = cpool.tile([128, NBI], F32, tag="BI")
            nc.sync.dma_start(BIsb[:], d_BI[:])
            WBsb = cpool.tile([128, NWB], BF16, tag="WB")
            nc.sync.dma_start(WBsb[:], d_WB[:])
            m0sb = cpool.tile([128, 1024], BF16, tag="m0")
            nc.sync.dma_start(m0sb[:, :].rearrange("p (u c) -> p u c", u=32),
                              d_m0T[:, :, :].rearrange("u p c -> p u c"))
            M1sb = cpool.tile([128, 864], BF16, tag="M1")
            nc.sync.dma_start(M1sb[:, :].rearrange("p (u o) -> p u o", u=32),
                              d_M1[:, :, :].rearrange("u p o -> p u o"))
            m1sb = cpool.tile([128, 256], BF16, tag="m1")
            nc.sync.dma_start(m1sb[:, :].rearrange("p (u c) -> p u c", u=4),
                              d_m1T[:, :, :].rearrange("u p c -> p u c"))
            M0sb = cpool.tile([128, 108], BF16, tag="M0")
            nc.sync.dma_start(M0sb[:, :].rearrange("p (u o) -> p u o", u=4),
                              d_M0[:, :, :].rearrange("u p o -> p u o"))
            W0Tsb = cpool.tile([64, 27 * 128], BF16, tag="W0T")
            nc.sync.dma_start(W0Tsb[:], d_W0T[:])
            W32sb = cpool.tile([32, 27 * 128], BF16, tag="W32")
            nc.sync.dma_start(W32sb[:], d_W32[:])
            lg = cpool.tile([64, NG], BF16, tag="lg")
            nc.sync.dma_start(lg[:], d_lgT[:])
            Stsb = cpool.tile([128, 4 * NG], BF16, tag="St")
            nc.sync.dma_start(Stsb[:, :].rearrange("p (q g) -> p q g", q=4),
                              d_Sh[:, :, :].rearrange("q p g -> p q g"))
            F32R = mybir.dt.float32r

            bias = lambda i: BIsb[:, i:i + 1]

            # ---------- preamble: tok / atoms (needed before main loop) -----
            tfr = prepool.tile([128, 256], BF16, tag="tfr")
            nc.scalar.activation(tfr[:], tfx[:], AF.Silu)
            ps_tok = pspre.tile([128, 128], F32, tag="pre")
            nc.tensor.matmul(ps_tok[:], WBsb[:, OFF_WTOK:OFF_WTOK + 128],
                             tfr[:, 0:128], start=True, stop=False)
            nc.tensor.matmul(ps_tok[:], WBsb[:, OFF_WTOK + 128:OFF_WTOK + 256],
                             tfr[:, 128:256], start=False, stop=True)
            tokT = cpool.tile([128, NT], F32, tag="tokT")
            nc.scalar.activation(tokT[:], ps_tok[:], AF.Identity, bias=bias(BI_TOK))

            ps_at = pspre.tile([128, NAH], F32, tag="pre")
            nc.tensor.matmul(ps_at[:], W64sb[:, 0:128], la[:], start=True, stop=True)
            atomsT = cpool.tile([128, NAH], BF16, tag="atomsT")
            nc.scalar.activation(atomsT[:], ps_at[:], AF.Identity, bias=bias(BI_ATOM))

            # ---------- deferred preamble tasks (interleaved into loop) ----
            state = {}

            def task_silu1():
                s0 = cpool.tile([128, 1024], BF16, tag="s0")
                nc.scalar.activation(s0[:], m0sb[:], AF.Silu)
                state["s0"] = s0

            def task_S1():
                S1 = pspre.tile([32, 27], F32, tag="pre")
                for u in range(32):
                    nc.tensor.matmul(S1[:], state["s0"][:, 32 * u:32 * u + 32],
                                     M1sb[:, 27 * u:27 * u + 27],
                                     start=(u == 0), stop=(u == 31))
                S1b = prepool.tile([32, 27], BF16, tag="S1b")
                nc.scalar.activation(S1b[:], S1[:], AF.Copy)
                state["S1b"] = S1b

            def task_p1():
                pp = pspre.tile([128, 1], F32, tag="pre")
                for o in range(27):
                    nc.tensor.matmul(pp[:], W32sb[:, 128 * o:128 * o + 128],
                                     state["S1b"][:, o:o + 1],
                                     start=(o == 0), stop=(o == 26))
                sp1 = prepool.tile([128, 1], BF16, tag="sp1")
                nc.scalar.activation(sp1[:], pp[:], AF.Silu, bias=bias(BI_C1))
                state["sp1"] = sp1

            def task_silu0():
                s1 = prepool.tile([128, 256], BF16, tag="s1")
                nc.scalar.activation(s1[:], m1sb[:], AF.Silu)
                state["s1"] = s1

            def task_S0():
                S0 = pspre.tile([64, 27], F32, tag="pre")
                for u in range(4):
                    nc.tensor.matmul(S0[:], state["s1"][:, 64 * u:64 * u + 64],
                                     M0sb[:, 27 * u:27 * u + 27],
                                     start=(u == 0), stop=(u == 3))
                S0b = prepool.tile([64, 27], BF16, tag="S0b")
                nc.scalar.activation(S0b[:], S0[:], AF.Copy)
                state["S0b"] = S0b

            def task_p0():
                pp = pspre.tile([128, 1], F32, tag="pre")
                for o in range(27):
                    nc.tensor.matmul(pp[:], W0Tsb[:, 128 * o:128 * o + 128],
                                     state["S0b"][:, o:o + 1],
                                     start=(o == 0), stop=(o == 26))
                sp0 = prepool.tile([128, 1], BF16, tag="sp0")
                nc.scalar.activation(sp0[:], pp[:], AF.Silu, bias=bias(BI_C0))
                state["sp0"] = sp0

            def task_pocket():
                ps_pk = pspre.tile([128, 1], F32, tag="pre")
                nc.tensor.matmul(ps_pk[:], WBsb[:, OFF_WPK:OFF_WPK + 128],
                                 state["sp0"][:], start=True, stop=False)
                nc.tensor.matmul(ps_pk[:], WBsb[:, OFF_WPK + 128:OFF_WPK + 256],
                                 state["sp1"][:], start=False, stop=True)
                pocket = prepool.tile([128, 1], BF16, tag="pocket")
                nc.scalar.activation(pocket[:], ps_pk[:], AF.Identity, bias=bias(BI_PK))
                state["pocket"] = pocket

            def task_pf():
                tok_sum = prepool.tile([128, 1], F32, tag="toksum")
                junkt = jpool.tile([128, NT], F32, tag="junk")
                nc.vector.tensor_scalar(junkt[:], tokT[:], 1.0, 0.0, op0=ALU.mult,
                                        op1=ALU.add, accum_out=tok_sum[:])
                tok_sum_b = prepool.tile([128, 1], BF16, tag="toksumb")
                nc.scalar.activation(tok_sum_b[:], tok_sum[:], AF.Copy)
                ps_pf = pspre.tile([128, 2], F32, tag="pre")
                chunks = [state["pocket"], tok_sum_b, tok_sum_b]
                for q in range(3):
                    nc.tensor.matmul(ps_pf[:, 0:1],
                                     WBsb[:, OFF_WCAT + 128 * q:OFF_WCAT + 128 * (q + 1)],
                                     chunks[q][:], start=(q == 0), stop=(q == 2))
                for q in range(3):
                    nc.tensor.matmul(ps_pf[:, 1:2],
                                     WBsb[:, OFF_WGATE + 128 * q:OFF_WGATE + 128 * (q + 1)],
                                     chunks[q][:], start=(q == 0), stop=(q == 2))
                # sigmoid(z + bg) = 0.5 + 0.5*tanh(0.5z + 0.5bg)
                gt = prepool.tile([128, 1], F32, tag="gt")
                nc.scalar.activation(gt[:], ps_pf[:, 1:2], AF.Tanh,
                                     bias=bias(BI_GH), scale=0.5)
                pf_sig = prepool.tile([128, 1], F32, tag="pfsig")
                nc.gpsimd.tensor_scalar(pf_sig[:], gt[:], 0.5, 0.5, op0=ALU.mult, op1=ALU.add)
                pf_lin = prepool.tile([128, 1], F32, tag="pflin")
                nc.scalar.activation(pf_lin[:], ps_pf[:, 0:1], AF.Identity, bias=bias(BI_CAT))
                pf = prepool.tile([128, 1], BF16, tag="pf")
                nc.gpsimd.tensor_tensor(pf[:], pf_lin[:], pf_sig[:], op=ALU.mult)
                state["pf"] = pf

            def task_gf():
                ps_gf = pspre.tile([128, NG], F32, tag="pre")
                nc.tensor.matmul(ps_gf[:], W64sb[:, 128:256], lg[:], start=True, stop=True)
                gfT = prepool.tile([128, NG], BF16, tag="gfT")
                nc.scalar.activation(gfT[:], ps_gf[:], AF.Identity, bias=bias(BI_GR))
                state["gfT"] = gfT

            def task_bias1():
                ps_u = pspre.tile([128, 1], F32, tag="pre")
                nc.tensor.matmul(ps_u[:], WBsb[:, OFF_WB1:OFF_WB1 + 128],
                                 state["pf"][:], start=True, stop=True)
                ub = prepool.tile([128, 1], F32, tag="ub")
                nc.scalar.activation(ub[:], ps_u[:], AF.Identity, bias=bias(BI_B1))
                ps_hb = pspre.tile([128, NG], F32, tag="pre")
                nc.tensor.matmul(ps_hb[:], WBsb[:, OFF_WB1 + 128:OFF_WB1 + 256],
                                 state["gfT"][:], start=True, stop=True)
                hb = prepool.tile([128, NG], BF16, tag="hb")
                nc.scalar.activation(hb[:], ps_hb[:], AF.Prelu, bias=ub[:], alpha=0.01)
                state["hb"] = hb

            def task_bias2():
                ps_b2 = pspre.tile([1, NG], F32, tag="pre")
                nc.tensor.matmul(ps_b2[:], WBsb[:, OFF_WB2:OFF_WB2 + 1],
                                 state["hb"][:], start=True, stop=True)
                nc.scalar.activation(res[:, NG:2 * NG], ps_b2[:], AF.Identity, bias=bb2)

            pre_tasks = [task_silu1, task_S1, task_p1, task_silu0, task_S0,
                         task_p0, task_pocket, task_pf, task_gf, task_bias1,
                         task_bias2]
            TASK_AT = {12 + 4 * i: t for i, t in enumerate(pre_tasks)}

            res = cpool.tile([1, 128], F32, tag="res")

            # ---------- main loop ----------
            # 64 units u of 2 tokens; y2[o, 512v + a] for token j = 2u+v.
            # zq8 (per 64-token block) col layout: 8*(j%64) + 2*a_chunk + {pe,pg}
            wpegr = cpool.tile([128, 2], F32R, tag="wpegr")
            nc.scalar.activation(wpegr[:], BIsb[:, BI_WPEG:BI_WPEG + 2], AF.Copy)
            wpeg_ap = wpegr[:]
            upeg_ap = WBsb[:, OFF_UPEG:OFF_UPEG + 2]
            wint_ap = WBsb[:, OFF_WINT:OFF_WINT + 128]
            zq_tiles = [None, None]
            ae_parts = cpool.tile([128, 12], F32, tag="aeparts")
            pending = []

            def emit_unit(u):
                y2 = psy.tile([128, 1024], F32, tag="y")
                ujs = []
                for v in range(2):
                    j = 2 * u + v
                    Wj = xpool.tile([128, 128], BF16, tag="x")
                    nc.gpsimd.tensor_scalar_mul(Wj[:], wint_ap, tokT[:, j:j + 1])
                    nc.tensor.matmul(y2[:, 512 * v:512 * (v + 1)], Wj[:], atomsT[:],
                                     start=True, stop=True)
                    if ENG[u] == 'B':
                        uj = upool.tile([128, 2], BF16, tag="u")
                        nc.gpsimd.tensor_scalar_mul(uj[:], upeg_ap, tokT[:, j:j + 1])
                        ujs.append(uj)
                return (u, y2, ujs)

            def flush_unit(ent):
                u, y2, ujs = ent
                h = hpool.tile([128, 1024], F32R, tag="h")
                if ENG[u] == 'A':
                    nc.scalar.activation(h[:], y2[:], AF.Prelu, bias=bias(BI_INT),
                                         alpha=0.01)
                else:
                    # h = 0.99*relu(y); the 0.01*y linear part of lrelu is
                    # folded into the zq accumulation via upeg below
                    nc.vector.tensor_scalar(h[:], y2[:], 0.0, 0.99,
                                            op0=ALU.max, op1=ALU.mult)
                for v in range(2):
                    j = 2 * u + v
                    b, jj = j // 64, j % 64
                    if zq_tiles[b] is None:
                        zq_tiles[b] = psz.tile([128, 512], F32, tag="z", name=f"zq{b}")
                    zq = zq_tiles[b]
                    for a in range(4):
                        cols = zq[:, 8 * jj + 2 * a:8 * jj + 2 * a + 2]
                        if ENG[u] == 'A':
                            nc.tensor.matmul(cols, h[:, 512 * v + 128 * a:512 * v + 128 * (a + 1)],
                                             wpeg_ap, start=True, stop=True)
                        else:
                            nc.tensor.matmul(cols, h[:, 512 * v + 128 * a:512 * v + 128 * (a + 1)],
                                             wpeg_ap, start=True, stop=False)
                            nc.tensor.matmul(cols, atomsT[:, 128 * a:128 * (a + 1)],
                                             ujs[v][:], start=False, stop=True)

            def gates(b, c0, c1, slot):
                # process zq cols [c0:c1] -> ae_parts cols 4*slot : 4*slot+4
                zq = zq_tiles[b]
                n2 = (c1 - c0) // 2
                s = gpool.tile([128, 256], F32, tag="s")
                nc.scalar.activation(s[:, 0:n2], zq[:, c0 + 1:c1:2], AF.Tanh,
                                     bias=0.5 * bpg, scale=0.5)
                w = gpool.tile([128, 256], F32, tag="w")
                nc.gpsimd.tensor_scalar(w[:, 0:n2], s[:, 0:n2], 0.5, 0.5,
                                        op0=ALU.mult, op1=ALU.add)
                t = gpool.tile([128, 256], F32, tag="t")
                nc.vector.scalar_tensor_tensor(t[:, 0:n2], zq[:, c0:c1:2], bpe, w[:, 0:n2],
                                               op0=ALU.add, op1=ALU.mult)
                for a in range(4):
                    junka = jpool.tile([128, 64], F32, tag="junka")
                    nc.vector.tensor_scalar(junka[:, 0:n2 // 4], t[:, a:n2:4], 1.0, 0.0,
                                            op0=ALU.mult, op1=ALU.add,
                                            accum_out=ae_parts[:, 4 * slot + a:
                                                              4 * slot + a + 1])

            for u in range(64):
                pending.append(emit_unit(u))
                if len(pending) > 1:
                    flush_unit(pending.pop(0))
                fu = u - 1  # unit just flushed
                if fu == 15:
                    gates(0, 0)
                elif fu == 31:
                    gates(0, 1)
                elif fu == 47:
                    gates(1, 0)
                if fu in TASK_AT:
                    TASK_AT[fu]()

            flush_unit(pending.pop(0))
            gates(1, 1)

            # atom_e reduce -> seg matmul -> out
            ae8 = prepool.tile([128, 8], F32, tag="ae8")
            nc.gpsimd.tensor_tensor(ae8[:], ae_parts[:, 0:8], ae_parts[:, 4:12], op=ALU.add)
            ae4b = prepool.tile([128, 4], BF16, tag="ae4b")
            nc.gpsimd.tensor_tensor(ae4b[:], ae8[:, 0:4], ae_parts[:, 8:12], op=ALU.add)
            ps_seg = pspre.tile([1, NG], F32, tag="pre")
            for q in range(4):
                nc.tensor.matmul(ps_seg[:], ae4b[:, q:q + 1], Stsb[:, q * NG:(q + 1) * NG],
                                 start=(q == 0), stop=(q == 3))
            nc.scalar.activation(res[:, 0:NG], ps_seg[:], AF.Copy)
            nc.sync.dma_start(d_res[:], res[:])

    _legalize_waits(nc)
    nc._tile_ctx = tc_ref
    return nc


def kernel(**inputs) -> np.ndarray:
    f = lambda a: np.ascontiguousarray(np.asarray(a), dtype=np.float32)
    bf = lambda a: np.ascontiguousarray(np.asarray(a, dtype=np.float32)).astype(ml_dtypes.bfloat16)
    tf = f(inputs["token_features"])
    la = f(inputs["lig_atom"])
    lgr = f(inputs["lig_graph"])
    m0 = f(inputs["ms_feat_0"])
    m1 = f(inputs["ms_feat_1"])
    lb = np.asarray(inputs["ligand_batch"])
    S = (lb[:, None] == np.arange(NG)[None, :]).astype(np.float32)

    # ---- weight prep (host-side layout/scale transforms only) ----
    wint_bf = bf(inputs["W_int"])                       # [128,128]
    wpe = f(inputs["W_pe"]); wpg = f(inputs["W_pg"])    # [128,1]
    wpeg = np.concatenate([wpe, wpg], axis=1)           # [128,2]
    u_pe = wint_bf.astype(np.float64) @ wpe.astype(np.float64)
    u_pg = wint_bf.astype(np.float64) @ wpg.astype(np.float64)
    upeg = 0.01 * np.concatenate([u_pe, u_pg], axis=1)  # [128,2]

    wcat = f(inputs["W_cat"]).copy()                    # [384,128]
    wgate = f(inputs["W_gate"]).copy()
    wcat[2 * HID:] /= float(NT)
    wgate[2 * HID:] /= float(NT)

    WB = np.zeros((128, NWB), dtype=np.float32)
    WB[:, OFF_WINT:OFF_WINT + 128] = wint_bf.astype(np.float32)
    WB[:, OFF_WTOK:OFF_WTOK + 256] = f(inputs["W_token"]).reshape(2, 128, HID).transpose(1, 0, 2).reshape(128, 256)
    WB[:, OFF_WPK:OFF_WPK + 256] = f(inputs["W_pocket"]).reshape(2, 128, HID).transpose(1, 0, 2).reshape(128, 256)
    WB[:, OFF_WCAT:OFF_WCAT + 384] = wcat.reshape(3, 128, HID).transpose(1, 0, 2).reshape(128, 384)
    WB[:, OFF_WGATE:OFF_WGATE + 384] = wgate.reshape(3, 128, HID).transpose(1, 0, 2).reshape(128, 384)
    WB[:, OFF_WB1:OFF_WB1 + 256] = f(inputs["W_bias1"]).reshape(2, 128, HID).transpose(1, 0, 2).reshape(128, 256)
    WB[:, OFF_WB2:OFF_WB2 + 1] = f(inputs["W_bias2"])
    WB[:, OFF_WPEG:OFF_WPEG + 2] = wpeg
    WB[:, OFF_UPEG:OFF_UPEG + 2] = upeg
    WB_bf = WB.astype(ml_dtypes.bfloat16)

    # conv weights as [c, off*128 + o], scaled by 1/num_output_positions
    Wc0 = f(inputs["Wc0"])  # [128,64,3,3,3] applied to ms_feat_1
    Wc1 = f(inputs["Wc1"])  # [128,32,3,3,3] applied to ms_feat_0
    W0T = np.ascontiguousarray(Wc0.reshape(128, 64, 27).transpose(1, 2, 0)).reshape(64, 27 * 128) / 216.0
    W32 = np.ascontiguousarray(Wc1.reshape(128, 32, 27).transpose(1, 2, 0)).reshape(32, 27 * 128) / 2744.0

    W64 = np.zeros((64, 256), dtype=np.float32)
    W64[:, 0:128] = f(inputs["W_atom"])
    W64[:, 128:256] = f(inputs["W_graph"])

    col = lambda a: f(a).reshape(128, 1)
    BI = np.zeros((128, NBI), dtype=np.float32)
    BI[:, BI_TOK] = f(inputs["b_token"])
    BI[:, BI_ATOM] = f(inputs["b_atom"])
    BI[:, BI_INT] = f(inputs["b_int"])
    BI[:, BI_PK] = f(inputs["b_pocket"])
    BI[:, BI_CAT] = f(inputs["b_cat"])
    BI[:, BI_GH] = 0.5 * f(inputs["b_gate"])
    BI[:, BI_GR] = f(inputs["b_graph"])
    BI[:, BI_B1] = f(inputs["b_bias1"])
    BI[:, BI_C0] = f(inputs["bc0"])
    BI[:, BI_C1] = f(inputs["bc1"])
    BI[:, BI_WPEG:BI_WPEG + 2] = wpeg

    # window-membership masks: M[pos, off] = 1 iff pos-off in valid out range
    def win_mask(D, O):
        g = np.arange(D)
        z, y, x = np.meshgrid(g, g, g, indexing="ij")
        pos = np.stack([z.ravel(), y.ravel(), x.ravel()], 1)  # [D^3, 3]
        d = np.arange(3)
        dz, dy, dx = np.meshgrid(d, d, d, indexing="ij")
        off = np.stack([dz.ravel(), dy.ravel(), dx.ravel()], 1)  # [27, 3]
        r = pos[:, None, :] - off[None, :, :]
        return np.all((r >= 0) & (r < O), axis=2).astype(np.float32)  # [D^3, 27]

    M1 = win_mask(16, 14).reshape(32, 128, 27)
    M0 = win_mask(8, 6).reshape(4, 128, 27)

    bpe = float(np.asarray(inputs["b_pe"]).reshape(-1)[0])
    bpg = float(np.asarray(inputs["b_pg"]).reshape(-1)[0])
    bb2 = float(np.asarray(inputs["b_bias2"]).reshape(-1)[0])

    shared = {
        "WB": WB_bf, "BI": BI,
        "W64": W64.astype(ml_dtypes.bfloat16),
        "W0T": W0T.astype(ml_dtypes.bfloat16),
        "W32": W32.astype(ml_dtypes.bfloat16),
        "M1m": M1.astype(ml_dtypes.bfloat16),
        "M0m": M0.astype(ml_dtypes.bfloat16),
    }

    in_maps = []
    for c in range(NCORES):
        n, h = c // 2, c % 2
        m = dict(shared)
        m["tfX"] = np.ascontiguousarray(tf[n].T.reshape(2, 128, 128))
        m["laT"] = bf(la[n, 512 * h:512 * (h + 1)].T)
        m["lgT"] = bf(lgr[n].T)
        m["m0T"] = bf(m0[n].reshape(32, 4096).T.reshape(32, 128, 32))
        m["m1T"] = bf(m1[n].reshape(64, 512).T.reshape(4, 128, 64))
        m["Sh"] = bf(S[512 * h:512 * (h + 1)].reshape(4, 128, NG))
        in_maps.append(m)

    bint_zero = bool(np.all(np.asarray(inputs["b_int"]) == 0.0))
    nc = build_program(bpe, bpg, bb2, bint_zero)
    r = run_bass_kernel_spmd(nc, in_maps, core_ids=list(range(NCORES)),
                             trace=TRACE, **(TRACE_KW if TRACE else {}))
    global LAST
    LAST = r
    res = r.results

    out = np.zeros((NI, NG), dtype=np.float32)
    for n in range(NI):
        out[n] = (res[2 * n]["res_out"][0, 0:NG] + res[2 * n + 1]["res_out"][0, 0:NG]
                  + res[2 * n]["res_out"][0, NG:2 * NG])
    return out


# revision 19
# speedup vs baseline: 2.0948x; 1.0005x over previous
import sys
import numpy as np
import ml_dtypes

sys.path.insert(0, "/opt/trn_rl_repo")

import concourse.bass as bass
import concourse.tile as tile
from concourse import mybir
from concourse.bass_utils import run_bass_kernel_spmd

F32 = mybir.dt.float32
BF16 = mybir.dt.bfloat16
AF = mybir.ActivationFunctionType
ALU = mybir.AluOpType

HID = 128
NT = 128       # tokens per image
NAH = 512      # atoms per core (half of 1024)
NG = 64        # ligand graphs
NI = 4         # images
NCORES = 8

# WB (128-partition weight concat, bf16) column offsets
OFF_WINT = 0
OFF_WTOK = 128
OFF_WPK = 384
OFF_WCAT = 640
OFF_WGATE = 1024
OFF_WB1 = 1408
OFF_WB2 = 1664
OFF_WPEG = 1665
OFF_UPEG = 1667
NWB = 1669

# BI (f32 bias concat) columns
BI_TOK, BI_ATOM, BI_INT, BI_PK, BI_CAT, BI_GH, BI_GR, BI_B1, BI_C0, BI_C1 = range(10)
BI_WPEG = 10   # cols 10:12 = [W_pe, W_pg] f32
NBI = 12

# lrelu unit assignment: 'A' = ACT Prelu, 'B' = DVE relu99 + linear-fold
N_A_UNITS = 32

TRACE = False
TRACE_KW = {}
LAST = None


_COMPUTE_INSTS = (
    "InstActivation", "InstTensorCopy", "InstTensorScalar", "InstTensorScalarPtr",
    "InstTensorTensor", "InstTensorTensorReduce", "InstTensorReduce", "InstMemSet",
    "InstMatmult", "InstScalarTensorTensor", "InstTensorTensorScan", "InstLdweights",
    "InstDMACopy", "InstDMATransposeAnt", "InstTriggeredCopy", "InstDrain",
    "InstEventSemaphoreOp", "InstSemaphoreOp", "InstCopy", "InstIota", "InstSelect",
)


def _legalize_waits(nc):
    # walrus in this toolchain accepts at most ONE sync wait on TPB compute
    # instructions; hoist extras into same-engine NoOps placed just before.
    k = 0
    for f in nc.m.functions:
        for blk in f.blocks:
            insts = blk.instructions
            out = []
            for ins in insts:
                si = getattr(ins, "sync_info", None)
                if (si is not None and len(si.on_wait) > 1
                        and type(ins).__name__ in _COMPUTE_INSTS):
                    waits = list(si.on_wait)
                    for w in waits[:-1]:
                        nop = mybir.InstNoOp(
                            name=f"WNOP-{k}", engine=ins.engine,
                            sync_info=mybir.SyncInfo(on_wait=[w], on_update=[]))
                        k += 1
                        out.append(nop)
                    ins.sync_info = mybir.SyncInfo(on_wait=[waits[-1]],
                                                   on_update=list(si.on_update))
                out.append(ins)
            blk.instructions = out
    return k


def _register_const(nc, val, dtype=F32):
    if (dtype, float(val)) in nc.const_aps.aps:
        return
    t = nc.alloc_sbuf_tensor(f"uconst-{dtype.name}-{val}", [128, 1], dtype)
    nc.gpsimd.memset(t.ap(), float(val))
    nc.const_aps.aps[(dtype, float(val))] = t.ap()


def _unit_engines():
    # interleave N_A_UNITS 'A' units among 64 as evenly as possible
    eng = []
    for u in range(64):
        if (u + 1) * N_A_UNITS // 64 > u * N_A_UNITS // 64:
            eng.append('A')
        else:
            eng.append('B')
    return eng


def build_program(bpe: float, bpg: float, bb2: float, bint_zero: bool = True,
                  sim_trace: bool = False) -> bass.Bass:
    nc = bass.Bass()
    _register_const(nc, 0.5 * bpg)
    _register_const(nc, bb2)
    nc.all_engine_barrier()

    # ---- DRAM inputs (per-core views; same names across SPMD cores) ----
    d_WB = nc.dram_tensor("WB", [128, NWB], BF16, kind="ExternalInput")
    d_BI = nc.dram_tensor("BI", [128, NBI], F32, kind="ExternalInput")
    d_tfX = nc.dram_tensor("tfX", [2, 128, 128], F32, kind="ExternalInput")
    d_laT = nc.dram_tensor("laT", [64, NAH], BF16, kind="ExternalInput")
    d_W64 = nc.dram_tensor("W64", [64, 256], BF16, kind="ExternalInput")
    d_m0T = nc.dram_tensor("m0T", [32, 128, 32], BF16, kind="ExternalInput")
    d_M1 = nc.dram_tensor("M1m", [32, 128, 27], BF16, kind="ExternalInput")
    d_m1T = nc.dram_tensor("m1T", [4, 128, 64], BF16, kind="ExternalInput")
    d_M0 = nc.dram_tensor("M0m", [4, 128, 27], BF16, kind="ExternalInput")
    d_W0T = nc.dram_tensor("W0T", [64, 27 * 128], BF16, kind="ExternalInput")
    d_W32 = nc.dram_tensor("W32", [32, 27 * 128], BF16, kind="ExternalInput")
    d_lgT = nc.dram_tensor("lgT", [64, NG], BF16, kind="ExternalInput")
    d_Sh = nc.dram_tensor("Sh", [4, 128, NG], BF16, kind="ExternalInput")

    d_res = nc.dram_tensor("res_out", [1, 128], F32, kind="ExternalOutput")

    ENG = _unit_engines()
    if not bint_zero:
        ENG[:] = ['A'] * 64

    tc_ref = tile.TileContext(nc, trace_sim=sim_trace)
    with tc_ref as tc:
        with (
            tc.tile_pool(name="const", bufs=1) as cpool,
            tc.tile_pool(name="pre", bufs=1) as prepool,
            tc.tile_pool(name="x", bufs=8) as xpool,
            tc.tile_pool(name="u", bufs=4) as upool,
            tc.tile_pool(name="h", bufs=6) as hpool,
            tc.tile_pool(name="g", bufs=2) as gpool,
            tc.tile_pool(name="j", bufs=2) as jpool,
            tc.tile_pool(name="ps_y", bufs=3, space="PSUM") as psy,
            tc.tile_pool(name="ps_z", bufs=1, space="PSUM") as psz,
            tc.tile_pool(name="ps_p", bufs=1, space="PSUM") as pspre,
        ):
            # ---------- engine warmups (hide ACT table load + start PE pstate clock)
            warm = cpool.tile([128, 1], F32, tag="warm")
            nc.gpsimd.memset(warm[:], 0.0)
            warma = cpool.tile([128, 1], F32, tag="warma")
            nc.scalar.activation(warma[:], warm[:], AF.Silu)
            ps_warm = pspre.tile([1, 1], F32, tag="pre")
            nc.tensor.matmul(ps_warm[:], warm[:], warm[:], start=True, stop=True)
            warmb = cpool.tile([1, 1], F32, tag="warmb")
            nc.scalar.activation(warmb[:], ps_warm[:], AF.Copy)

            # ---------- input DMAs (order = DMA device service priority) ----
            tfx = prepool.tile([128, 256], F32, tag="tfx")
            nc.sync.dma_start(tfx[:, :].rearrange("p (c k) -> p c k", c=2),
                              d_tfX[:, :, :].rearrange("c p k -> p c k"))
            la = prepool.tile([64, NAH], BF16, tag="la")
            nc.sync.dma_start(la[:], d_laT[:])
            W64sb = cpool.tile([64, 256], BF16, tag="W64")
            nc.sync.dma_start(W64sb[:], d_W64[:])
            BIsb = cpool.tile([128, NBI], F32, tag="BI")
            nc.sync.dma_start(BIsb[:], d_BI[:])
            WBsb = cpool.tile([128, NWB], BF16, tag="WB")
            nc.sync.dma_start(WBsb[:], d_WB[:])
            m0sb = cpool.tile([128, 1024], BF16, tag="m0")
            nc.sync.dma_start(m0sb[:, :].rearrange("p (u c) -> p u c", u=32),
                              d_m0T[:, :, :].rearrange("u p c -> p u c"))
            M1sb = cpool.tile([128, 864], BF16, tag="M1")
            nc.sync.dma_start(M1sb[:, :].rearrange("p (u o) -> p u o", u=32),
                              d_M1[:, :, :].rearrange("u p o -> p u o"))
            m1sb = cpool.tile([128, 256], BF16, tag="m1")
            nc.sync.dma_start(m1sb[:, :].rearrange("p (u c) -> p u c", u=4),
                              d_m1T[:, :, :].rearrange("u p c -> p u c"))
            M0sb = cpool.tile([128, 108], BF16, tag="M0")
            nc.sync.dma_start(M0sb[:, :].rearrange("p (u o) -> p u o", u=4),
                              d_M0[:, :, :].rearrange("u p o -> p u o"))
            W0Tsb = cpool.tile([64, 27 * 128], BF16, tag="W0T")
            nc.sync.dma_start(W0Tsb[:], d_W0T[:])
            W32sb = cpool.tile([32, 27 * 128], BF16, tag="W32")
            nc.sync.dma_start(W32sb[:], d_W32[:])
            lg = cpool.tile([64, NG], BF16, tag="lg")
            nc.sync.dma_start(lg[:], d_lgT[:])
            Stsb = cpool.tile([128, 4 * NG], BF16, tag="St")
            nc.sync.dma_start(Stsb[:, :].rearrange("p (q g) -> p q g", q=4),
                              d_Sh[:, :, :].rearrange("q p g -> p q g"))
            F32R = mybir.dt.float32r

            bias = lambda i: BIsb[:, i:i + 1]

            # ---------- preamble: tok / atoms (needed before main loop) -----
            tfr = prepool.tile([128, 256], BF16, tag="tfr")
            nc.scalar.activation(tfr[:], tfx[:], AF.Silu)
            ps_tok = pspre.tile([128, 128], F32, tag="pre")
            nc.tensor.matmul(ps_tok[:], WBsb[:, OFF_WTOK:OFF_WTOK + 128],
                             tfr[:, 0:128], start=True, stop=False)
            nc.tensor.matmul(ps_tok[:], WBsb[:, OFF_WTOK + 128:OFF_WTOK + 256],
                             tfr[:, 128:256], start=False, stop=True)
            tokT = cpool.tile([128, NT], F32, tag="tokT")
            nc.scalar.activation(tokT[:], ps_tok[:], AF.Identity, bias=bias(BI_TOK))

            ps_at = pspre.tile([128, NAH], F32, tag="pre")
            nc.tensor.matmul(ps_at[:], W64sb[:, 0:128], la[:], start=True, stop=True)
            atomsT = cpool.tile([128, NAH], BF16, tag="atomsT")
            nc.scalar.activation(atomsT[:], ps_at[:], AF.Identity, bias=bias(BI_ATOM))

            # ---------- deferred preamble tasks (interleaved into loop) ----
            state = {}

            def task_silu1():
                s0 = cpool.tile([128, 1024], BF16, tag="s0")
                nc.scalar.activation(s0[:], m0sb[:], AF.Silu)
                state["s0"] = s0

            def task_S1():
                S1 = pspre.tile([32, 27], F32, tag="pre")
                for u in range(32):
                    nc.tensor.matmul(S1[:], state["s0"][:, 32 * u:32 * u + 32],
                                     M1sb[:, 27 * u:27 * u + 27],
                                     start=(u == 0), stop=(u == 31))
                S1b = prepool.tile([32, 27], BF16, tag="S1b")
                nc.scalar.activation(S1b[:], S1[:], AF.Copy)
                state["S1b"] = S1b

            def task_p1():
                pp = pspre.tile([128, 1], F32, tag="pre")
                for o in range(27):
                    nc.tensor.matmul(pp[:], W32sb[:, 128 * o:128 * o + 128],
                                     state["S1b"][:, o:o + 1],
                                     start=(o == 0), stop=(o == 26))
                sp1 = prepool.tile([128, 1], BF16, tag="sp1")
                nc.scalar.activation(sp1[:], pp[:], AF.Silu, bias=bias(BI_C1))
                state["sp1"] = sp1

            def task_silu0():
                s1 = prepool.tile([128, 256], BF16, tag="s1")
                nc.scalar.activation(s1[:], m1sb[:], AF.Silu)
                state["s1"] = s1

            def task_S0():
                S0 = pspre.tile([64, 27], F32, tag="pre")
                for u in range(4):
                    nc.tensor.matmul(S0[:], state["s1"][:, 64 * u:64 * u + 64],
                                     M0sb[:, 27 * u:27 * u + 27],
                                     start=(u == 0), stop=(u == 3))
                S0b = prepool.tile([64, 27], BF16, tag="S0b")
                nc.scalar.activation(S0b[:], S0[:], AF.Copy)
                state["S0b"] = S0b

            def task_p0():
                pp = pspre.tile([128, 1], F32, tag="pre")
                for o in range(27):
                    nc.tensor.matmul(pp[:], W0Tsb[:, 128 * o:128 * o + 128],
                                     state["S0b"][:, o:o + 1],
                                     start=(o == 0), stop=(o == 26))
                sp0 = prepool.tile([128, 1], BF16, tag="sp0")
                nc.scalar.activation(sp0[:], pp[:], AF.Silu, bias=bias(BI_C0))
                state["sp0"] = sp0

            def task_pocket():
                ps_pk = pspre.tile([128, 1], F32, tag="pre")
                nc.tensor.matmul(ps_pk[:], WBsb[:, OFF_WPK:OFF_WPK + 128],
                                 state["sp0"][:], start=True, stop=False)
                nc.tensor.matmul(ps_pk[:], WBsb[:, OFF_WPK + 128:OFF_WPK + 256],
                                 state["sp1"][:], start=False, stop=True)
                pocket = prepool.tile([128, 1], BF16, tag="pocket")
                nc.scalar.activation(pocket[:], ps_pk[:], AF.Identity, bias=bias(BI_PK))
                state["pocket"] = pocket

            def task_pf():
                tok_sum = prepool.tile([128, 1], F32, tag="toksum")
                junkt = jpool.tile([128, NT], F32, tag="junk")
                nc.vector.tensor_scalar(junkt[:], tokT[:], 1.0, 0.0, op0=ALU.mult,
                                        op1=ALU.add, accum_out=tok_sum[:])
                tok_sum_b = prepool.tile([128, 1], BF16, tag="toksumb")
                nc.scalar.activation(tok_sum_b[:], tok_sum[:], AF.Copy)
                ps_pf = pspre.tile([128, 2], F32, tag="pre")
                chunks = [state["pocket"], tok_sum_b, tok_sum_b]
                for q in range(3):
                    nc.tensor.matmul(ps_pf[:, 0:1],
                                     WBsb[:, OFF_WCAT + 128 * q:OFF_WCAT + 128 * (q + 1)],
                                     chunks[q][:], start=(q == 0), stop=(q == 2))
                for q in range(3):
                    nc.tensor.matmul(ps_pf[:, 1:2],
                                     WBsb[:, OFF_WGATE + 128 * q:OFF_WGATE + 128 * (q + 1)],
                                     chunks[q][:], start=(q == 0), stop=(q == 2))
                # sigmoid(z + bg) = 0.5 + 0.5*tanh(0.5z + 0.5bg)
                gt = prepool.tile([128, 1], F32, tag="gt")
                nc.scalar.activation(gt[:], ps_pf[:, 1:2], AF.Tanh,
                                     bias=bias(BI_GH), scale=0.5)
                pf_sig = prepool.tile([128, 1], F32, tag="pfsig")
                nc.gpsimd.tensor_scalar(pf_sig[:], gt[:], 0.5, 0.5, op0=ALU.mult, op1=ALU.add)
                pf_lin = prepool.tile([128, 1], F32, tag="pflin")
                nc.scalar.activation(pf_lin[:], ps_pf[:, 0:1], AF.Identity, bias=bias(BI_CAT))
                pf = prepool.tile([128, 1], BF16, tag="pf")
                nc.gpsimd.tensor_tensor(pf[:], pf_lin[:], pf_sig[:], op=ALU.mult)
                state["pf"] = pf

            def task_gf():
                ps_gf = pspre.tile([128, NG], F32, tag="pre")
                nc.tensor.matmul(ps_gf[:], W64sb[:, 128:256], lg[:], start=True, stop=True)
                gfT = prepool.tile([128, NG], BF16, tag="gfT")
                nc.scalar.activation(gfT[:], ps_gf[:], AF.Identity, bias=bias(BI_GR))
                state["gfT"] = gfT

            def task_bias1():
                ps_u = pspre.tile([128, 1], F32, tag="pre")
                nc.tensor.matmul(ps_u[:], WBsb[:, OFF_WB1:OFF_WB1 + 128],
                                 state["pf"][:], start=True, stop=True)
                ub = prepool.tile([128, 1], F32, tag="ub")
                nc.scalar.activation(ub[:], ps_u[:], AF.Identity, bias=bias(BI_B1))
                ps_hb = pspre.tile([128, NG], F32, tag="pre")
                nc.tensor.matmul(ps_hb[:], WBsb[:, OFF_WB1 + 128:OFF_WB1 + 256],
                                 state["gfT"][:], start=True, stop=True)
                hb = prepool.tile([128, NG], BF16, tag="hb")
                nc.scalar.activation(hb[:], ps_hb[:], AF.Prelu, bias=ub[:], alpha=0.01)
                state["hb"] = hb

            def task_bias2():
                ps_b2 = pspre.tile([1, NG], F32, tag="pre")
                nc.tensor.matmul(ps_b2[:], WBsb[:, OFF_WB2:OFF_WB2 + 1],
                                 state["hb"][:], start=True, stop=True)
                nc.scalar.activation(res[:, NG:2 * NG], ps_b2[:], AF.Identity, bias=bb2)

            pre_tasks = [task_silu1, task_S1, task_p1, task_silu0, task_S0,
                         task_p0, task_pocket, task_pf, task_gf, task_bias1,
                         task_bias2]
            TASK_AT = {12 + 4 * i: t for i, t in enumerate(pre_tasks)}

            res = cpool.tile([1, 128], F32, tag="res")

            # ---------- main loop ----------
            # 64 units u of 2 tokens; y2[o, 512v + a] for token j = 2u+v.
            # zq8 (per 64-token block) col layout: 8*(j%64) + 2*a_chunk + {pe,pg}
            wpegr = cpool.tile([128, 2], F32R, tag="wpegr")
            nc.scalar.activation(wpegr[:], BIsb[:, BI_WPEG:BI_WPEG + 2], AF.Copy)
            wpeg_ap = wpegr[:]
            upeg_ap = WBsb[:, OFF_UPEG:OFF_UPEG + 2]
            wint_ap = WBsb[:, OFF_WINT:OFF_WINT + 128]
            zq_tiles = [None, None]
            ae_parts = cpool.tile([128, 12], F32, tag="aeparts")
            pending = []

            def emit_unit(u):
                y2 = psy.tile([128, 1024], F32, tag="y")
                ujs = []
                for v in range(2):
                    j = 2 * u + v
                    Wj = xpool.tile([128, 128], BF16, tag="x")
                    nc.gpsimd.tensor_scalar_mul(Wj[:], wint_ap, tokT[:, j:j + 1])
                    nc.tensor.matmul(y2[:, 512 * v:512 * (v + 1)], Wj[:], atomsT[:],
                                     start=True, stop=True)
                    if ENG[u] == 'B':
                        uj = upool.tile([128, 2], BF16, tag="u")
                        nc.gpsimd.tensor_scalar_mul(uj[:], upeg_ap, tokT[:, j:j + 1])
                        ujs.append(uj)
                return (u, y2, ujs)

            def flush_unit(ent):
                u, y2, ujs = ent
                h = hpool.tile([128, 1024], F32R, tag="h")
                if ENG[u] == 'A':
                    nc.scalar.activation(h[:], y2[:], AF.Prelu, bias=bias(BI_INT),
                                         alpha=0.01)
                else:
                    # h = 0.99*relu(y); the 0.01*y linear part of lrelu is
                    # folded into the zq accumulation via upeg below
                    nc.vector.tensor_scalar(h[:], y2[:], 0.0, 0.99,
                                            op0=ALU.max, op1=ALU.mult)
                for v in range(2):
                    j = 2 * u + v
                    b, jj = j // 64, j % 64
                    if zq_tiles[b] is None:
                        zq_tiles[b] = psz.tile([128, 512], F32, tag="z", name=f"zq{b}")
                    zq = zq_tiles[b]
                    for a in range(4):
                        cols = zq[:, 8 * jj + 2 * a:8 * jj + 2 * a + 2]
                        if ENG[u] == 'A':
                            nc.tensor.matmul(cols, h[:, 512 * v + 128 * a:512 * v + 128 * (a + 1)],
                                             wpeg_ap, start=True, stop=True)
                        else:
                            nc.tensor.matmul(cols, h[:, 512 * v + 128 * a:512 * v + 128 * (a + 1)],
                                             wpeg_ap, start=True, stop=False)
                            nc.tensor.matmul(cols, atomsT[:, 128 * a:128 * (a + 1)],
                                             ujs[v][:], start=False, stop=True)

            def gates(b, c0, c1, slot):
                # process zq cols [c0:c1] -> ae_parts cols 4*slot : 4*slot+4
                zq = zq_tiles[b]
                n2 = (c1 - c0) // 2
                s = gpool.tile([128, 256], F32, tag="s")
                nc.scalar.activation(s[:, 0:n2], zq[:, c0 + 1:c1:2], AF.Tanh,
                                     bias=0.5 * bpg, scale=0.5)
                w = gpool.tile([128, 256], F32, tag="w")
                nc.gpsimd.tensor_scalar(w[:, 0:n2], s[:, 0:n2], 0.5, 0.5,
                                        op0=ALU.mult, op1=ALU.add)
                t = gpool.tile([128, 256], F32, tag="t")
                nc.vector.scalar_tensor_tensor(t[:, 0:n2], zq[:, c0:c1:2], bpe, w[:, 0:n2],
                                               op0=ALU.add, op1=ALU.mult)
                for a in range(4):
                    junka = jpool.tile([128, 64], F32, tag="junka")
                    nc.vector.tensor_scalar(junka[:, 0:n2 // 4], t[:, a:n2:4], 1.0, 0.0,
                                            op0=ALU.mult, op1=ALU.add,
                                            accum_out=ae_parts[:, 4 * slot + a:
                                                              4 * slot + a + 1])

            for u in range(64):
                pending.append(emit_unit(u))
                if len(pending) > 1:
                    flush_unit(pending.pop(0))
                fu = u - 1  # unit just flushed
                if fu == 15:
                    gates(0, 0)
                elif fu == 31:
                    gates(0, 1)
                elif fu == 47:
                    gates(1, 0)
                if fu in TASK_AT:
                    _old_pri = tc.cur_priority
                    tc.cur_priority = _old_pri + 100000
                    TASK_AT[fu]()
                    tc.cur_priority = _old_pri

            flush_unit(pending.pop(0))
            gates(1, 1)

            # atom_e reduce -> seg matmul -> out
            ae8 = prepool.tile([128, 8], F32, tag="ae8")
            nc.gpsimd.tensor_tensor(ae8[:], ae_parts[:, 0:8], ae_parts[:, 4:12], op=ALU.add)
            ae4b = prepool.tile([128, 4], BF16, tag="ae4b")
            nc.gpsimd.tensor_tensor(ae4b[:], ae8[:, 0:4], ae_parts[:, 8:12], op=ALU.add)
            ps_seg = pspre.tile([1, NG], F32, tag="pre")
            for q in range(4):
                nc.tensor.matmul(ps_seg[:], ae4b[:, q:q + 1], Stsb[:, q * NG:(q + 1) * NG],
                                 start=(q == 0), stop=(q == 3))
            nc.scalar.activation(res[:, 0:NG], ps_seg[:], AF.Copy)
            nc.sync.dma_start(d_res[:], res[:])

    _legalize_waits(nc)
    nc._tile_ctx = tc_ref
    return nc


def kernel(**inputs) -> np.ndarray:
    f = lambda a: np.ascontiguousarray(np.asarray(a), dtype=np.float32)
    bf = lambda a: np.ascontiguousarray(np.asarray(a, dtype=np.float32)).astype(ml_dtypes.bfloat16)
    tf = f(inputs["token_features"])
    la = f(inputs["lig_atom"])
    lgr = f(inputs["lig_graph"])
    m0 = f(inputs["ms_feat_0"])
    m1 = f(inputs["ms_feat_1"])
    lb = np.asarray(inputs["ligand_batch"])
    S = (lb[:, None] == np.arange(NG)[None, :]).astype(np.float32)

    # ---- weight prep (host-side layout/scale transforms only) ----
    wint_bf = bf(inputs["W_int"])                       # [128,128]
    wpe = f(inputs["W_pe"]); wpg = f(inputs["W_pg"])    # [128,1]
    wpeg = np.concatenate([wpe, wpg], axis=1)           # [128,2]
    u_pe = wint_bf.astype(np.float64) @ wpe.astype(np.float64)
    u_pg = wint_bf.astype(np.float64) @ wpg.astype(np.float64)
    upeg = 0.01 * np.concatenate([u_pe, u_pg], axis=1)  # [128,2]

    wcat = f(inputs["W_cat"]).copy()                    # [384,128]
    wgate = f(inputs["W_gate"]).copy()
    wcat[2 * HID:] /= float(NT)
    wgate[2 * HID:] /= float(NT)

    WB = np.zeros((128, NWB), dtype=np.float32)
    WB[:, OFF_WINT:OFF_WINT + 128] = wint_bf.astype(np.float32)
    WB[:, OFF_WTOK:OFF_WTOK + 256] = f(inputs["W_token"]).reshape(2, 128, HID).transpose(1, 0, 2).reshape(128, 256)
    WB[:, OFF_WPK:OFF_WPK + 256] = f(inputs["W_pocket"]).reshape(2, 128, HID).transpose(1, 0, 2).reshape(128, 256)
    WB[:, OFF_WCAT:OFF_WCAT + 384] = wcat.reshape(3, 128, HID).transpose(1, 0, 2).reshape(128, 384)
    WB[:, OFF_WGATE:OFF_WGATE + 384] = wgate.reshape(3, 128, HID).transpose(1, 0, 2).reshape(128, 384)
    WB[:, OFF_WB1:OFF_WB1 + 256] = f(inputs["W_bias1"]).reshape(2, 128, HID).transpose(1, 0, 2).reshape(128, 256)
    WB[:, OFF_WB2:OFF_WB2 + 1] = f(inputs["W_bias2"])
    WB[:, OFF_WPEG:OFF_WPEG + 2] = wpeg
    WB[:, OFF_UPEG:OFF_UPEG + 2] = upeg
    WB_bf = WB.astype(ml_dtypes.bfloat16)

    # conv weights as [c, off*128 + o], scaled by 1/num_output_positions
    Wc0 = f(inputs["Wc0"])  # [128,64,3,3,3] applied to ms_feat_1
    Wc1 = f(inputs["Wc1"])  # [128,32,3,3,3] applied to ms_feat_0
    W0T = np.ascontiguousarray(Wc0.reshape(128, 64, 27).transpose(1, 2, 0)).reshape(64, 27 * 128) / 216.0
    W32 = np.ascontiguousarray(Wc1.reshape(128, 32, 27).transpose(1, 2, 0)).reshape(32, 27 * 128) / 2744.0

    W64 = np.zeros((64, 256), dtype=np.float32)
    W64[:, 0:128] = f(inputs["W_atom"])
    W64[:, 128:256] = f(inputs["W_graph"])

    col = lambda a: f(a).reshape(128, 1)
    BI = np.zeros((128, NBI), dtype=np.float32)
    BI[:, BI_TOK] = f(inputs["b_token"])
    BI[:, BI_ATOM] = f(inputs["b_atom"])
    BI[:, BI_INT] = f(inputs["b_int"])
    BI[:, BI_PK] = f(inputs["b_pocket"])
    BI[:, BI_CAT] = f(inputs["b_cat"])
    BI[:, BI_GH] = 0.5 * f(inputs["b_gate"])
    BI[:, BI_GR] = f(inputs["b_graph"])
    BI[:, BI_B1] = f(inputs["b_bias1"])
    BI[:, BI_C0] = f(inputs["bc0"])
    BI[:, BI_C1] = f(inputs["bc1"])
    BI[:, BI_WPEG:BI_WPEG + 2] = wpeg

    # window-membership masks: M[pos, off] = 1 iff pos-off in valid out range
    def win_mask(D, O):
        g = np.arange(D)
        z, y, x = np.meshgrid(g, g, g, indexing="ij")
        pos = np.stack([z.ravel(), y.ravel(), x.ravel()], 1)  # [D^3, 3]
        d = np.arange(3)
        dz, dy, dx = np.meshgrid(d, d, d, indexing="ij")
        off = np.stack([dz.ravel(), dy.ravel(), dx.ravel()], 1)  # [27, 3]
        r = pos[:, None, :] - off[None, :, :]
        return np.all((r >= 0) & (r < O), axis=2).astype(np.float32)  # [D^3, 27]

    M1 = win_mask(16, 14).reshape(32, 128, 27)
    M0 = win_mask(8, 6).reshape(4, 128, 27)

    bpe = float(np.asarray(inputs["b_pe"]).reshape(-1)[0])
    bpg = float(np.asarray(inputs["b_pg"]).reshape(-1)[0])
    bb2 = float(np.asarray(inputs["b_bias2"]).reshape(-1)[0])

    shared = {
        "WB": WB_bf, "BI": BI,
        "W64": W64.astype(ml_dtypes.bfloat16),
        "W0T": W0T.astype(ml_dtypes.bfloat16),
        "W32": W32.astype(ml_dtypes.bfloat16),
        "M1m": M1.astype(ml_dtypes.bfloat16),
        "M0m": M0.astype(ml_dtypes.bfloat16),
    }

    in_maps = []
    for c in range(NCORES):
        n, h = c // 2, c % 2
        m = dict(shared)
        m["tfX"] = np.ascontiguousarray(tf[n].T.reshape(2, 128, 128))
        m["laT"] = bf(la[n, 512 * h:512 * (h + 1)].T)
        m["lgT"] = bf(lgr[n].T)
        m["m0T"] = bf(m0[n].reshape(32, 4096).T.reshape(32, 128, 32))
        m["m1T"] = bf(m1[n].reshape(64, 512).T.reshape(4, 128, 64))
        m["Sh"] = bf(S[512 * h:512 * (h + 1)].reshape(4, 128, NG))
        in_maps.append(m)

    bint_zero = bool(np.all(np.asarray(inputs["b_int"]) == 0.0))
    nc = build_program(bpe, bpg, bb2, bint_zero)
    r = run_bass_kernel_spmd(nc, in_maps, core_ids=list(range(NCORES)),
                             trace=TRACE, **(TRACE_KW if TRACE else {}))
    global LAST
    LAST = r
    res = r.results

    out = np.zeros((NI, NG), dtype=np.float32)
    for n in range(NI):
        out[n] = (res[2 * n]["res_out"][0, 0:NG] + res[2 * n + 1]["res_out"][0, 0:NG]
                  + res[2 * n]["res_out"][0, NG:2 * NG])
    return out


# revision 20
# speedup vs baseline: 2.0995x; 1.0022x over previous
import sys
import numpy as np
import ml_dtypes

sys.path.insert(0, "/opt/trn_rl_repo")

import concourse.bass as bass
import concourse.tile as tile
from concourse import mybir
from concourse.bass_utils import run_bass_kernel_spmd

F32 = mybir.dt.float32
BF16 = mybir.dt.bfloat16
AF = mybir.ActivationFunctionType
ALU = mybir.AluOpType

HID = 128
NT = 128       # tokens per image
NAH = 512      # atoms per core (half of 1024)
NG = 64        # ligand graphs
NI = 4         # images
NCORES = 8

# WB (128-partition weight concat, bf16) column offsets
OFF_WINT = 0
OFF_WTOK = 128
OFF_WPK = 384
OFF_WCAT = 640
OFF_WGATE = 1024
OFF_WB1 = 1408
OFF_WB2 = 1664
OFF_WPEG = 1665
OFF_UPEG = 1667
NWB = 1669

# BI (f32 bias concat) columns
BI_TOK, BI_ATOM, BI_INT, BI_PK, BI_CAT, BI_GH, BI_GR, BI_B1, BI_C0, BI_C1 = range(10)
BI_WPEG = 10   # cols 10:12 = [W_pe, W_pg] f32
NBI = 12

# lrelu unit assignment: 'A' = ACT Prelu, 'B' = DVE relu99 + linear-fold
N_A_UNITS = 32

TRACE = False
TRACE_KW = {}
LAST = None


_COMPUTE_INSTS = (
    "InstActivation", "InstTensorCopy", "InstTensorScalar", "InstTensorScalarPtr",
    "InstTensorTensor", "InstTensorTensorReduce", "InstTensorReduce", "InstMemSet",
    "InstMatmult", "InstScalarTensorTensor", "InstTensorTensorScan", "InstLdweights",
    "InstDMACopy", "InstDMATransposeAnt", "InstTriggeredCopy", "InstDrain",
    "InstEventSemaphoreOp", "InstSemaphoreOp", "InstCopy", "InstIota", "InstSelect",
)


def _legalize_waits(nc):
    # walrus in this toolchain accepts at most ONE sync wait on TPB compute
    # instructions; hoist extras into same-engine NoOps placed just before.
    k = 0
    for f in nc.m.functions:
        for blk in f.blocks:
            insts = blk.instructions
            out = []
            for ins in insts:
                si = getattr(ins, "sync_info", None)
                if (si is not None and len(si.on_wait) > 1
                        and type(ins).__name__ in _COMPUTE_INSTS):
                    waits = list(si.on_wait)
                    for w in waits[:-1]:
                        nop = mybir.InstNoOp(
                            name=f"WNOP-{k}", engine=ins.engine,
                            sync_info=mybir.SyncInfo(on_wait=[w], on_update=[]))
                        k += 1
                        out.append(nop)
                    ins.sync_info = mybir.SyncInfo(on_wait=[waits[-1]],
                                                   on_update=list(si.on_update))
                out.append(ins)
            blk.instructions = out
    return k


def _register_const(nc, val, dtype=F32):
    if (dtype, float(val)) in nc.const_aps.aps:
        return
    t = nc.alloc_sbuf_tensor(f"uconst-{dtype.name}-{val}", [128, 1], dtype)
    nc.gpsimd.memset(t.ap(), float(val))
    nc.const_aps.aps[(dtype, float(val))] = t.ap()


def _unit_engines():
    # interleave N_A_UNITS 'A' units among 64 as evenly as possible
    eng = []
    for u in range(64):
        if (u + 1) * N_A_UNITS // 64 > u * N_A_UNITS // 64:
            eng.append('A')
        else:
            eng.append('B')
    return eng


def build_program(bpe: float, bpg: float, bb2: float, bint_zero: bool = True,
                  sim_trace: bool = False) -> bass.Bass:
    nc = bass.Bass()
    _register_const(nc, 0.5 * bpg)
    _register_const(nc, bb2)
    nc.all_engine_barrier()

    # ---- DRAM inputs (per-core views; same names across SPMD cores) ----
    d_WB = nc.dram_tensor("WB", [128, NWB], BF16, kind="ExternalInput")
    d_BI = nc.dram_tensor("BI", [128, NBI], F32, kind="ExternalInput")
    d_tfX = nc.dram_tensor("tfX", [2, 128, 128], F32, kind="ExternalInput")
    d_laT = nc.dram_tensor("laT", [64, NAH], BF16, kind="ExternalInput")
    d_W64 = nc.dram_tensor("W64", [64, 256], BF16, kind="ExternalInput")
    d_m0T = nc.dram_tensor("m0T", [32, 128, 32], BF16, kind="ExternalInput")
    d_M1 = nc.dram_tensor("M1m", [32, 128, 27], BF16, kind="ExternalInput")
    d_m1T = nc.dram_tensor("m1T", [4, 128, 64], BF16, kind="ExternalInput")
    d_M0 = nc.dram_tensor("M0m", [4, 128, 27], BF16, kind="ExternalInput")
    d_W0T = nc.dram_tensor("W0T", [64, 27 * 128], BF16, kind="ExternalInput")
    d_W32 = nc.dram_tensor("W32", [32, 27 * 128], BF16, kind="ExternalInput")
    d_lgT = nc.dram_tensor("lgT", [64, NG], BF16, kind="ExternalInput")
    d_Sh = nc.dram_tensor("Sh", [4, 128, NG], BF16, kind="ExternalInput")

    d_res = nc.dram_tensor("res_out", [1, 128], F32, kind="ExternalOutput")

    ENG = _unit_engines()
    if not bint_zero:
        ENG[:] = ['A'] * 64

    tc_ref = tile.TileContext(nc, trace_sim=sim_trace)
    with tc_ref as tc:
        with (
            tc.tile_pool(name="const", bufs=1) as cpool,
            tc.tile_pool(name="pre", bufs=1) as prepool,
            tc.tile_pool(name="x", bufs=8) as xpool,
            tc.tile_pool(name="u", bufs=4) as upool,
            tc.tile_pool(name="h", bufs=6) as hpool,
            tc.tile_pool(name="g", bufs=2) as gpool,
            tc.tile_pool(name="j", bufs=2) as jpool,
            tc.tile_pool(name="ps_y", bufs=3, space="PSUM") as psy,
            tc.tile_pool(name="ps_z", bufs=1, space="PSUM") as psz,
            tc.tile_pool(name="ps_p", bufs=1, space="PSUM") as pspre,
        ):
            # ---------- engine warmups (hide ACT table load + start PE pstate clock)
            warm = cpool.tile([128, 1], F32, tag="warm")
            nc.gpsimd.memset(warm[:], 0.0)
            warma = cpool.tile([128, 1], F32, tag="warma")
            nc.scalar.activation(warma[:], warm[:], AF.Silu)
            ps_warm = pspre.tile([1, 1], F32, tag="pre")
            nc.tensor.matmul(ps_warm[:], warm[:], warm[:], start=True, stop=True)
            warmb = cpool.tile([1, 1], F32, tag="warmb")
            nc.scalar.activation(warmb[:], ps_warm[:], AF.Copy)

            # ---------- input DMAs (order = DMA device service priority) ----
            tfx = prepool.tile([128, 256], F32, tag="tfx")
            nc.sync.dma_start(tfx[:, :].rearrange("p (c k) -> p c k", c=2),
                              d_tfX[:, :, :].rearrange("c p k -> p c k"))
            la = prepool.tile([64, NAH], BF16, tag="la")
            nc.sync.dma_start(la[:], d_laT[:])
            W64sb = cpool.tile([64, 256], BF16, tag="W64")
            nc.sync.dma_start(W64sb[:], d_W64[:])
            BIsb = cpool.tile([128, NBI], F32, tag="BI")
            nc.sync.dma_start(BIsb[:], d_BI[:])
            WBsb = cpool.tile([128, NWB], BF16, tag="WB")
            nc.sync.dma_start(WBsb[:], d_WB[:])
            m0sb = cpool.tile([128, 1024], BF16, tag="m0")
            nc.sync.dma_start(m0sb[:, :].rearrange("p (u c) -> p u c", u=32),
                              d_m0T[:, :, :].rearrange("u p c -> p u c"))
            M1sb = cpool.tile([128, 864], BF16, tag="M1")
            nc.sync.dma_start(M1sb[:, :].rearrange("p (u o) -> p u o", u=32),
                              d_M1[:, :, :].rearrange("u p o -> p u o"))
            m1sb = cpool.tile([128, 256], BF16, tag="m1")
            nc.sync.dma_start(m1sb[:, :].rearrange("p (u c) -> p u c", u=4),
                              d_m1T[:, :, :].rearrange("u p c -> p u c"))
            M0sb = cpool.tile([128, 108], BF16, tag="M0")
            nc.sync.dma_start(M0sb[:, :].rearrange("p (u o) -> p u o", u=4),
                              d_M0[:, :, :].rearrange("u p o -> p u o"))
            W0Tsb = cpool.tile([64, 27 * 128], BF16, tag="W0T")
            nc.sync.dma_start(W0Tsb[:], d_W0T[:])
            W32sb = cpool.tile([32, 27 * 128], BF16, tag="W32")
            nc.sync.dma_start(W32sb[:], d_W32[:])
            lg = cpool.tile([64, NG], BF16, tag="lg")
            nc.sync.dma_start(lg[:], d_lgT[:])
            Stsb = cpool.tile([128, 4 * NG], BF16, tag="St")
            nc.sync.dma_start(Stsb[:, :].rearrange("p (q g) -> p q g", q=4),
                              d_Sh[:, :, :].rearrange("q p g -> p q g"))
            F32R = mybir.dt.float32r

            bias = lambda i: BIsb[:, i:i + 1]

            # ---------- preamble: tok / atoms (needed before main loop) -----
            tfr = prepool.tile([128, 256], BF16, tag="tfr")
            nc.scalar.activation(tfr[:], tfx[:], AF.Silu)
            ps_tok = pspre.tile([128, 128], F32, tag="pre")
            nc.tensor.matmul(ps_tok[:], WBsb[:, OFF_WTOK:OFF_WTOK + 128],
                             tfr[:, 0:128], start=True, stop=False)
            nc.tensor.matmul(ps_tok[:], WBsb[:, OFF_WTOK + 128:OFF_WTOK + 256],
                             tfr[:, 128:256], start=False, stop=True)
            tokT = cpool.tile([128, NT], F32, tag="tokT")
            nc.scalar.activation(tokT[:], ps_tok[:], AF.Identity, bias=bias(BI_TOK))

            ps_at = pspre.tile([128, NAH], F32, tag="pre")
            nc.tensor.matmul(ps_at[:], W64sb[:, 0:128], la[:], start=True, stop=True)
            atomsT = cpool.tile([128, NAH], BF16, tag="atomsT")
            nc.scalar.activation(atomsT[:], ps_at[:], AF.Identity, bias=bias(BI_ATOM))

            # ---------- deferred preamble tasks (interleaved into loop) ----
            state = {}

            def task_silu1():
                s0 = cpool.tile([128, 1024], BF16, tag="s0")
                nc.scalar.activation(s0[:], m0sb[:], AF.Silu)
                state["s0"] = s0

            def task_S1():
                S1 = pspre.tile([32, 27], F32, tag="pre")
                for u in range(32):
                    nc.tensor.matmul(S1[:], state["s0"][:, 32 * u:32 * u + 32],
                                     M1sb[:, 27 * u:27 * u + 27],
                                     start=(u == 0), stop=(u == 31))
                S1b = prepool.tile([32, 27], BF16, tag="S1b")
                nc.scalar.activation(S1b[:], S1[:], AF.Copy)
                state["S1b"] = S1b

            def task_p1():
                pp = pspre.tile([128, 1], F32, tag="pre")
                for o in range(27):
                    nc.tensor.matmul(pp[:], W32sb[:, 128 * o:128 * o + 128],
                                     state["S1b"][:, o:o + 1],
                                     start=(o == 0), stop=(o == 26))
                sp1 = prepool.tile([128, 1], BF16, tag="sp1")
                nc.scalar.activation(sp1[:], pp[:], AF.Silu, bias=bias(BI_C1))
                state["sp1"] = sp1

            def task_silu0():
                s1 = prepool.tile([128, 256], BF16, tag="s1")
                nc.scalar.activation(s1[:], m1sb[:], AF.Silu)
                state["s1"] = s1

            def task_S0():
                S0 = pspre.tile([64, 27], F32, tag="pre")
                for u in range(4):
                    nc.tensor.matmul(S0[:], state["s1"][:, 64 * u:64 * u + 64],
                                     M0sb[:, 27 * u:27 * u + 27],
                                     start=(u == 0), stop=(u == 3))
                S0b = prepool.tile([64, 27], BF16, tag="S0b")
                nc.scalar.activation(S0b[:], S0[:], AF.Copy)
                state["S0b"] = S0b

            def task_p0():
                pp = pspre.tile([128, 1], F32, tag="pre")
                for o in range(27):
                    nc.tensor.matmul(pp[:], W0Tsb[:, 128 * o:128 * o + 128],
                                     state["S0b"][:, o:o + 1],
                                     start=(o == 0), stop=(o == 26))
                sp0 = prepool.tile([128, 1], BF16, tag="sp0")
                nc.scalar.activation(sp0[:], pp[:], AF.Silu, bias=bias(BI_C0))
                state["sp0"] = sp0

            def task_pocket():
                ps_pk = pspre.tile([128, 1], F32, tag="pre")
                nc.tensor.matmul(ps_pk[:], WBsb[:, OFF_WPK:OFF_WPK + 128],
                                 state["sp0"][:], start=True, stop=False)
                nc.tensor.matmul(ps_pk[:], WBsb[:, OFF_WPK + 128:OFF_WPK + 256],
                                 state["sp1"][:], start=False, stop=True)
                pocket = prepool.tile([128, 1], BF16, tag="pocket")
                nc.scalar.activation(pocket[:], ps_pk[:], AF.Identity, bias=bias(BI_PK))
                state["pocket"] = pocket

            def task_pf():
                tok_sum = prepool.tile([128, 1], F32, tag="toksum")
                junkt = jpool.tile([128, NT], F32, tag="junk")
                nc.vector.tensor_scalar(junkt[:], tokT[:], 1.0, 0.0, op0=ALU.mult,
                                        op1=ALU.add, accum_out=tok_sum[:])
                tok_sum_b = prepool.tile([128, 1], BF16, tag="toksumb")
                nc.scalar.activation(tok_sum_b[:], tok_sum[:], AF.Copy)
                ps_pf = pspre.tile([128, 2], F32, tag="pre")
                chunks = [state["pocket"], tok_sum_b, tok_sum_b]
                for q in range(3):
                    nc.tensor.matmul(ps_pf[:, 0:1],
                                     WBsb[:, OFF_WCAT + 128 * q:OFF_WCAT + 128 * (q + 1)],
                                     chunks[q][:], start=(q == 0), stop=(q == 2))
                for q in range(3):
                    nc.tensor.matmul(ps_pf[:, 1:2],
                                     WBsb[:, OFF_WGATE + 128 * q:OFF_WGATE + 128 * (q + 1)],
                                     chunks[q][:], start=(q == 0), stop=(q == 2))
                # sigmoid(z + bg) = 0.5 + 0.5*tanh(0.5z + 0.5bg)
                gt = prepool.tile([128, 1], F32, tag="gt")
                nc.scalar.activation(gt[:], ps_pf[:, 1:2], AF.Tanh,
                                     bias=bias(BI_GH), scale=0.5)
                pf_sig = prepool.tile([128, 1], F32, tag="pfsig")
                nc.gpsimd.tensor_scalar(pf_sig[:], gt[:], 0.5, 0.5, op0=ALU.mult, op1=ALU.add)
                pf_lin = prepool.tile([128, 1], F32, tag="pflin")
                nc.scalar.activation(pf_lin[:], ps_pf[:, 0:1], AF.Identity, bias=bias(BI_CAT))
                pf = prepool.tile([128, 1], BF16, tag="pf")
                nc.gpsimd.tensor_tensor(pf[:], pf_lin[:], pf_sig[:], op=ALU.mult)
                state["pf"] = pf

            def task_gf():
                ps_gf = pspre.tile([128, NG], F32, tag="pre")
                nc.tensor.matmul(ps_gf[:], W64sb[:, 128:256], lg[:], start=True, stop=True)
                gfT = prepool.tile([128, NG], BF16, tag="gfT")
                nc.scalar.activation(gfT[:], ps_gf[:], AF.Identity, bias=bias(BI_GR))
                state["gfT"] = gfT

            def task_bias1():
                ps_u = pspre.tile([128, 1], F32, tag="pre")
                nc.tensor.matmul(ps_u[:], WBsb[:, OFF_WB1:OFF_WB1 + 128],
                                 state["pf"][:], start=True, stop=True)
                ub = prepool.tile([128, 1], F32, tag="ub")
                nc.scalar.activation(ub[:], ps_u[:], AF.Identity, bias=bias(BI_B1))
                ps_hb = pspre.tile([128, NG], F32, tag="pre")
                nc.tensor.matmul(ps_hb[:], WBsb[:, OFF_WB1 + 128:OFF_WB1 + 256],
                                 state["gfT"][:], start=True, stop=True)
                hb = prepool.tile([128, NG], BF16, tag="hb")
                nc.scalar.activation(hb[:], ps_hb[:], AF.Prelu, bias=ub[:], alpha=0.01)
                state["hb"] = hb

            def task_bias2():
                ps_b2 = pspre.tile([1, NG], F32, tag="pre")
                nc.tensor.matmul(ps_b2[:], WBsb[:, OFF_WB2:OFF_WB2 + 1],
                                 state["hb"][:], start=True, stop=True)
                nc.scalar.activation(res[:, NG:2 * NG], ps_b2[:], AF.Identity, bias=bb2)

            pre_tasks = [task_silu1, task_S1, task_p1, task_silu0, task_S0,
                         task_p0, task_pocket, task_pf, task_gf, task_bias1,
                         task_bias2]
            TASK_AT = {12 + 4 * i: t for i, t in enumerate(pre_tasks)}

            res = cpool.tile([1, 128], F32, tag="res")

            # ---------- main loop ----------
            # 64 units u of 2 tokens; y2[o, 512v + a] for token j = 2u+v.
            # zq8 (per 64-token block) col layout: 8*(j%64) + 2*a_chunk + {pe,pg}
            wpegr = cpool.tile([128, 2], F32R, tag="wpegr")
            nc.scalar.activation(wpegr[:], BIsb[:, BI_WPEG:BI_WPEG + 2], AF.Copy)
            wpeg_ap = wpegr[:]
            upeg_ap = WBsb[:, OFF_UPEG:OFF_UPEG + 2]
            wint_ap = WBsb[:, OFF_WINT:OFF_WINT + 128]
            zq_tiles = [None, None]
            ae_parts = cpool.tile([128, 16], F32, tag="aeparts")
            pending = []

            def emit_unit(u):
                y2 = psy.tile([128, 1024], F32, tag="y")
                ujs = []
                for v in range(2):
                    j = 2 * u + v
                    Wj = xpool.tile([128, 128], BF16, tag="x")
                    nc.gpsimd.tensor_scalar_mul(Wj[:], wint_ap, tokT[:, j:j + 1])
                    nc.tensor.matmul(y2[:, 512 * v:512 * (v + 1)], Wj[:], atomsT[:],
                                     start=True, stop=True)
                    if ENG[u] == 'B':
                        uj = upool.tile([128, 2], BF16, tag="u")
                        nc.gpsimd.tensor_scalar_mul(uj[:], upeg_ap, tokT[:, j:j + 1])
                        ujs.append(uj)
                return (u, y2, ujs)

            def flush_unit(ent):
                u, y2, ujs = ent
                h = hpool.tile([128, 1024], F32R, tag="h")
                if ENG[u] == 'A':
                    nc.scalar.activation(h[:], y2[:], AF.Prelu, bias=bias(BI_INT),
                                         alpha=0.01)
                else:
                    # h = 0.99*relu(y); the 0.01*y linear part of lrelu is
                    # folded into the zq accumulation via upeg below
                    nc.vector.tensor_scalar(h[:], y2[:], 0.0, 0.99,
                                            op0=ALU.max, op1=ALU.mult)
                for v in range(2):
                    j = 2 * u + v
                    b, jj = j // 64, j % 64
                    if zq_tiles[b] is None:
                        zq_tiles[b] = psz.tile([128, 512], F32, tag="z", name=f"zq{b}")
                    zq = zq_tiles[b]
                    for a in range(4):
                        cols = zq[:, 8 * jj + 2 * a:8 * jj + 2 * a + 2]
                        if ENG[u] == 'A':
                            nc.tensor.matmul(cols, h[:, 512 * v + 128 * a:512 * v + 128 * (a + 1)],
                                             wpeg_ap, start=True, stop=True)
                        else:
                            nc.tensor.matmul(cols, h[:, 512 * v + 128 * a:512 * v + 128 * (a + 1)],
                                             wpeg_ap, start=True, stop=False)
                            nc.tensor.matmul(cols, atomsT[:, 128 * a:128 * (a + 1)],
                                             ujs[v][:], start=False, stop=True)

            def gates(b, c0, c1, slot):
                # process zq cols [c0:c1] -> ae_parts cols 4*slot : 4*slot+4
                zq = zq_tiles[b]
                n2 = (c1 - c0) // 2
                s = gpool.tile([128, 256], F32, tag="s")
                nc.scalar.activation(s[:, 0:n2], zq[:, c0 + 1:c1:2], AF.Tanh,
                                     bias=0.5 * bpg, scale=0.5)
                w = gpool.tile([128, 256], F32, tag="w")
                nc.gpsimd.tensor_scalar(w[:, 0:n2], s[:, 0:n2], 0.5, 0.5,
                                        op0=ALU.mult, op1=ALU.add)
                t = gpool.tile([128, 256], F32, tag="t")
                nc.vector.scalar_tensor_tensor(t[:, 0:n2], zq[:, c0:c1:2], bpe, w[:, 0:n2],
                                               op0=ALU.add, op1=ALU.mult)
                for a in range(4):
                    junka = jpool.tile([128, 64], F32, tag="junka")
                    nc.vector.tensor_scalar(junka[:, 0:n2 // 4], t[:, a:n2:4], 1.0, 0.0,
                                            op0=ALU.mult, op1=ALU.add,
                                            accum_out=ae_parts[:, 4 * slot + a:
                                                              4 * slot + a + 1])

            for u in range(64):
                pending.append(emit_unit(u))
                if len(pending) > 1:
                    flush_unit(pending.pop(0))
                fu = u - 1  # unit just flushed
                if fu == 15:
                    gates(0, 0)
                elif fu == 31:
                    gates(0, 1)
                elif fu == 47:
                    gates(1, 0)
                if fu in TASK_AT:
                    _old_pri = tc.cur_priority
                    tc.cur_priority = _old_pri + 100000
                    TASK_AT[fu]()
                    tc.cur_priority = _old_pri

            flush_unit(pending.pop(0))
            gates(1, 1)

            # atom_e reduce -> seg matmul -> out
            ae8 = prepool.tile([128, 8], F32, tag="ae8")
            nc.gpsimd.tensor_tensor(ae8[:], ae_parts[:, 0:8], ae_parts[:, 8:16], op=ALU.add)
            ae4b = prepool.tile([128, 4], BF16, tag="ae4b")
            nc.gpsimd.tensor_tensor(ae4b[:], ae8[:, 0:4], ae8[:, 4:8], op=ALU.add)
            ps_seg = pspre.tile([1, NG], F32, tag="pre")
            for q in range(4):
                nc.tensor.matmul(ps_seg[:], ae4b[:, q:q + 1], Stsb[:, q * NG:(q + 1) * NG],
                                 start=(q == 0), stop=(q == 3))
            nc.scalar.activation(res[:, 0:NG], ps_seg[:], AF.Copy)
            nc.sync.dma_start(d_res[:], res[:])

    _legalize_waits(nc)
    nc._tile_ctx = tc_ref
    return nc


def kernel(**inputs) -> np.ndarray:
    f = lambda a: np.ascontiguousarray(np.asarray(a), dtype=np.float32)
    bf = lambda a: np.ascontiguousarray(np.asarray(a, dtype=np.float32)).astype(ml_dtypes.bfloat16)
    tf = f(inputs["token_features"])
    la = f(inputs["lig_atom"])
    lgr = f(inputs["lig_graph"])
    m0 = f(inputs["ms_feat_0"])
    m1 = f(inputs["ms_feat_1"])
    lb = np.asarray(inputs["ligand_batch"])
    S = (lb[:, None] == np.arange(NG)[None, :]).astype(np.float32)

    # ---- weight prep (host-side layout/scale transforms only) ----
    wint_bf = bf(inputs["W_int"])                       # [128,128]
    wpe = f(inputs["W_pe"]); wpg = f(inputs["W_pg"])    # [128,1]
    wpeg = np.concatenate([wpe, wpg], axis=1)           # [128,2]
    u_pe = wint_bf.astype(np.float64) @ wpe.astype(np.float64)
    u_pg = wint_bf.astype(np.float64) @ wpg.astype(np.float64)
    upeg = 0.01 * np.concatenate([u_pe, u_pg], axis=1)  # [128,2]

    wcat = f(inputs["W_cat"]).copy()                    # [384,128]
    wgate = f(inputs["W_gate"]).copy()
    wcat[2 * HID:] /= float(NT)
    wgate[2 * HID:] /= float(NT)

    WB = np.zeros((128, NWB), dtype=np.float32)
    WB[:, OFF_WINT:OFF_WINT + 128] = wint_bf.astype(np.float32)
    WB[:, OFF_WTOK:OFF_WTOK + 256] = f(inputs["W_token"]).reshape(2, 128, HID).transpose(1, 0, 2).reshape(128, 256)
    WB[:, OFF_WPK:OFF_WPK + 256] = f(inputs["W_pocket"]).reshape(2, 128, HID).transpose(1, 0, 2).reshape(128, 256)
    WB[:, OFF_WCAT:OFF_WCAT + 384] = wcat.reshape(3, 128, HID).transpose(1, 0, 2).reshape(128, 384)
    WB[:, OFF_WGATE:OFF_WGATE + 384] = wgate.reshape(3, 128, HID).transpose(1, 0, 2).reshape(128, 384)
    WB[:, OFF_WB1:OFF_WB1 + 256] = f(inputs["W_bias1"]).reshape(2, 128, HID).transpose(1, 0, 2).reshape(128, 256)
    WB[:, OFF_WB2:OFF_WB2 + 1] = f(inputs["W_bias2"])
    WB[:, OFF_WPEG:OFF_WPEG + 2] = wpeg
    WB[:, OFF_UPEG:OFF_UPEG + 2] = upeg
    WB_bf = WB.astype(ml_dtypes.bfloat16)

    # conv weights as [c, off*128 + o], scaled by 1/num_output_positions
    Wc0 = f(inputs["Wc0"])  # [128,64,3,3,3] applied to ms_feat_1
    Wc1 = f(inputs["Wc1"])  # [128,32,3,3,3] applied to ms_feat_0
    W0T = np.ascontiguousarray(Wc0.reshape(128, 64, 27).transpose(1, 2, 0)).reshape(64, 27 * 128) / 216.0
    W32 = np.ascontiguousarray(Wc1.reshape(128, 32, 27).transpose(1, 2, 0)).reshape(32, 27 * 128) / 2744.0

    W64 = np.zeros((64, 256), dtype=np.float32)
    W64[:, 0:128] = f(inputs["W_atom"])
    W64[:, 128:256] = f(inputs["W_graph"])

    col = lambda a: f(a).reshape(128, 1)
    BI = np.zeros((128, NBI), dtype=np.float32)
    BI[:, BI_TOK] = f(inputs["b_token"])
    BI[:, BI_ATOM] = f(inputs["b_atom"])
    BI[:, BI_INT] = f(inputs["b_int"])
    BI[:, BI_PK] = f(inputs["b_pocket"])
    BI[:, BI_CAT] = f(inputs["b_cat"])
    BI[:, BI_GH] = 0.5 * f(inputs["b_gate"])
    BI[:, BI_GR] = f(inputs["b_graph"])
    BI[:, BI_B1] = f(inputs["b_bias1"])
    BI[:, BI_C0] = f(inputs["bc0"])
    BI[:, BI_C1] = f(inputs["bc1"])
    BI[:, BI_WPEG:BI_WPEG + 2] = wpeg

    # window-membership masks: M[pos, off] = 1 iff pos-off in valid out range
    def win_mask(D, O):
        g = np.arange(D)
        z, y, x = np.meshgrid(g, g, g, indexing="ij")
        pos = np.stack([z.ravel(), y.ravel(), x.ravel()], 1)  # [D^3, 3]
        d = np.arange(3)
        dz, dy, dx = np.meshgrid(d, d, d, indexing="ij")
        off = np.stack([dz.ravel(), dy.ravel(), dx.ravel()], 1)  # [27, 3]
        r = pos[:, None, :] - off[None, :, :]
        return np.all((r >= 0) & (r < O), axis=2).astype(np.float32)  # [D^3, 27]

    M1 = win_mask(16, 14).reshape(32, 128, 27)
    M0 = win_mask(8, 6).reshape(4, 128, 27)

    bpe = float(np.asarray(inputs["b_pe"]).reshape(-1)[0])
    bpg = float(np.asarray(inputs["b_pg"]).reshape(-1)[0])
    bb2 = float(np.asarray(inputs["b_bias2"]).reshape(-1)[0])

    shared = {
        "WB": WB_bf, "BI": BI,
        "W64": W64.astype(ml_dtypes.bfloat16),
        "W0T": W0T.astype(ml_dtypes.bfloat16),
        "W32": W32.astype(ml_dtypes.bfloat16),
        "M1m": M1.astype(ml_dtypes.bfloat16),
        "M0m": M0.astype(ml_dtypes.bfloat16),
    }

    in_maps = []
    for c in range(NCORES):
        n, h = c // 2, c % 2
        m = dict(shared)
        m["tfX"] = np.ascontiguousarray(tf[n].T.reshape(2, 128, 128))
        m["laT"] = bf(la[n, 512 * h:512 * (h + 1)].T)
        m["lgT"] = bf(lgr[n].T)
        m["m0T"] = bf(m0[n].reshape(32, 4096).T.reshape(32, 128, 32))
        m["m1T"] = bf(m1[n].reshape(64, 512).T.reshape(4, 128, 64))
        m["Sh"] = bf(S[512 * h:512 * (h + 1)].reshape(4, 128, NG))
        in_maps.append(m)

    bint_zero = bool(np.all(np.asarray(inputs["b_int"]) == 0.0))
    nc = build_program(bpe, bpg, bb2, bint_zero)
    r = run_bass_kernel_spmd(nc, in_maps, core_ids=list(range(NCORES)),
                             trace=TRACE, **(TRACE_KW if TRACE else {}))
    global LAST
    LAST = r
    res = r.results

    out = np.zeros((NI, NG), dtype=np.float32)
    for n in range(NI):
        out[n] = (res[2 * n]["res_out"][0, 0:NG] + res[2 * n + 1]["res_out"][0, 0:NG]
                  + res[2 * n]["res_out"][0, NG:2 * NG])
    return out


# revision 21
# speedup vs baseline: 2.1009x; 1.0007x over previous
import sys
import numpy as np
import ml_dtypes

sys.path.insert(0, "/opt/trn_rl_repo")

import concourse.bass as bass
import concourse.tile as tile
from concourse import mybir
from concourse.bass_utils import run_bass_kernel_spmd

F32 = mybir.dt.float32
BF16 = mybir.dt.bfloat16
AF = mybir.ActivationFunctionType
ALU = mybir.AluOpType

HID = 128
NT = 128       # tokens per image
NAH = 512      # atoms per core (half of 1024)
NG = 64        # ligand graphs
NI = 4         # images
NCORES = 8

# WB (128-partition weight concat, bf16) column offsets
OFF_WINT = 0
OFF_WTOK = 128
OFF_WPK = 384
OFF_WCAT = 640
OFF_WGATE = 1024
OFF_WB1 = 1408
OFF_WB2 = 1664
OFF_WPEG = 1665
OFF_UPEG = 1667
NWB = 1669

# BI (f32 bias concat) columns
BI_TOK, BI_ATOM, BI_INT, BI_PK, BI_CAT, BI_GH, BI_GR, BI_B1, BI_C0, BI_C1 = range(10)
BI_WPEG = 10   # cols 10:12 = [W_pe, W_pg] f32
NBI = 12

# lrelu unit assignment: 'A' = ACT Prelu, 'B' = DVE relu99 + linear-fold
N_A_UNITS = 32

TRACE = False
TRACE_KW = {}
LAST = None


_COMPUTE_INSTS = (
    "InstActivation", "InstTensorCopy", "InstTensorScalar", "InstTensorScalarPtr",
    "InstTensorTensor", "InstTensorTensorReduce", "InstTensorReduce", "InstMemSet",
    "InstMatmult", "InstScalarTensorTensor", "InstTensorTensorScan", "InstLdweights",
    "InstDMACopy", "InstDMATransposeAnt", "InstTriggeredCopy", "InstDrain",
    "InstEventSemaphoreOp", "InstSemaphoreOp", "InstCopy", "InstIota", "InstSelect",
)


def _legalize_waits(nc):
    # walrus in this toolchain accepts at most ONE sync wait on TPB compute
    # instructions; hoist extras into same-engine NoOps placed just before.
    k = 0
    for f in nc.m.functions:
        for blk in f.blocks:
            insts = blk.instructions
            out = []
            for ins in insts:
                si = getattr(ins, "sync_info", None)
                if (si is not None and len(si.on_wait) > 1
                        and type(ins).__name__ in _COMPUTE_INSTS):
                    waits = list(si.on_wait)
                    for w in waits[:-1]:
                        nop = mybir.InstNoOp(
                            name=f"WNOP-{k}", engine=ins.engine,
                            sync_info=mybir.SyncInfo(on_wait=[w], on_update=[]))
                        k += 1
                        out.append(nop)
                    ins.sync_info = mybir.SyncInfo(on_wait=[waits[-1]],
                                                   on_update=list(si.on_update))
                out.append(ins)
            blk.instructions = out
    return k


def _register_const(nc, val, dtype=F32):
    if (dtype, float(val)) in nc.const_aps.aps:
        return
    t = nc.alloc_sbuf_tensor(f"uconst-{dtype.name}-{val}", [128, 1], dtype)
    nc.gpsimd.memset(t.ap(), float(val))
    nc.const_aps.aps[(dtype, float(val))] = t.ap()


def _unit_engines():
    # interleave N_A_UNITS 'A' units among 64 as evenly as possible
    eng = []
    for u in range(64):
        if (u + 1) * N_A_UNITS // 64 > u * N_A_UNITS // 64:
            eng.append('A')
        else:
            eng.append('B')
    return eng


def build_program(bpe: float, bpg: float, bb2: float, bint_zero: bool = True,
                  sim_trace: bool = False) -> bass.Bass:
    nc = bass.Bass()
    _register_const(nc, 0.5 * bpg)
    _register_const(nc, bb2)
    nc.all_engine_barrier()

    # ---- DRAM inputs (per-core views; same names across SPMD cores) ----
    d_WB = nc.dram_tensor("WB", [128, NWB], BF16, kind="ExternalInput")
    d_BI = nc.dram_tensor("BI", [128, NBI], F32, kind="ExternalInput")
    d_tfX = nc.dram_tensor("tfX", [2, 128, 128], F32, kind="ExternalInput")
    d_laT = nc.dram_tensor("laT", [64, NAH], BF16, kind="ExternalInput")
    d_W64 = nc.dram_tensor("W64", [64, 256], BF16, kind="ExternalInput")
    d_m0T = nc.dram_tensor("m0T", [32, 128, 32], BF16, kind="ExternalInput")
    d_M1 = nc.dram_tensor("M1m", [32, 128, 27], BF16, kind="ExternalInput")
    d_m1T = nc.dram_tensor("m1T", [4, 128, 64], BF16, kind="ExternalInput")
    d_M0 = nc.dram_tensor("M0m", [4, 128, 27], BF16, kind="ExternalInput")
    d_W0T = nc.dram_tensor("W0T", [64, 27 * 128], BF16, kind="ExternalInput")
    d_W32 = nc.dram_tensor("W32", [32, 27 * 128], BF16, kind="ExternalInput")
    d_lgT = nc.dram_tensor("lgT", [64, NG], BF16, kind="ExternalInput")
    d_Sh = nc.dram_tensor("Sh", [4, 128, NG], BF16, kind="ExternalInput")

    d_res = nc.dram_tensor("res_out", [1, 128], F32, kind="ExternalOutput")

    ENG = _unit_engines()
    if not bint_zero:
        ENG[:] = ['A'] * 64

    tc_ref = tile.TileContext(nc, trace_sim=sim_trace)
    with tc_ref as tc:
        with (
            tc.tile_pool(name="const", bufs=1) as cpool,
            tc.tile_pool(name="pre", bufs=1) as prepool,
            tc.tile_pool(name="x", bufs=8) as xpool,
            tc.tile_pool(name="u", bufs=4) as upool,
            tc.tile_pool(name="h", bufs=6) as hpool,
            tc.tile_pool(name="g", bufs=2) as gpool,
            tc.tile_pool(name="j", bufs=2) as jpool,
            tc.tile_pool(name="ps_y", bufs=3, space="PSUM") as psy,
            tc.tile_pool(name="ps_z", bufs=1, space="PSUM") as psz,
            tc.tile_pool(name="ps_p", bufs=1, space="PSUM") as pspre,
        ):
            # ---------- engine warmups (hide ACT table load + start PE pstate clock)
            warm = cpool.tile([128, 1], F32, tag="warm")
            nc.gpsimd.memset(warm[:], 0.0)
            warma = cpool.tile([128, 1], F32, tag="warma")
            nc.scalar.activation(warma[:], warm[:], AF.Silu)
            ps_warm = pspre.tile([1, 1], F32, tag="pre")
            nc.tensor.matmul(ps_warm[:], warm[:], warm[:], start=True, stop=True)
            warmb = cpool.tile([1, 1], F32, tag="warmb")
            nc.scalar.activation(warmb[:], ps_warm[:], AF.Copy)

            # ---------- input DMAs (order = DMA device service priority) ----
            tfx = prepool.tile([128, 256], F32, tag="tfx")
            nc.sync.dma_start(tfx[:, :].rearrange("p (c k) -> p c k", c=2),
                              d_tfX[:, :, :].rearrange("c p k -> p c k"))
            la = prepool.tile([64, NAH], BF16, tag="la")
            nc.sync.dma_start(la[:], d_laT[:])
            W64sb = cpool.tile([64, 256], BF16, tag="W64")
            nc.sync.dma_start(W64sb[:], d_W64[:])
            BIsb = cpool.tile([128, NBI], F32, tag="BI")
            nc.sync.dma_start(BIsb[:], d_BI[:])
            WBsb = cpool.tile([128, NWB], BF16, tag="WB")
            nc.sync.dma_start(WBsb[:], d_WB[:])
            m0sb = cpool.tile([128, 1024], BF16, tag="m0")
            nc.sync.dma_start(m0sb[:, :].rearrange("p (u c) -> p u c", u=32),
                              d_m0T[:, :, :].rearrange("u p c -> p u c"))
            M1sb = cpool.tile([128, 864], BF16, tag="M1")
            nc.sync.dma_start(M1sb[:, :].rearrange("p (u o) -> p u o", u=32),
                              d_M1[:, :, :].rearrange("u p o -> p u o"))
            m1sb = cpool.tile([128, 256], BF16, tag="m1")
            nc.sync.dma_start(m1sb[:, :].rearrange("p (u c) -> p u c", u=4),
                              d_m1T[:, :, :].rearrange("u p c -> p u c"))
            M0sb = cpool.tile([128, 108], BF16, tag="M0")
            nc.sync.dma_start(M0sb[:, :].rearrange("p (u o) -> p u o", u=4),
                              d_M0[:, :, :].rearrange("u p o -> p u o"))
            W0Tsb = cpool.tile([64, 27 * 128], BF16, tag="W0T")
            nc.sync.dma_start(W0Tsb[:], d_W0T[:])
            W32sb = cpool.tile([32, 27 * 128], BF16, tag="W32")
            nc.sync.dma_start(W32sb[:], d_W32[:])
            lg = cpool.tile([64, NG], BF16, tag="lg")
            nc.sync.dma_start(lg[:], d_lgT[:])
            Stsb = cpool.tile([128, 4 * NG], BF16, tag="St")
            nc.sync.dma_start(Stsb[:, :].rearrange("p (q g) -> p q g", q=4),
                              d_Sh[:, :, :].rearrange("q p g -> p q g"))
            F32R = mybir.dt.float32r

            bias = lambda i: BIsb[:, i:i + 1]

            # ---------- preamble: tok / atoms (needed before main loop) -----
            tfr = prepool.tile([128, 256], BF16, tag="tfr")
            nc.scalar.activation(tfr[:], tfx[:], AF.Silu)
            ps_tok = pspre.tile([128, 128], F32, tag="pre")
            nc.tensor.matmul(ps_tok[:], WBsb[:, OFF_WTOK:OFF_WTOK + 128],
                             tfr[:, 0:128], start=True, stop=False)
            nc.tensor.matmul(ps_tok[:], WBsb[:, OFF_WTOK + 128:OFF_WTOK + 256],
                             tfr[:, 128:256], start=False, stop=True)
            tokT = cpool.tile([128, NT], F32, tag="tokT")
            nc.scalar.activation(tokT[:], ps_tok[:], AF.Identity, bias=bias(BI_TOK))

            ps_at = pspre.tile([128, NAH], F32, tag="pre")
            nc.tensor.matmul(ps_at[:], W64sb[:, 0:128], la[:], start=True, stop=True)
            atomsT = cpool.tile([128, NAH], BF16, tag="atomsT")
            nc.scalar.activation(atomsT[:], ps_at[:], AF.Identity, bias=bias(BI_ATOM))

            # ---------- deferred preamble tasks (interleaved into loop) ----
            state = {}

            def task_silu1():
                s0 = cpool.tile([128, 1024], BF16, tag="s0")
                nc.scalar.activation(s0[:], m0sb[:], AF.Silu)
                state["s0"] = s0

            def task_S1():
                S1 = pspre.tile([32, 27], F32, tag="pre")
                for u in range(32):
                    nc.tensor.matmul(S1[:], state["s0"][:, 32 * u:32 * u + 32],
                                     M1sb[:, 27 * u:27 * u + 27],
                                     start=(u == 0), stop=(u == 31))
                S1b = prepool.tile([32, 27], BF16, tag="S1b")
                nc.scalar.activation(S1b[:], S1[:], AF.Copy)
                state["S1b"] = S1b

            def task_p1():
                pp = pspre.tile([128, 1], F32, tag="pre")
                for o in range(27):
                    nc.tensor.matmul(pp[:], W32sb[:, 128 * o:128 * o + 128],
                                     state["S1b"][:, o:o + 1],
                                     start=(o == 0), stop=(o == 26))
                sp1 = prepool.tile([128, 1], BF16, tag="sp1")
                nc.scalar.activation(sp1[:], pp[:], AF.Silu, bias=bias(BI_C1))
                state["sp1"] = sp1

            def task_silu0():
                s1 = prepool.tile([128, 256], BF16, tag="s1")
                nc.scalar.activation(s1[:], m1sb[:], AF.Silu)
                state["s1"] = s1

            def task_S0():
                S0 = pspre.tile([64, 27], F32, tag="pre")
                for u in range(4):
                    nc.tensor.matmul(S0[:], state["s1"][:, 64 * u:64 * u + 64],
                                     M0sb[:, 27 * u:27 * u + 27],
                                     start=(u == 0), stop=(u == 3))
                S0b = prepool.tile([64, 27], BF16, tag="S0b")
                nc.scalar.activation(S0b[:], S0[:], AF.Copy)
                state["S0b"] = S0b

            def task_p0():
                pp = pspre.tile([128, 1], F32, tag="pre")
                for o in range(27):
                    nc.tensor.matmul(pp[:], W0Tsb[:, 128 * o:128 * o + 128],
                                     state["S0b"][:, o:o + 1],
                                     start=(o == 0), stop=(o == 26))
                sp0 = prepool.tile([128, 1], BF16, tag="sp0")
                nc.scalar.activation(sp0[:], pp[:], AF.Silu, bias=bias(BI_C0))
                state["sp0"] = sp0

            def task_pocket():
                ps_pk = pspre.tile([128, 1], F32, tag="pre")
                nc.tensor.matmul(ps_pk[:], WBsb[:, OFF_WPK:OFF_WPK + 128],
                                 state["sp0"][:], start=True, stop=False)
                nc.tensor.matmul(ps_pk[:], WBsb[:, OFF_WPK + 128:OFF_WPK + 256],
                                 state["sp1"][:], start=False, stop=True)
                pocket = prepool.tile([128, 1], BF16, tag="pocket")
                nc.scalar.activation(pocket[:], ps_pk[:], AF.Identity, bias=bias(BI_PK))
                state["pocket"] = pocket

            def task_pf():
                tok_sum = prepool.tile([128, 1], F32, tag="toksum")
                junkt = jpool.tile([128, NT], F32, tag="junk")
                nc.vector.tensor_scalar(junkt[:], tokT[:], 1.0, 0.0, op0=ALU.mult,
                                        op1=ALU.add, accum_out=tok_sum[:])
                tok_sum_b = prepool.tile([128, 1], BF16, tag="toksumb")
                nc.scalar.activation(tok_sum_b[:], tok_sum[:], AF.Copy)
                ps_pf = pspre.tile([128, 2], F32, tag="pre")
                chunks = [state["pocket"], tok_sum_b, tok_sum_b]
                for q in range(3):
                    nc.tensor.matmul(ps_pf[:, 0:1],
                                     WBsb[:, OFF_WCAT + 128 * q:OFF_WCAT + 128 * (q + 1)],
                                     chunks[q][:], start=(q == 0), stop=(q == 2))
                for q in range(3):
                    nc.tensor.matmul(ps_pf[:, 1:2],
                                     WBsb[:, OFF_WGATE + 128 * q:OFF_WGATE + 128 * (q + 1)],
                                     chunks[q][:], start=(q == 0), stop=(q == 2))
                # sigmoid(z + bg) = 0.5 + 0.5*tanh(0.5z + 0.5bg)
                gt = prepool.tile([128, 1], F32, tag="gt")
                nc.scalar.activation(gt[:], ps_pf[:, 1:2], AF.Tanh,
                                     bias=bias(BI_GH), scale=0.5)
                pf_sig = prepool.tile([128, 1], F32, tag="pfsig")
                nc.gpsimd.tensor_scalar(pf_sig[:], gt[:], 0.5, 0.5, op0=ALU.mult, op1=ALU.add)
                pf_lin = prepool.tile([128, 1], F32, tag="pflin")
                nc.scalar.activation(pf_lin[:], ps_pf[:, 0:1], AF.Identity, bias=bias(BI_CAT))
                pf = prepool.tile([128, 1], BF16, tag="pf")
                nc.gpsimd.tensor_tensor(pf[:], pf_lin[:], pf_sig[:], op=ALU.mult)
                state["pf"] = pf

            def task_gf():
                ps_gf = pspre.tile([128, NG], F32, tag="pre")
                nc.tensor.matmul(ps_gf[:], W64sb[:, 128:256], lg[:], start=True, stop=True)
                gfT = prepool.tile([128, NG], BF16, tag="gfT")
                nc.scalar.activation(gfT[:], ps_gf[:], AF.Identity, bias=bias(BI_GR))
                state["gfT"] = gfT

            def task_bias1():
                ps_u = pspre.tile([128, 1], F32, tag="pre")
                nc.tensor.matmul(ps_u[:], WBsb[:, OFF_WB1:OFF_WB1 + 128],
                                 state["pf"][:], start=True, stop=True)
                ub = prepool.tile([128, 1], F32, tag="ub")
                nc.scalar.activation(ub[:], ps_u[:], AF.Identity, bias=bias(BI_B1))
                ps_hb = pspre.tile([128, NG], F32, tag="pre")
                nc.tensor.matmul(ps_hb[:], WBsb[:, OFF_WB1 + 128:OFF_WB1 + 256],
                                 state["gfT"][:], start=True, stop=True)
                hb = prepool.tile([128, NG], BF16, tag="hb")
                nc.scalar.activation(hb[:], ps_hb[:], AF.Prelu, bias=ub[:], alpha=0.01)
                state["hb"] = hb

            def task_bias2():
                ps_b2 = pspre.tile([1, NG], F32, tag="pre")
                nc.tensor.matmul(ps_b2[:], WBsb[:, OFF_WB2:OFF_WB2 + 1],
                                 state["hb"][:], start=True, stop=True)
                nc.scalar.activation(res[:, NG:2 * NG], ps_b2[:], AF.Identity, bias=bb2)

            pre_tasks = [task_silu1, task_S1, task_p1, task_silu0, task_S0,
                         task_p0, task_pocket, task_pf, task_gf, task_bias1,
                         task_bias2]
            TASK_AT = {12 + 4 * i: t for i, t in enumerate(pre_tasks)}

            res = cpool.tile([1, 128], F32, tag="res")

            # ---------- main loop ----------
            # 64 units u of 2 tokens; y2[o, 512v + a] for token j = 2u+v.
            # zq8 (per 64-token block) col layout: 8*(j%64) + 2*a_chunk + {pe,pg}
            wpegr = cpool.tile([128, 2], F32R, tag="wpegr")
            nc.scalar.activation(wpegr[:], BIsb[:, BI_WPEG:BI_WPEG + 2], AF.Copy)
            wpeg_ap = wpegr[:]
            upeg_ap = WBsb[:, OFF_UPEG:OFF_UPEG + 2]
            wint_ap = WBsb[:, OFF_WINT:OFF_WINT + 128]
            zq_tiles = [None, None]
            ae_parts = cpool.tile([128, 20], F32, tag="aeparts")
            pending = []

            def emit_unit(u):
                y2 = psy.tile([128, 1024], F32, tag="y")
                ujs = []
                for v in range(2):
                    j = 2 * u + v
                    Wj = xpool.tile([128, 128], BF16, tag="x")
                    nc.gpsimd.tensor_scalar_mul(Wj[:], wint_ap, tokT[:, j:j + 1])
                    nc.tensor.matmul(y2[:, 512 * v:512 * (v + 1)], Wj[:], atomsT[:],
                                     start=True, stop=True)
                    if ENG[u] == 'B':
                        uj = upool.tile([128, 2], BF16, tag="u")
                        nc.gpsimd.tensor_scalar_mul(uj[:], upeg_ap, tokT[:, j:j + 1])
                        ujs.append(uj)
                return (u, y2, ujs)

            def flush_unit(ent):
                u, y2, ujs = ent
                h = hpool.tile([128, 1024], F32R, tag="h")
                if ENG[u] == 'A':
                    nc.scalar.activation(h[:], y2[:], AF.Prelu, bias=bias(BI_INT),
                                         alpha=0.01)
                else:
                    # h = 0.99*relu(y); the 0.01*y linear part of lrelu is
                    # folded into the zq accumulation via upeg below
                    nc.vector.tensor_scalar(h[:], y2[:], 0.0, 0.99,
                                            op0=ALU.max, op1=ALU.mult)
                for v in range(2):
                    j = 2 * u + v
                    b, jj = j // 64, j % 64
                    if zq_tiles[b] is None:
                        zq_tiles[b] = psz.tile([128, 512], F32, tag="z", name=f"zq{b}")
                    zq = zq_tiles[b]
                    for a in range(4):
                        cols = zq[:, 8 * jj + 2 * a:8 * jj + 2 * a + 2]
                        if ENG[u] == 'A':
                            nc.tensor.matmul(cols, h[:, 512 * v + 128 * a:512 * v + 128 * (a + 1)],
                                             wpeg_ap, start=True, stop=True)
                        else:
                            nc.tensor.matmul(cols, h[:, 512 * v + 128 * a:512 * v + 128 * (a + 1)],
                                             wpeg_ap, start=True, stop=False)
                            nc.tensor.matmul(cols, atomsT[:, 128 * a:128 * (a + 1)],
                                             ujs[v][:], start=False, stop=True)

            def gates(b, c0, c1, slot):
                # process zq cols [c0:c1] -> ae_parts cols 4*slot : 4*slot+4
                zq = zq_tiles[b]
                n2 = (c1 - c0) // 2
                s = gpool.tile([128, 256], F32, tag="s")
                nc.scalar.activation(s[:, 0:n2], zq[:, c0 + 1:c1:2], AF.Tanh,
                                     bias=0.5 * bpg, scale=0.5)
                w = gpool.tile([128, 256], F32, tag="w")
                nc.gpsimd.tensor_scalar(w[:, 0:n2], s[:, 0:n2], 0.5, 0.5,
                                        op0=ALU.mult, op1=ALU.add)
                t = gpool.tile([128, 256], F32, tag="t")
                nc.vector.scalar_tensor_tensor(t[:, 0:n2], zq[:, c0:c1:2], bpe, w[:, 0:n2],
                                               op0=ALU.add, op1=ALU.mult)
                for a in range(4):
                    junka = jpool.tile([128, 64], F32, tag="junka")
                    nc.vector.tensor_scalar(junka[:, 0:n2 // 4], t[:, a:n2:4], 1.0, 0.0,
                                            op0=ALU.mult, op1=ALU.add,
                                            accum_out=ae_parts[:, 4 * slot + a:
                                                              4 * slot + a + 1])

            for u in range(64):
                pending.append(emit_unit(u))
                if len(pending) > 1:
                    flush_unit(pending.pop(0))
                fu = u - 1  # unit just flushed
                if fu == 15:
                    gates(0, 0)
                elif fu == 31:
                    gates(0, 1)
                elif fu == 47:
                    gates(1, 0)
                if fu in TASK_AT:
                    _old_pri = tc.cur_priority
                    tc.cur_priority = _old_pri + 100000
                    TASK_AT[fu]()
                    tc.cur_priority = _old_pri

            flush_unit(pending.pop(0))
            gates(1, 1)

            # atom_e reduce -> seg matmul -> out
            ae8 = prepool.tile([128, 8], F32, tag="ae8")
            nc.gpsimd.tensor_tensor(ae8[:], ae_parts[:, 0:8], ae_parts[:, 8:16], op=ALU.add)
            ae4f = prepool.tile([128, 4], F32, tag="ae4f")
            nc.gpsimd.tensor_tensor(ae4f[:], ae8[:, 0:4], ae8[:, 4:8], op=ALU.add)
            ae4b = prepool.tile([128, 4], BF16, tag="ae4b")
            nc.vector.tensor_tensor(ae4b[:], ae4f[:], ae_parts[:, 16:20], op=ALU.add)
            ps_seg = pspre.tile([1, NG], F32, tag="pre")
            for q in range(4):
                nc.tensor.matmul(ps_seg[:], ae4b[:, q:q + 1], Stsb[:, q * NG:(q + 1) * NG],
                                 start=(q == 0), stop=(q == 3))
            nc.scalar.activation(res[:, 0:NG], ps_seg[:], AF.Copy)
            nc.sync.dma_start(d_res[:], res[:])

    _legalize_waits(nc)
    nc._tile_ctx = tc_ref
    return nc


def kernel(**inputs) -> np.ndarray:
    f = lambda a: np.ascontiguousarray(np.asarray(a), dtype=np.float32)
    bf = lambda a: np.ascontiguousarray(np.asarray(a, dtype=np.float32)).astype(ml_dtypes.bfloat16)
    tf = f(inputs["token_features"])
    la = f(inputs["lig_atom"])
    lgr = f(inputs["lig_graph"])
    m0 = f(inputs["ms_feat_0"])
    m1 = f(inputs["ms_feat_1"])
    lb = np.asarray(inputs["ligand_batch"])
    S = (lb[:, None] == np.arange(NG)[None, :]).astype(np.float32)

    # ---- weight prep (host-side layout/scale transforms only) ----
    wint_bf = bf(inputs["W_int"])                       # [128,128]
    wpe = f(inputs["W_pe"]); wpg = f(inputs["W_pg"])    # [128,1]
    wpeg = np.concatenate([wpe, wpg], axis=1)           # [128,2]
    u_pe = wint_bf.astype(np.float64) @ wpe.astype(np.float64)
    u_pg = wint_bf.astype(np.float64) @ wpg.astype(np.float64)
    upeg = 0.01 * np.concatenate([u_pe, u_pg], axis=1)  # [128,2]

    wcat = f(inputs["W_cat"]).copy()                    # [384,128]
    wgate = f(inputs["W_gate"]).copy()
    wcat[2 * HID:] /= float(NT)
    wgate[2 * HID:] /= float(NT)

    WB = np.zeros((128, NWB), dtype=np.float32)
    WB[:, OFF_WINT:OFF_WINT + 128] = wint_bf.astype(np.float32)
    WB[:, OFF_WTOK:OFF_WTOK + 256] = f(inputs["W_token"]).reshape(2, 128, HID).transpose(1, 0, 2).reshape(128, 256)
    WB[:, OFF_WPK:OFF_WPK + 256] = f(inputs["W_pocket"]).reshape(2, 128, HID).transpose(1, 0, 2).reshape(128, 256)
    WB[:, OFF_WCAT:OFF_WCAT + 384] = wcat.reshape(3, 128, HID).transpose(1, 0, 2).reshape(128, 384)
    WB[:, OFF_WGATE:OFF_WGATE + 384] = wgate.reshape(3, 128, HID).transpose(1, 0, 2).reshape(128, 384)
    WB[:, OFF_WB1:OFF_WB1 + 256] = f(inputs["W_bias1"]).reshape(2, 128, HID).transpose(1, 0, 2).reshape(128, 256)
    WB[:, OFF_WB2:OFF_WB2 + 1] = f(inputs["W_bias2"])
    WB[:, OFF_WPEG:OFF_WPEG + 2] = wpeg
    WB[:, OFF_UPEG:OFF_UPEG + 2] = upeg
    WB_bf = WB.astype(ml_dtypes.bfloat16)

    # conv weights as [c, off*128 + o], scaled by 1/num_output_positions
    Wc0 = f(inputs["Wc0"])  # [128,64,3,3,3] applied to ms_feat_1
    Wc1 = f(inputs["Wc1"])  # [128,32,3,3,3] applied to ms_feat_0
    W0T = np.ascontiguousarray(Wc0.reshape(128, 64, 27).transpose(1, 2, 0)).reshape(64, 27 * 128) / 216.0
    W32 = np.ascontiguousarray(Wc1.reshape(128, 32, 27).transpose(1, 2, 0)).reshape(32, 27 * 128) / 2744.0

    W64 = np.zeros((64, 256), dtype=np.float32)
    W64[:, 0:128] = f(inputs["W_atom"])
    W64[:, 128:256] = f(inputs["W_graph"])

    col = lambda a: f(a).reshape(128, 1)
    BI = np.zeros((128, NBI), dtype=np.float32)
    BI[:, BI_TOK] = f(inputs["b_token"])
    BI[:, BI_ATOM] = f(inputs["b_atom"])
    BI[:, BI_INT] = f(inputs["b_int"])
    BI[:, BI_PK] = f(inputs["b_pocket"])
    BI[:, BI_CAT] = f(inputs["b_cat"])
    BI[:, BI_GH] = 0.5 * f(inputs["b_gate"])
    BI[:, BI_GR] = f(inputs["b_graph"])
    BI[:, BI_B1] = f(inputs["b_bias1"])
    BI[:, BI_C0] = f(inputs["bc0"])
    BI[:, BI_C1] = f(inputs["bc1"])
    BI[:, BI_WPEG:BI_WPEG + 2] = wpeg

    # window-membership masks: M[pos, off] = 1 iff pos-off in valid out range
    def win_mask(D, O):
        g = np.arange(D)
        z, y, x = np.meshgrid(g, g, g, indexing="ij")
        pos = np.stack([z.ravel(), y.ravel(), x.ravel()], 1)  # [D^3, 3]
        d = np.arange(3)
        dz, dy, dx = np.meshgrid(d, d, d, indexing="ij")
        off = np.stack([dz.ravel(), dy.ravel(), dx.ravel()], 1)  # [27, 3]
        r = pos[:, None, :] - off[None, :, :]
        return np.all((r >= 0) & (r < O), axis=2).astype(np.float32)  # [D^3, 27]

    M1 = win_mask(16, 14).reshape(32, 128, 27)
    M0 = win_mask(8, 6).reshape(4, 128, 27)

    bpe = float(np.asarray(inputs["b_pe"]).reshape(-1)[0])
    bpg = float(np.asarray(inputs["b_pg"]).reshape(-1)[0])
    bb2 = float(np.asarray(inputs["b_bias2"]).reshape(-1)[0])

    shared = {
        "WB": WB_bf, "BI": BI,
        "W64": W64.astype(ml_dtypes.bfloat16),
        "W0T": W0T.astype(ml_dtypes.bfloat16),
        "W32": W32.astype(ml_dtypes.bfloat16),
        "M1m": M1.astype(ml_dtypes.bfloat16),
        "M0m": M0.astype(ml_dtypes.bfloat16),
    }

    in_maps = []
    for c in range(NCORES):
        n, h = c // 2, c % 2
        m = dict(shared)
        m["tfX"] = np.ascontiguousarray(tf[n].T.reshape(2, 128, 128))
        m["laT"] = bf(la[n, 512 * h:512 * (h + 1)].T)
        m["lgT"] = bf(lgr[n].T)
        m["m0T"] = bf(m0[n].reshape(32, 4096).T.reshape(32, 128, 32))
        m["m1T"] = bf(m1[n].reshape(64, 512).T.reshape(4, 128, 64))
        m["Sh"] = bf(S[512 * h:512 * (h + 1)].reshape(4, 128, NG))
        in_maps.append(m)

    bint_zero = bool(np.all(np.asarray(inputs["b_int"]) == 0.0))
    nc = build_program(bpe, bpg, bb2, bint_zero)
    r = run_bass_kernel_spmd(nc, in_maps, core_ids=list(range(NCORES)),
                             trace=TRACE, **(TRACE_KW if TRACE else {}))
    global LAST
    LAST = r
    res = r.results

    out = np.zeros((NI, NG), dtype=np.float32)
    for n in range(NI):
        out[n] = (res[2 * n]["res_out"][0, 0:NG] + res[2 * n + 1]["res_out"][0, 0:NG]
                  + res[2 * n]["res_out"][0, NG:2 * NG])
    return out


# revision 27
# speedup vs baseline: 2.1350x; 1.0162x over previous
import sys
import numpy as np
import ml_dtypes

sys.path.insert(0, "/opt/trn_rl_repo")

import concourse.bass as bass
import concourse.tile as tile
from concourse import mybir
from concourse.bass_utils import run_bass_kernel_spmd

F32 = mybir.dt.float32
BF16 = mybir.dt.bfloat16
AF = mybir.ActivationFunctionType
ALU = mybir.AluOpType

HID = 128
NT = 128       # tokens per image
NAH = 512      # atoms per core (half of 1024)
NG = 64        # ligand graphs
NI = 4         # images
NCORES = 8

# WB (128-partition weight concat, bf16) column offsets
OFF_WINT = 0
OFF_WTOK = 128
OFF_WPK = 384
OFF_WCAT = 640
OFF_WGATE = 1024
OFF_WB1 = 1408
OFF_WB2 = 1664
OFF_WPEG = 1665
OFF_UPEG = 1667
NWB = 1669

# BI (f32 bias concat) columns
BI_TOK, BI_ATOM, BI_INT, BI_PK, BI_CAT, BI_GH, BI_GR, BI_B1, BI_C0, BI_C1 = range(10)
BI_WPEG = 10   # cols 10:12 = [W_pe, W_pg] f32
NBI = 12

# lrelu unit assignment: 'A' = ACT Prelu, 'B' = DVE relu99 + linear-fold
N_A_UNITS = 32

TRACE = False
TRACE_KW = {}
LAST = None


_COMPUTE_INSTS = (
    "InstActivation", "InstTensorCopy", "InstTensorScalar", "InstTensorScalarPtr",
    "InstTensorTensor", "InstTensorTensorReduce", "InstTensorReduce", "InstMemSet",
    "InstMatmult", "InstScalarTensorTensor", "InstTensorTensorScan", "InstLdweights",
    "InstDMACopy", "InstDMATransposeAnt", "InstTriggeredCopy", "InstDrain",
    "InstEventSemaphoreOp", "InstSemaphoreOp", "InstCopy", "InstIota", "InstSelect",
)


def _legalize_waits(nc):
    # walrus in this toolchain accepts at most ONE sync wait on TPB compute
    # instructions; hoist extras into same-engine NoOps placed just before.
    k = 0
    for f in nc.m.functions:
        for blk in f.blocks:
            insts = blk.instructions
            out = []
            for ins in insts:
                si = getattr(ins, "sync_info", None)
                if (si is not None and len(si.on_wait) > 1
                        and type(ins).__name__ in _COMPUTE_INSTS):
                    waits = list(si.on_wait)
                    for w in waits[:-1]:
                        nop = mybir.InstNoOp(
                            name=f"WNOP-{k}", engine=ins.engine,
                            sync_info=mybir.SyncInfo(on_wait=[w], on_update=[]))
                        k += 1
                        out.append(nop)
                    ins.sync_info = mybir.SyncInfo(on_wait=[waits[-1]],
                                                   on_update=list(si.on_update))
                out.append(ins)
            blk.instructions = out
    return k


def _register_const(nc, val, dtype=F32):
    if (dtype, float(val)) in nc.const_aps.aps:
        return
    t = nc.alloc_sbuf_tensor(f"uconst-{dtype.name}-{val}", [128, 1], dtype)
    nc.gpsimd.memset(t.ap(), float(val))
    nc.const_aps.aps[(dtype, float(val))] = t.ap()


def _unit_engines():
    # interleave N_A_UNITS 'A' units among 64 as evenly as possible
    eng = []
    for u in range(64):
        if (u + 1) * N_A_UNITS // 64 > u * N_A_UNITS // 64:
            eng.append('A')
        else:
            eng.append('B')
    return eng


def build_program(bpe: float, bpg: float, bb2: float, bint_zero: bool = True,
                  sim_trace: bool = False) -> bass.Bass:
    nc = bass.Bass()
    _register_const(nc, 0.5 * bpg)
    _register_const(nc, bb2)
    nc.all_engine_barrier()

    # ---- DRAM inputs (per-core views; same names across SPMD cores) ----
    d_WB = nc.dram_tensor("WB", [128, NWB], BF16, kind="ExternalInput")
    d_WE = nc.dram_tensor("WE", [128, 384], BF16, kind="ExternalInput")
    d_BI = nc.dram_tensor("BI", [128, NBI], F32, kind="ExternalInput")
    d_tfX = nc.dram_tensor("tfX", [2, 128, 128], F32, kind="ExternalInput")
    d_laT = nc.dram_tensor("laT", [64, NAH], BF16, kind="ExternalInput")
    d_W64 = nc.dram_tensor("W64", [64, 256], BF16, kind="ExternalInput")
    d_m0T = nc.dram_tensor("m0T", [32, 128, 32], BF16, kind="ExternalInput")
    d_M1 = nc.dram_tensor("M1m", [32, 128, 27], BF16, kind="ExternalInput")
    d_m1T = nc.dram_tensor("m1T", [4, 128, 64], BF16, kind="ExternalInput")
    d_M0 = nc.dram_tensor("M0m", [4, 128, 27], BF16, kind="ExternalInput")
    d_W0T = nc.dram_tensor("W0T", [64, 27 * 128], BF16, kind="ExternalInput")
    d_W32 = nc.dram_tensor("W32", [32, 27 * 128], BF16, kind="ExternalInput")
    d_lgT = nc.dram_tensor("lgT", [64, NG], BF16, kind="ExternalInput")
    d_Sh = nc.dram_tensor("Sh", [4, 128, NG], BF16, kind="ExternalInput")

    d_res = nc.dram_tensor("res_out", [1, 128], F32, kind="ExternalOutput")

    ENG = _unit_engines()
    if not bint_zero:
        ENG[:] = ['A'] * 64

    tc_ref = tile.TileContext(nc, trace_sim=sim_trace)
    with tc_ref as tc:
        with (
            tc.tile_pool(name="const", bufs=1) as cpool,
            tc.tile_pool(name="pre", bufs=1) as prepool,
            tc.tile_pool(name="x", bufs=12) as xpool,
            tc.tile_pool(name="u", bufs=8) as upool,
            tc.tile_pool(name="h", bufs=8) as hpool,
            tc.tile_pool(name="g", bufs=3) as gpool,
            tc.tile_pool(name="j", bufs=4) as jpool,
            tc.tile_pool(name="ps_y", bufs=3, space="PSUM") as psy,
            tc.tile_pool(name="ps_z", bufs=1, space="PSUM") as psz,
            tc.tile_pool(name="ps_p", bufs=1, space="PSUM") as pspre,
        ):
            # ---------- engine warmups (hide ACT table load + start PE pstate clock)
            warm = cpool.tile([128, 1], F32, tag="warm")
            nc.gpsimd.memset(warm[:], 0.0)
            warma = cpool.tile([128, 1], F32, tag="warma")
            nc.scalar.activation(warma[:], warm[:], AF.Silu)
            ps_warm = pspre.tile([1, 1], F32, tag="pre")
            nc.tensor.matmul(ps_warm[:], warm[:], warm[:], start=True, stop=True)
            warmb = cpool.tile([1, 1], F32, tag="warmb")
            nc.scalar.activation(warmb[:], ps_warm[:], AF.Copy)

            # ---------- input DMAs (order = DMA device service priority) ----
            tfx = prepool.tile([128, 256], F32, tag="tfx")
            nc.sync.dma_start(tfx[:, :].rearrange("p (c k) -> p c k", c=2),
                              d_tfX[:, :, :].rearrange("c p k -> p c k"))
            la = prepool.tile([64, NAH], BF16, tag="la")
            nc.sync.dma_start(la[:], d_laT[:])
            W64sb = cpool.tile([64, 256], BF16, tag="W64")
            nc.sync.dma_start(W64sb[:], d_W64[:])
            WEsb = cpool.tile([128, 384], BF16, tag="WE")
            nc.sync.dma_start(WEsb[:], d_WE[:])
            BIsb = cpool.tile([128, NBI], F32, tag="BI")
            nc.sync.dma_start(BIsb[:], d_BI[:])
            WBsb = cpool.tile([128, NWB], BF16, tag="WB")
            nc.sync.dma_start(WBsb[:], d_WB[:])
            m0sb = cpool.tile([128, 1024], BF16, tag="m0")
            nc.sync.dma_start(m0sb[:, :].rearrange("p (u c) -> p u c", u=32),
                              d_m0T[:, :, :].rearrange("u p c -> p u c"))
            M1sb = cpool.tile([128, 864], BF16, tag="M1")
            nc.sync.dma_start(M1sb[:, :].rearrange("p (u o) -> p u o", u=32),
                              d_M1[:, :, :].rearrange("u p o -> p u o"))
            m1sb = cpool.tile([128, 256], BF16, tag="m1")
            nc.sync.dma_start(m1sb[:, :].rearrange("p (u c) -> p u c", u=4),
                              d_m1T[:, :, :].rearrange("u p c -> p u c"))
            M0sb = cpool.tile([128, 108], BF16, tag="M0")
            nc.sync.dma_start(M0sb[:, :].rearrange("p (u o) -> p u o", u=4),
                              d_M0[:, :, :].rearrange("u p o -> p u o"))
            W0Tsb = cpool.tile([64, 27 * 128], BF16, tag="W0T")
            nc.sync.dma_start(W0Tsb[:], d_W0T[:])
            W32sb = cpool.tile([32, 27 * 128], BF16, tag="W32")
            nc.sync.dma_start(W32sb[:], d_W32[:])
            lg = cpool.tile([64, NG], BF16, tag="lg")
            nc.sync.dma_start(lg[:], d_lgT[:])
            Stsb = cpool.tile([128, 4 * NG], BF16, tag="St")
            nc.sync.dma_start(Stsb[:, :].rearrange("p (q g) -> p q g", q=4),
                              d_Sh[:, :, :].rearrange("q p g -> p q g"))
            F32R = mybir.dt.float32r

            bias = lambda i: BIsb[:, i:i + 1]

            # ---------- preamble: tok / atoms (needed before main loop) -----
            tfr = prepool.tile([128, 256], BF16, tag="tfr")
            nc.scalar.activation(tfr[:], tfx[:], AF.Silu)
            ps_tok = pspre.tile([128, 128], F32, tag="pre")
            nc.tensor.matmul(ps_tok[:], WEsb[:, 128:256],
                             tfr[:, 0:128], start=True, stop=False)
            nc.tensor.matmul(ps_tok[:], WEsb[:, 256:384],
                             tfr[:, 128:256], start=False, stop=True)
            tokT = cpool.tile([128, NT], F32, tag="tokT")
            nc.scalar.activation(tokT[:], ps_tok[:], AF.Identity, bias=bias(BI_TOK))

            ps_at = pspre.tile([128, NAH], F32, tag="pre")
            nc.tensor.matmul(ps_at[:], W64sb[:, 0:128], la[:], start=True, stop=True)
            atomsT = cpool.tile([128, NAH], BF16, tag="atomsT")
            nc.vector.tensor_scalar(atomsT[:], ps_at[:], bias(BI_ATOM), 0.0,
                                    op0=ALU.add, op1=ALU.add)

            # ---------- deferred preamble tasks (interleaved into loop) ----
            state = {}

            def task_silu1():
                s0 = cpool.tile([128, 1024], BF16, tag="s0")
                nc.scalar.activation(s0[:], m0sb[:], AF.Silu)
                state["s0"] = s0

            def task_S1():
                S1 = pspre.tile([32, 27], F32, tag="pre")
                for u in range(32):
                    nc.tensor.matmul(S1[:], state["s0"][:, 32 * u:32 * u + 32],
                                     M1sb[:, 27 * u:27 * u + 27],
                                     start=(u == 0), stop=(u == 31))
                S1b = prepool.tile([32, 27], BF16, tag="S1b")
                nc.scalar.activation(S1b[:], S1[:], AF.Copy)
                state["S1b"] = S1b

            def task_p1():
                pp = pspre.tile([128, 1], F32, tag="pre")
                for o in range(27):
                    nc.tensor.matmul(pp[:], W32sb[:, 128 * o:128 * o + 128],
                                     state["S1b"][:, o:o + 1],
                                     start=(o == 0), stop=(o == 26))
                sp1 = prepool.tile([128, 1], BF16, tag="sp1")
                nc.scalar.activation(sp1[:], pp[:], AF.Silu, bias=bias(BI_C1))
                state["sp1"] = sp1

            def task_silu0():
                s1 = prepool.tile([128, 256], BF16, tag="s1")
                nc.scalar.activation(s1[:], m1sb[:], AF.Silu)
                state["s1"] = s1

            def task_S0():
                S0 = pspre.tile([64, 27], F32, tag="pre")
                for u in range(4):
                    nc.tensor.matmul(S0[:], state["s1"][:, 64 * u:64 * u + 64],
                                     M0sb[:, 27 * u:27 * u + 27],
                                     start=(u == 0), stop=(u == 3))
                S0b = prepool.tile([64, 27], BF16, tag="S0b")
                nc.scalar.activation(S0b[:], S0[:], AF.Copy)
                state["S0b"] = S0b

            def task_p0():
                pp = pspre.tile([128, 1], F32, tag="pre")
                for o in range(27):
                    nc.tensor.matmul(pp[:], W0Tsb[:, 128 * o:128 * o + 128],
                                     state["S0b"][:, o:o + 1],
                                     start=(o == 0), stop=(o == 26))
                sp0 = prepool.tile([128, 1], BF16, tag="sp0")
                nc.scalar.activation(sp0[:], pp[:], AF.Silu, bias=bias(BI_C0))
                state["sp0"] = sp0

            def task_pocket():
                ps_pk = pspre.tile([128, 1], F32, tag="pre")
                nc.tensor.matmul(ps_pk[:], WBsb[:, OFF_WPK:OFF_WPK + 128],
                                 state["sp0"][:], start=True, stop=False)
                nc.tensor.matmul(ps_pk[:], WBsb[:, OFF_WPK + 128:OFF_WPK + 256],
                                 state["sp1"][:], start=False, stop=True)
                pocket = prepool.tile([128, 1], BF16, tag="pocket")
                nc.scalar.activation(pocket[:], ps_pk[:], AF.Identity, bias=bias(BI_PK))
                state["pocket"] = pocket

            def task_pf():
                tok_sum = prepool.tile([128, 1], F32, tag="toksum")
                junkt = jpool.tile([128, NT], F32, tag="junk")
                nc.vector.tensor_scalar(junkt[:], tokT[:], 1.0, 0.0, op0=ALU.mult,
                                        op1=ALU.add, accum_out=tok_sum[:])
                tok_sum_b = prepool.tile([128, 1], BF16, tag="toksumb")
                nc.scalar.activation(tok_sum_b[:], tok_sum[:], AF.Copy)
                ps_pf = pspre.tile([128, 2], F32, tag="pre")
                chunks = [state["pocket"], tok_sum_b, tok_sum_b]
                for q in range(3):
                    nc.tensor.matmul(ps_pf[:, 0:1],
                                     WBsb[:, OFF_WCAT + 128 * q:OFF_WCAT + 128 * (q + 1)],
                                     chunks[q][:], start=(q == 0), stop=(q == 2))
                for q in range(3):
                    nc.tensor.matmul(ps_pf[:, 1:2],
                                     WBsb[:, OFF_WGATE + 128 * q:OFF_WGATE + 128 * (q + 1)],
                                     chunks[q][:], start=(q == 0), stop=(q == 2))
                # sigmoid(z + bg) = 0.5 + 0.5*tanh(0.5z + 0.5bg)
                gt = prepool.tile([128, 1], F32, tag="gt")
                nc.scalar.activation(gt[:], ps_pf[:, 1:2], AF.Tanh,
                                     bias=bias(BI_GH), scale=0.5)
                pf_sig = prepool.tile([128, 1], F32, tag="pfsig")
                nc.gpsimd.tensor_scalar(pf_sig[:], gt[:], 0.5, 0.5, op0=ALU.mult, op1=ALU.add)
                pf_lin = prepool.tile([128, 1], F32, tag="pflin")
                nc.scalar.activation(pf_lin[:], ps_pf[:, 0:1], AF.Identity, bias=bias(BI_CAT))
                pf = prepool.tile([128, 1], BF16, tag="pf")
                nc.gpsimd.tensor_tensor(pf[:], pf_lin[:], pf_sig[:], op=ALU.mult)
                state["pf"] = pf

            def task_gf():
                ps_gf = pspre.tile([128, NG], F32, tag="pre")
                nc.tensor.matmul(ps_gf[:], W64sb[:, 128:256], lg[:], start=True, stop=True)
                gfT = prepool.tile([128, NG], BF16, tag="gfT")
                nc.scalar.activation(gfT[:], ps_gf[:], AF.Identity, bias=bias(BI_GR))
                state["gfT"] = gfT

            def task_bias1():
                ps_u = pspre.tile([128, 1], F32, tag="pre")
                nc.tensor.matmul(ps_u[:], WBsb[:, OFF_WB1:OFF_WB1 + 128],
                                 state["pf"][:], start=True, stop=True)
                ub = prepool.tile([128, 1], F32, tag="ub")
                nc.scalar.activation(ub[:], ps_u[:], AF.Identity, bias=bias(BI_B1))
                ps_hb = pspre.tile([128, NG], F32, tag="pre")
                nc.tensor.matmul(ps_hb[:], WBsb[:, OFF_WB1 + 128:OFF_WB1 + 256],
                                 state["gfT"][:], start=True, stop=True)
                hb = prepool.tile([128, NG], BF16, tag="hb")
                nc.scalar.activation(hb[:], ps_hb[:], AF.Prelu, bias=ub[:], alpha=0.01)
                state["hb"] = hb

            def task_bias2():
                ps_b2 = pspre.tile([1, NG], F32, tag="pre")
                nc.tensor.matmul(ps_b2[:], WBsb[:, OFF_WB2:OFF_WB2 + 1],
                                 state["hb"][:], start=True, stop=True)
                nc.scalar.activation(res[:, NG:2 * NG], ps_b2[:], AF.Identity, bias=bb2)

            pre_tasks = [task_silu1, task_S1, task_p1, task_silu0, task_S0,
                         task_p0, task_pocket, task_pf, task_gf, task_bias1,
                         task_bias2]
            TASK_AT = {12 + 4 * i: t for i, t in enumerate(pre_tasks)}

            res = cpool.tile([1, 128], F32, tag="res")

            # ---------- main loop ----------
            # 64 units u of 2 tokens; y2[o, 512v + a] for token j = 2u+v.
            # zq8 (per 64-token block) col layout: 8*(j%64) + 2*a_chunk + {pe,pg}
            wpegr = cpool.tile([128, 2], F32R, tag="wpegr")
            nc.scalar.activation(wpegr[:], BIsb[:, BI_WPEG:BI_WPEG + 2], AF.Copy)
            wpeg_ap = wpegr[:]
            upeg_ap = WBsb[:, OFF_UPEG:OFF_UPEG + 2]
            wint_ap = WEsb[:, 0:128]
            zq_tiles = [None, None]
            ae_parts = cpool.tile([128, 20], F32, tag="aeparts")
            pending = []

            def emit_unit(u):
                y2 = psy.tile([128, 1024], F32, tag="y")
                ujs = []
                for v in range(2):
                    j = 2 * u + v
                    Wj = xpool.tile([128, 128], BF16, tag="x")
                    nc.gpsimd.tensor_scalar_mul(Wj[:], wint_ap, tokT[:, j:j + 1])
                    nc.tensor.matmul(y2[:, 512 * v:512 * (v + 1)], Wj[:], atomsT[:],
                                     start=True, stop=True)
                    if ENG[u] == 'B':
                        uj = upool.tile([128, 2], BF16, tag="u")
                        nc.gpsimd.tensor_scalar_mul(uj[:], upeg_ap, tokT[:, j:j + 1])
                        ujs.append(uj)
                return (u, y2, ujs)

            def flush_unit(ent):
                u, y2, ujs = ent
                h = hpool.tile([128, 1024], F32R, tag="h")
                if ENG[u] == 'A':
                    nc.scalar.activation(h[:], y2[:], AF.Prelu, bias=bias(BI_INT),
                                         alpha=0.01)
                else:
                    # h = 0.99*relu(y); the 0.01*y linear part of lrelu is
                    # folded into the zq accumulation via upeg below
                    nc.vector.tensor_scalar(h[:], y2[:], 0.0, 0.99,
                                            op0=ALU.max, op1=ALU.mult)
                for v in range(2):
                    j = 2 * u + v
                    b, jj = j // 64, j % 64
                    if zq_tiles[b] is None:
                        zq_tiles[b] = psz.tile([128, 512], F32, tag="z", name=f"zq{b}")
                    zq = zq_tiles[b]
                    for a in range(4):
                        cols = zq[:, 8 * jj + 2 * a:8 * jj + 2 * a + 2]
                        if ENG[u] == 'A':
                            nc.tensor.matmul(cols, h[:, 512 * v + 128 * a:512 * v + 128 * (a + 1)],
                                             wpeg_ap, start=True, stop=True)
                        else:
                            nc.tensor.matmul(cols, h[:, 512 * v + 128 * a:512 * v + 128 * (a + 1)],
                                             wpeg_ap, start=True, stop=False)
                            nc.tensor.matmul(cols, atomsT[:, 128 * a:128 * (a + 1)],
                                             ujs[v][:], start=False, stop=True)

            def gates(b, c0, c1, slot):
                # process zq cols [c0:c1] -> ae_parts cols 4*slot : 4*slot+4
                zq = zq_tiles[b]
                n2 = (c1 - c0) // 2
                s = gpool.tile([128, 256], F32, tag="s")
                nc.scalar.activation(s[:, 0:n2], zq[:, c0 + 1:c1:2], AF.Tanh,
                                     bias=0.5 * bpg, scale=0.5)
                w = gpool.tile([128, 256], F32, tag="w")
                nc.gpsimd.tensor_scalar(w[:, 0:n2], s[:, 0:n2], 0.5, 0.5,
                                        op0=ALU.mult, op1=ALU.add)
                t = gpool.tile([128, 256], F32, tag="t")
                nc.vector.scalar_tensor_tensor(t[:, 0:n2], zq[:, c0:c1:2], bpe, w[:, 0:n2],
                                               op0=ALU.add, op1=ALU.mult)
                for a in range(4):
                    junka = jpool.tile([128, 64], F32, tag="junka")
                    nc.vector.tensor_scalar(junka[:, 0:n2 // 4], t[:, a:n2:4], 1.0, 0.0,
                                            op0=ALU.mult, op1=ALU.add,
                                            accum_out=ae_parts[:, 4 * slot + a:
                                                              4 * slot + a + 1])

            for u in range(64):
                pending.append(emit_unit(u))
                if len(pending) > 1:
                    flush_unit(pending.pop(0))
                fu = u - 1  # unit just flushed
                if fu == 15:
                    gates(0, 0)
                elif fu == 31:
                    gates(0, 1)
                elif fu == 47:
                    gates(1, 0)
                if fu in TASK_AT:
                    _old_pri = tc.cur_priority
                    tc.cur_priority = _old_pri + 100000
                    TASK_AT[fu]()
                    tc.cur_priority = _old_pri

            flush_unit(pending.pop(0))
            gates(1, 1)

            # atom_e reduce -> seg matmul -> out
            ae8 = prepool.tile([128, 8], F32, tag="ae8")
            nc.gpsimd.tensor_tensor(ae8[:], ae_parts[:, 0:8], ae_parts[:, 8:16], op=ALU.add)
            ae4f = prepool.tile([128, 4], F32, tag="ae4f")
            nc.gpsimd.tensor_tensor(ae4f[:], ae8[:, 0:4], ae8[:, 4:8], op=ALU.add)
            ae4b = prepool.tile([128, 4], BF16, tag="ae4b")
            nc.vector.tensor_tensor(ae4b[:], ae4f[:], ae_parts[:, 16:20], op=ALU.add)
            ps_seg = pspre.tile([1, NG], F32, tag="pre")
            for q in range(4):
                nc.tensor.matmul(ps_seg[:], ae4b[:, q:q + 1], Stsb[:, q * NG:(q + 1) * NG],
                                 start=(q == 0), stop=(q == 3))
            nc.scalar.activation(res[:, 0:NG], ps_seg[:], AF.Copy)
            nc.sync.dma_start(d_res[:], res[:])

    _legalize_waits(nc)
    nc._tile_ctx = tc_ref
    return nc


def kernel(**inputs) -> np.ndarray:
    f = lambda a: np.ascontiguousarray(np.asarray(a), dtype=np.float32)
    bf = lambda a: np.ascontiguousarray(np.asarray(a, dtype=np.float32)).astype(ml_dtypes.bfloat16)
    tf = f(inputs["token_features"])
    la = f(inputs["lig_atom"])
    lgr = f(inputs["lig_graph"])
    m0 = f(inputs["ms_feat_0"])
    m1 = f(inputs["ms_feat_1"])
    lb = np.asarray(inputs["ligand_batch"])
    S = (lb[:, None] == np.arange(NG)[None, :]).astype(np.float32)

    # ---- weight prep (host-side layout/scale transforms only) ----
    wint_bf = bf(inputs["W_int"])                       # [128,128]
    wpe = f(inputs["W_pe"]); wpg = f(inputs["W_pg"])    # [128,1]
    wpeg = np.concatenate([wpe, wpg], axis=1)           # [128,2]
    u_pe = wint_bf.astype(np.float64) @ wpe.astype(np.float64)
    u_pg = wint_bf.astype(np.float64) @ wpg.astype(np.float64)
    upeg = 0.01 * np.concatenate([u_pe, u_pg], axis=1)  # [128,2]

    wcat = f(inputs["W_cat"]).copy()                    # [384,128]
    wgate = f(inputs["W_gate"]).copy()
    wcat[2 * HID:] /= float(NT)
    wgate[2 * HID:] /= float(NT)

    WB = np.zeros((128, NWB), dtype=np.float32)
    WB[:, OFF_WINT:OFF_WINT + 128] = wint_bf.astype(np.float32)
    WB[:, OFF_WTOK:OFF_WTOK + 256] = f(inputs["W_token"]).reshape(2, 128, HID).transpose(1, 0, 2).reshape(128, 256)
    WB[:, OFF_WPK:OFF_WPK + 256] = f(inputs["W_pocket"]).reshape(2, 128, HID).transpose(1, 0, 2).reshape(128, 256)
    WB[:, OFF_WCAT:OFF_WCAT + 384] = wcat.reshape(3, 128, HID).transpose(1, 0, 2).reshape(128, 384)
    WB[:, OFF_WGATE:OFF_WGATE + 384] = wgate.reshape(3, 128, HID).transpose(1, 0, 2).reshape(128, 384)
    WB[:, OFF_WB1:OFF_WB1 + 256] = f(inputs["W_bias1"]).reshape(2, 128, HID).transpose(1, 0, 2).reshape(128, 256)
    WB[:, OFF_WB2:OFF_WB2 + 1] = f(inputs["W_bias2"])
    WB[:, OFF_WPEG:OFF_WPEG + 2] = wpeg
    WB[:, OFF_UPEG:OFF_UPEG + 2] = upeg
    WB_bf = WB.astype(ml_dtypes.bfloat16)

    # conv weights as [c, off*128 + o], scaled by 1/num_output_positions
    Wc0 = f(inputs["Wc0"])  # [128,64,3,3,3] applied to ms_feat_1
    Wc1 = f(inputs["Wc1"])  # [128,32,3,3,3] applied to ms_feat_0
    W0T = np.ascontiguousarray(Wc0.reshape(128, 64, 27).transpose(1, 2, 0)).reshape(64, 27 * 128) / 216.0
    W32 = np.ascontiguousarray(Wc1.reshape(128, 32, 27).transpose(1, 2, 0)).reshape(32, 27 * 128) / 2744.0

    W64 = np.zeros((64, 256), dtype=np.float32)
    W64[:, 0:128] = f(inputs["W_atom"])
    W64[:, 128:256] = f(inputs["W_graph"])

    col = lambda a: f(a).reshape(128, 1)
    BI = np.zeros((128, NBI), dtype=np.float32)
    BI[:, BI_TOK] = f(inputs["b_token"])
    BI[:, BI_ATOM] = f(inputs["b_atom"])
    BI[:, BI_INT] = f(inputs["b_int"])
    BI[:, BI_PK] = f(inputs["b_pocket"])
    BI[:, BI_CAT] = f(inputs["b_cat"])
    BI[:, BI_GH] = 0.5 * f(inputs["b_gate"])
    BI[:, BI_GR] = f(inputs["b_graph"])
    BI[:, BI_B1] = f(inputs["b_bias1"])
    BI[:, BI_C0] = f(inputs["bc0"])
    BI[:, BI_C1] = f(inputs["bc1"])
    BI[:, BI_WPEG:BI_WPEG + 2] = wpeg

    # window-membership masks: M[pos, off] = 1 iff pos-off in valid out range
    def win_mask(D, O):
        g = np.arange(D)
        z, y, x = np.meshgrid(g, g, g, indexing="ij")
        pos = np.stack([z.ravel(), y.ravel(), x.ravel()], 1)  # [D^3, 3]
        d = np.arange(3)
        dz, dy, dx = np.meshgrid(d, d, d, indexing="ij")
        off = np.stack([dz.ravel(), dy.ravel(), dx.ravel()], 1)  # [27, 3]
        r = pos[:, None, :] - off[None, :, :]
        return np.all((r >= 0) & (r < O), axis=2).astype(np.float32)  # [D^3, 27]

    M1 = win_mask(16, 14).reshape(32, 128, 27)
    M0 = win_mask(8, 6).reshape(4, 128, 27)

    bpe = float(np.asarray(inputs["b_pe"]).reshape(-1)[0])
    bpg = float(np.asarray(inputs["b_pg"]).reshape(-1)[0])
    bb2 = float(np.asarray(inputs["b_bias2"]).reshape(-1)[0])

    WE = np.zeros((128, 384), dtype=np.float32)
    WE[:, 0:128] = WB[:, OFF_WINT:OFF_WINT + 128]
    WE[:, 128:384] = WB[:, OFF_WTOK:OFF_WTOK + 256]
    shared = {
        "WB": WB_bf, "BI": BI,
        "WE": WE.astype(ml_dtypes.bfloat16),
        "W64": W64.astype(ml_dtypes.bfloat16),
        "W0T": W0T.astype(ml_dtypes.bfloat16),
        "W32": W32.astype(ml_dtypes.bfloat16),
        "M1m": M1.astype(ml_dtypes.bfloat16),
        "M0m": M0.astype(ml_dtypes.bfloat16),
    }

    in_maps = []
    for c in range(NCORES):
        n, h = c // 2, c % 2
        m = dict(shared)
        m["tfX"] = np.ascontiguousarray(tf[n].T.reshape(2, 128, 128))
        m["laT"] = bf(la[n, 512 * h:512 * (h + 1)].T)
        m["lgT"] = bf(lgr[n].T)
        m["m0T"] = bf(m0[n].reshape(32, 4096).T.reshape(32, 128, 32))
        m["m1T"] = bf(m1[n].reshape(64, 512).T.reshape(4, 128, 64))
        m["Sh"] = bf(S[512 * h:512 * (h + 1)].reshape(4, 128, NG))
        in_maps.append(m)

    bint_zero = bool(np.all(np.asarray(inputs["b_int"]) == 0.0))
    nc = build_program(bpe, bpg, bb2, bint_zero)
    r = run_bass_kernel_spmd(nc, in_maps, core_ids=list(range(NCORES)),
                             trace=TRACE, **(TRACE_KW if TRACE else {}))
    global LAST
    LAST = r
    res = r.results

    out = np.zeros((NI, NG), dtype=np.float32)
    for n in range(NI):
        out[n] = (res[2 * n]["res_out"][0, 0:NG] + res[2 * n + 1]["res_out"][0, 0:NG]
                  + res[2 * n]["res_out"][0, NG:2 * NG])
    return out


# revision 29
# speedup vs baseline: 2.1396x; 1.0021x over previous
import sys
import numpy as np
import ml_dtypes

sys.path.insert(0, "/opt/trn_rl_repo")

import concourse.bass as bass
import concourse.tile as tile
from concourse import mybir
from concourse.bass_utils import run_bass_kernel_spmd

F32 = mybir.dt.float32
BF16 = mybir.dt.bfloat16
AF = mybir.ActivationFunctionType
ALU = mybir.AluOpType

HID = 128
NT = 128       # tokens per image
NAH = 512      # atoms per core (half of 1024)
NG = 64        # ligand graphs
NI = 4         # images
NCORES = 8

# WB (128-partition weight concat, bf16) column offsets
OFF_WINT = 0
OFF_WTOK = 128
OFF_WPK = 384
OFF_WCAT = 640
OFF_WGATE = 1024
OFF_WB1 = 1408
OFF_WB2 = 1664
OFF_WPEG = 1665
OFF_UPEG = 1667
NWB = 1669

# BI (f32 bias concat) columns
BI_TOK, BI_ATOM, BI_INT, BI_PK, BI_CAT, BI_GH, BI_GR, BI_B1, BI_C0, BI_C1 = range(10)
BI_WPEG = 10   # cols 10:12 = [W_pe, W_pg] f32
NBI = 12

# lrelu unit assignment: 'A' = ACT Prelu, 'B' = DVE relu99 + linear-fold
N_A_UNITS = 32

TRACE = False
TRACE_KW = {}
LAST = None


_COMPUTE_INSTS = (
    "InstActivation", "InstTensorCopy", "InstTensorScalar", "InstTensorScalarPtr",
    "InstTensorTensor", "InstTensorTensorReduce", "InstTensorReduce", "InstMemSet",
    "InstMatmult", "InstScalarTensorTensor", "InstTensorTensorScan", "InstLdweights",
    "InstDMACopy", "InstDMATransposeAnt", "InstTriggeredCopy", "InstDrain",
    "InstEventSemaphoreOp", "InstSemaphoreOp", "InstCopy", "InstIota", "InstSelect",
)


def _legalize_waits(nc):
    # walrus in this toolchain accepts at most ONE sync wait on TPB compute
    # instructions; hoist extras into same-engine NoOps placed just before.
    k = 0
    for f in nc.m.functions:
        for blk in f.blocks:
            insts = blk.instructions
            out = []
            for ins in insts:
                si = getattr(ins, "sync_info", None)
                if (si is not None and len(si.on_wait) > 1
                        and type(ins).__name__ in _COMPUTE_INSTS):
                    waits = list(si.on_wait)
                    for w in waits[:-1]:
                        nop = mybir.InstNoOp(
                            name=f"WNOP-{k}", engine=ins.engine,
                            sync_info=mybir.SyncInfo(on_wait=[w], on_update=[]))
                        k += 1
                        out.append(nop)
                    ins.sync_info = mybir.SyncInfo(on_wait=[waits[-1]],
                                                   on_update=list(si.on_update))
                out.append(ins)
            blk.instructions = out
    return k


def _register_const(nc, val, dtype=F32):
    if (dtype, float(val)) in nc.const_aps.aps:
        return
    t = nc.alloc_sbuf_tensor(f"uconst-{dtype.name}-{val}", [128, 1], dtype)
    nc.gpsimd.memset(t.ap(), float(val))
    nc.const_aps.aps[(dtype, float(val))] = t.ap()


def _unit_engines():
    # interleave N_A_UNITS 'A' units among 64 as evenly as possible
    eng = []
    for u in range(64):
        if (u + 1) * N_A_UNITS // 64 > u * N_A_UNITS // 64:
            eng.append('A')
        else:
            eng.append('B')
    return eng


def build_program(bpe: float, bpg: float, bb2: float, bint_zero: bool = True,
                  sim_trace: bool = False) -> bass.Bass:
    nc = bass.Bass()
    _register_const(nc, 0.5 * bpg)
    _register_const(nc, bb2)
    nc.all_engine_barrier()

    # ---- DRAM inputs (per-core views; same names across SPMD cores) ----
    d_WB = nc.dram_tensor("WB", [128, NWB], BF16, kind="ExternalInput")
    d_WE = nc.dram_tensor("WE", [128, 384], BF16, kind="ExternalInput")
    d_BI = nc.dram_tensor("BI", [128, NBI], F32, kind="ExternalInput")
    d_tfX = nc.dram_tensor("tfX", [2, 128, 128], BF16, kind="ExternalInput")
    d_laT = nc.dram_tensor("laT", [64, NAH], BF16, kind="ExternalInput")
    d_W64 = nc.dram_tensor("W64", [64, 256], BF16, kind="ExternalInput")
    d_m0T = nc.dram_tensor("m0T", [32, 128, 32], BF16, kind="ExternalInput")
    d_M1 = nc.dram_tensor("M1m", [32, 128, 27], BF16, kind="ExternalInput")
    d_m1T = nc.dram_tensor("m1T", [4, 128, 64], BF16, kind="ExternalInput")
    d_M0 = nc.dram_tensor("M0m", [4, 128, 27], BF16, kind="ExternalInput")
    d_W0T = nc.dram_tensor("W0T", [64, 27 * 128], BF16, kind="ExternalInput")
    d_W32 = nc.dram_tensor("W32", [32, 27 * 128], BF16, kind="ExternalInput")
    d_lgT = nc.dram_tensor("lgT", [64, NG], BF16, kind="ExternalInput")
    d_Sh = nc.dram_tensor("Sh", [4, 128, NG], BF16, kind="ExternalInput")

    d_res = nc.dram_tensor("res_out", [1, 128], F32, kind="ExternalOutput")

    ENG = _unit_engines()
    if not bint_zero:
        ENG[:] = ['A'] * 64

    tc_ref = tile.TileContext(nc, trace_sim=sim_trace)
    with tc_ref as tc:
        with (
            tc.tile_pool(name="const", bufs=1) as cpool,
            tc.tile_pool(name="pre", bufs=1) as prepool,
            tc.tile_pool(name="x", bufs=12) as xpool,
            tc.tile_pool(name="u", bufs=8) as upool,
            tc.tile_pool(name="h", bufs=8) as hpool,
            tc.tile_pool(name="g", bufs=3) as gpool,
            tc.tile_pool(name="j", bufs=4) as jpool,
            tc.tile_pool(name="ps_y", bufs=3, space="PSUM") as psy,
            tc.tile_pool(name="ps_z", bufs=1, space="PSUM") as psz,
            tc.tile_pool(name="ps_p", bufs=1, space="PSUM") as pspre,
        ):
            # ---------- engine warmups (hide ACT table load + start PE pstate clock)
            warm = cpool.tile([128, 1], F32, tag="warm")
            nc.gpsimd.memset(warm[:], 0.0)
            warma = cpool.tile([128, 1], F32, tag="warma")
            nc.scalar.activation(warma[:], warm[:], AF.Silu)
            ps_warm = pspre.tile([1, 1], F32, tag="pre")
            nc.tensor.matmul(ps_warm[:], warm[:], warm[:], start=True, stop=True)
            warmb = cpool.tile([1, 1], F32, tag="warmb")
            nc.scalar.activation(warmb[:], ps_warm[:], AF.Copy)

            # ---------- input DMAs (order = DMA device service priority) ----
            tfx = prepool.tile([128, 256], BF16, tag="tfx")
            nc.sync.dma_start(tfx[:, :].rearrange("p (c k) -> p c k", c=2),
                              d_tfX[:, :, :].rearrange("c p k -> p c k"))
            WEsb = cpool.tile([128, 384], BF16, tag="WE")
            nc.sync.dma_start(WEsb[:], d_WE[:])
            la = prepool.tile([64, NAH], BF16, tag="la")
            nc.sync.dma_start(la[:], d_laT[:])
            W64sb = cpool.tile([64, 256], BF16, tag="W64")
            nc.sync.dma_start(W64sb[:], d_W64[:])
            BIsb = cpool.tile([128, NBI], F32, tag="BI")
            nc.sync.dma_start(BIsb[:], d_BI[:])
            WBsb = cpool.tile([128, NWB], BF16, tag="WB")
            nc.sync.dma_start(WBsb[:], d_WB[:])
            m0sb = cpool.tile([128, 1024], BF16, tag="m0")
            nc.sync.dma_start(m0sb[:, :].rearrange("p (u c) -> p u c", u=32),
                              d_m0T[:, :, :].rearrange("u p c -> p u c"))
            M1sb = cpool.tile([128, 864], BF16, tag="M1")
            nc.sync.dma_start(M1sb[:, :].rearrange("p (u o) -> p u o", u=32),
                              d_M1[:, :, :].rearrange("u p o -> p u o"))
            m1sb = cpool.tile([128, 256], BF16, tag="m1")
            nc.sync.dma_start(m1sb[:, :].rearrange("p (u c) -> p u c", u=4),
                              d_m1T[:, :, :].rearrange("u p c -> p u c"))
            M0sb = cpool.tile([128, 108], BF16, tag="M0")
            nc.sync.dma_start(M0sb[:, :].rearrange("p (u o) -> p u o", u=4),
                              d_M0[:, :, :].rearrange("u p o -> p u o"))
            W0Tsb = cpool.tile([64, 27 * 128], BF16, tag="W0T")
            nc.sync.dma_start(W0Tsb[:], d_W0T[:])
            W32sb = cpool.tile([32, 27 * 128], BF16, tag="W32")
            nc.sync.dma_start(W32sb[:], d_W32[:])
            lg = cpool.tile([64, NG], BF16, tag="lg")
            nc.sync.dma_start(lg[:], d_lgT[:])
            Stsb = cpool.tile([128, 4 * NG], BF16, tag="St")
            nc.sync.dma_start(Stsb[:, :].rearrange("p (q g) -> p q g", q=4),
                              d_Sh[:, :, :].rearrange("q p g -> p q g"))
            F32R = mybir.dt.float32r

            bias = lambda i: BIsb[:, i:i + 1]

            # ---------- preamble: tok / atoms (needed before main loop) -----
            tfr = prepool.tile([128, 256], BF16, tag="tfr")
            nc.scalar.activation(tfr[:], tfx[:], AF.Silu)
            ps_tok = pspre.tile([128, 128], F32, tag="pre")
            nc.tensor.matmul(ps_tok[:], WEsb[:, 128:256],
                             tfr[:, 0:128], start=True, stop=False)
            nc.tensor.matmul(ps_tok[:], WEsb[:, 256:384],
                             tfr[:, 128:256], start=False, stop=True)
            tokT = cpool.tile([128, NT], F32, tag="tokT")
            nc.scalar.activation(tokT[:], ps_tok[:], AF.Identity, bias=bias(BI_TOK))

            ps_at = pspre.tile([128, NAH], F32, tag="pre")
            nc.tensor.matmul(ps_at[:], W64sb[:, 0:128], la[:], start=True, stop=True)
            atomsT = cpool.tile([128, NAH], BF16, tag="atomsT")
            nc.vector.tensor_scalar(atomsT[:], ps_at[:], bias(BI_ATOM), 0.0,
                                    op0=ALU.add, op1=ALU.add)

            # ---------- deferred preamble tasks (interleaved into loop) ----
            state = {}

            def task_silu1():
                s0 = cpool.tile([128, 1024], BF16, tag="s0")
                nc.scalar.activation(s0[:], m0sb[:], AF.Silu)
                state["s0"] = s0

            def task_S1():
                S1 = pspre.tile([32, 27], F32, tag="pre")
                for u in range(32):
                    nc.tensor.matmul(S1[:], state["s0"][:, 32 * u:32 * u + 32],
                                     M1sb[:, 27 * u:27 * u + 27],
                                     start=(u == 0), stop=(u == 31))
                S1b = prepool.tile([32, 27], BF16, tag="S1b")
                nc.scalar.activation(S1b[:], S1[:], AF.Copy)
                state["S1b"] = S1b

            def task_p1():
                pp = pspre.tile([128, 1], F32, tag="pre")
                for o in range(27):
                    nc.tensor.matmul(pp[:], W32sb[:, 128 * o:128 * o + 128],
                                     state["S1b"][:, o:o + 1],
                                     start=(o == 0), stop=(o == 26))
                sp1 = prepool.tile([128, 1], BF16, tag="sp1")
                nc.scalar.activation(sp1[:], pp[:], AF.Silu, bias=bias(BI_C1))
                state["sp1"] = sp1

            def task_silu0():
                s1 = prepool.tile([128, 256], BF16, tag="s1")
                nc.scalar.activation(s1[:], m1sb[:], AF.Silu)
                state["s1"] = s1

            def task_S0():
                S0 = pspre.tile([64, 27], F32, tag="pre")
                for u in range(4):
                    nc.tensor.matmul(S0[:], state["s1"][:, 64 * u:64 * u + 64],
                                     M0sb[:, 27 * u:27 * u + 27],
                                     start=(u == 0), stop=(u == 3))
                S0b = prepool.tile([64, 27], BF16, tag="S0b")
                nc.scalar.activation(S0b[:], S0[:], AF.Copy)
                state["S0b"] = S0b

            def task_p0():
                pp = pspre.tile([128, 1], F32, tag="pre")
                for o in range(27):
                    nc.tensor.matmul(pp[:], W0Tsb[:, 128 * o:128 * o + 128],
                                     state["S0b"][:, o:o + 1],
                                     start=(o == 0), stop=(o == 26))
                sp0 = prepool.tile([128, 1], BF16, tag="sp0")
                nc.scalar.activation(sp0[:], pp[:], AF.Silu, bias=bias(BI_C0))
                state["sp0"] = sp0

            def task_pocket():
                ps_pk = pspre.tile([128, 1], F32, tag="pre")
                nc.tensor.matmul(ps_pk[:], WBsb[:, OFF_WPK:OFF_WPK + 128],
                                 state["sp0"][:], start=True, stop=False)
                nc.tensor.matmul(ps_pk[:], WBsb[:, OFF_WPK + 128:OFF_WPK + 256],
                                 state["sp1"][:], start=False, stop=True)
                pocket = prepool.tile([128, 1], BF16, tag="pocket")
                nc.scalar.activation(pocket[:], ps_pk[:], AF.Identity, bias=bias(BI_PK))
                state["pocket"] = pocket

            def task_pf():
                junkt = jpool.tile([128, NT], BF16, tag="junk")
                tok_sum = prepool.tile([128, 1], F32, tag="toksum")
                nc.scalar.activation(junkt[:], tokT[:], AF.Identity,
                                     accum_out=tok_sum[:])
                tok_sum_b = prepool.tile([128, 1], BF16, tag="toksumb")
                nc.scalar.activation(tok_sum_b[:], tok_sum[:], AF.Copy)
                ps_pf = pspre.tile([128, 2], F32, tag="pre")
                chunks = [state["pocket"], tok_sum_b, tok_sum_b]
                for q in range(3):
                    nc.tensor.matmul(ps_pf[:, 0:1],
                                     WBsb[:, OFF_WCAT + 128 * q:OFF_WCAT + 128 * (q + 1)],
                                     chunks[q][:], start=(q == 0), stop=(q == 2))
                for q in range(3):
                    nc.tensor.matmul(ps_pf[:, 1:2],
                                     WBsb[:, OFF_WGATE + 128 * q:OFF_WGATE + 128 * (q + 1)],
                                     chunks[q][:], start=(q == 0), stop=(q == 2))
                # sigmoid(z + bg) = 0.5 + 0.5*tanh(0.5z + 0.5bg)
                gt = prepool.tile([128, 1], F32, tag="gt")
                nc.scalar.activation(gt[:], ps_pf[:, 1:2], AF.Tanh,
                                     bias=bias(BI_GH), scale=0.5)
                pf_sig = prepool.tile([128, 1], F32, tag="pfsig")
                nc.gpsimd.tensor_scalar(pf_sig[:], gt[:], 0.5, 0.5, op0=ALU.mult, op1=ALU.add)
                pf_lin = prepool.tile([128, 1], F32, tag="pflin")
                nc.scalar.activation(pf_lin[:], ps_pf[:, 0:1], AF.Identity, bias=bias(BI_CAT))
                pf = prepool.tile([128, 1], BF16, tag="pf")
                nc.gpsimd.tensor_tensor(pf[:], pf_lin[:], pf_sig[:], op=ALU.mult)
                state["pf"] = pf

            def task_gf():
                ps_gf = pspre.tile([128, NG], F32, tag="pre")
                nc.tensor.matmul(ps_gf[:], W64sb[:, 128:256], lg[:], start=True, stop=True)
                gfT = prepool.tile([128, NG], BF16, tag="gfT")
                nc.scalar.activation(gfT[:], ps_gf[:], AF.Identity, bias=bias(BI_GR))
                state["gfT"] = gfT

            def task_bias1():
                ps_u = pspre.tile([128, 1], F32, tag="pre")
                nc.tensor.matmul(ps_u[:], WBsb[:, OFF_WB1:OFF_WB1 + 128],
                                 state["pf"][:], start=True, stop=True)
                ub = prepool.tile([128, 1], F32, tag="ub")
                nc.scalar.activation(ub[:], ps_u[:], AF.Identity, bias=bias(BI_B1))
                ps_hb = pspre.tile([128, NG], F32, tag="pre")
                nc.tensor.matmul(ps_hb[:], WBsb[:, OFF_WB1 + 128:OFF_WB1 + 256],
                                 state["gfT"][:], start=True, stop=True)
                hb = prepool.tile([128, NG], BF16, tag="hb")
                nc.scalar.activation(hb[:], ps_hb[:], AF.Prelu, bias=ub[:], alpha=0.01)
                state["hb"] = hb

            def task_bias2():
                ps_b2 = pspre.tile([1, NG], F32, tag="pre")
                nc.tensor.matmul(ps_b2[:], WBsb[:, OFF_WB2:OFF_WB2 + 1],
                                 state["hb"][:], start=True, stop=True)
                nc.scalar.activation(res[:, NG:2 * NG], ps_b2[:], AF.Identity, bias=bb2)

            pre_tasks = [task_silu1, task_S1, task_p1, task_silu0, task_S0,
                         task_p0, task_pocket, task_pf, task_gf, task_bias1,
                         task_bias2]
            TASK_AT = {12 + 4 * i: t for i, t in enumerate(pre_tasks)}

            res = cpool.tile([1, 128], F32, tag="res")

            # ---------- main loop ----------
            # 64 units u of 2 tokens; y2[o, 512v + a] for token j = 2u+v.
            # zq8 (per 64-token block) col layout: 8*(j%64) + 2*a_chunk + {pe,pg}
            wpegr = cpool.tile([128, 2], F32R, tag="wpegr")
            nc.scalar.activation(wpegr[:], BIsb[:, BI_WPEG:BI_WPEG + 2], AF.Copy)
            wpeg_ap = wpegr[:]
            upeg_ap = WBsb[:, OFF_UPEG:OFF_UPEG + 2]
            wint_ap = WEsb[:, 0:128]
            zq_tiles = [None, None]
            ae_parts = cpool.tile([128, 20], F32, tag="aeparts")
            pending = []

            def emit_unit(u):
                y2 = psy.tile([128, 1024], F32, tag="y")
                ujs = []
                for v in range(2):
                    j = 2 * u + v
                    Wj = xpool.tile([128, 128], BF16, tag="x")
                    nc.gpsimd.tensor_scalar_mul(Wj[:], wint_ap, tokT[:, j:j + 1])
                    nc.tensor.matmul(y2[:, 512 * v:512 * (v + 1)], Wj[:], atomsT[:],
                                     start=True, stop=True)
                    if ENG[u] == 'B':
                        uj = upool.tile([128, 2], BF16, tag="u")
                        nc.gpsimd.tensor_scalar_mul(uj[:], upeg_ap, tokT[:, j:j + 1])
                        ujs.append(uj)
                return (u, y2, ujs)

            def flush_unit(ent):
                u, y2, ujs = ent
                h = hpool.tile([128, 1024], F32R, tag="h")
                if ENG[u] == 'A':
                    nc.scalar.activation(h[:], y2[:], AF.Prelu, bias=bias(BI_INT),
                                         alpha=0.01)
                else:
                    # h = 0.99*relu(y); the 0.01*y linear part of lrelu is
                    # folded into the zq accumulation via upeg below
                    nc.vector.tensor_scalar(h[:], y2[:], 0.0, 0.99,
                                            op0=ALU.max, op1=ALU.mult)
                for v in range(2):
                    j = 2 * u + v
                    b, jj = j // 64, j % 64
                    if zq_tiles[b] is None:
                        zq_tiles[b] = psz.tile([128, 512], F32, tag="z", name=f"zq{b}")
                    zq = zq_tiles[b]
                    for a in range(4):
                        cols = zq[:, 8 * jj + 2 * a:8 * jj + 2 * a + 2]
                        if ENG[u] == 'A':
                            nc.tensor.matmul(cols, h[:, 512 * v + 128 * a:512 * v + 128 * (a + 1)],
                                             wpeg_ap, start=True, stop=True)
                        else:
                            nc.tensor.matmul(cols, h[:, 512 * v + 128 * a:512 * v + 128 * (a + 1)],
                                             wpeg_ap, start=True, stop=False)
                            nc.tensor.matmul(cols, atomsT[:, 128 * a:128 * (a + 1)],
                                             ujs[v][:], start=False, stop=True)

            def gates(b, c0, c1, slot):
                # process zq cols [c0:c1] -> ae_parts cols 4*slot : 4*slot+4
                zq = zq_tiles[b]
                n2 = (c1 - c0) // 2
                s = gpool.tile([128, 256], F32, tag="s")
                nc.scalar.activation(s[:, 0:n2], zq[:, c0 + 1:c1:2], AF.Tanh,
                                     bias=0.5 * bpg, scale=0.5)
                w = gpool.tile([128, 256], F32, tag="w")
                nc.gpsimd.tensor_scalar(w[:, 0:n2], s[:, 0:n2], 0.5, 0.5,
                                        op0=ALU.mult, op1=ALU.add)
                t = gpool.tile([128, 256], F32, tag="t")
                nc.vector.scalar_tensor_tensor(t[:, 0:n2], zq[:, c0:c1:2], bpe, w[:, 0:n2],
                                               op0=ALU.add, op1=ALU.mult)
                for a in range(4):
                    junka = jpool.tile([128, 64], F32, tag="junka")
                    nc.vector.tensor_scalar(junka[:, 0:n2 // 4], t[:, a:n2:4], 1.0, 0.0,
                                            op0=ALU.mult, op1=ALU.add,
                                            accum_out=ae_parts[:, 4 * slot + a:
                                                              4 * slot + a + 1])

            for u in range(64):
                pending.append(emit_unit(u))
                if len(pending) > 1:
                    flush_unit(pending.pop(0))
                fu = u - 1  # unit just flushed
                if fu == 15:
                    gates(0, 0)
                elif fu == 31:
                    gates(0, 1)
                elif fu == 47:
                    gates(1, 0)
                if fu in TASK_AT:
                    _old_pri = tc.cur_priority
                    tc.cur_priority = _old_pri + 100000
                    TASK_AT[fu]()
                    tc.cur_priority = _old_pri

            flush_unit(pending.pop(0))
            gates(1, 1)

            # atom_e reduce -> seg matmul -> out
            ae8 = prepool.tile([128, 8], F32, tag="ae8")
            nc.gpsimd.tensor_tensor(ae8[:], ae_parts[:, 0:8], ae_parts[:, 8:16], op=ALU.add)
            ae4f = prepool.tile([128, 4], F32, tag="ae4f")
            nc.gpsimd.tensor_tensor(ae4f[:], ae8[:, 0:4], ae8[:, 4:8], op=ALU.add)
            ae4b = prepool.tile([128, 4], BF16, tag="ae4b")
            nc.gpsimd.tensor_tensor(ae4b[:], ae4f[:], ae_parts[:, 16:20], op=ALU.add)
            ps_seg = pspre.tile([1, NG], F32, tag="pre")
            for q in range(4):
                nc.tensor.matmul(ps_seg[:], ae4b[:, q:q + 1], Stsb[:, q * NG:(q + 1) * NG],
                                 start=(q == 0), stop=(q == 3))
            nc.vector.tensor_scalar(res[:, 0:NG], ps_seg[:], 1.0, 0.0,
                                    op0=ALU.mult, op1=ALU.add)
            nc.sync.dma_start(d_res[:], res[:])

    _legalize_waits(nc)
    nc._tile_ctx = tc_ref
    return nc


def kernel(**inputs) -> np.ndarray:
    f = lambda a: np.ascontiguousarray(np.asarray(a), dtype=np.float32)
    bf = lambda a: np.ascontiguousarray(np.asarray(a, dtype=np.float32)).astype(ml_dtypes.bfloat16)
    tf = f(inputs["token_features"])
    la = f(inputs["lig_atom"])
    lgr = f(inputs["lig_graph"])
    m0 = f(inputs["ms_feat_0"])
    m1 = f(inputs["ms_feat_1"])
    lb = np.asarray(inputs["ligand_batch"])
    S = (lb[:, None] == np.arange(NG)[None, :]).astype(np.float32)

    # ---- weight prep (host-side layout/scale transforms only) ----
    wint_bf = bf(inputs["W_int"])                       # [128,128]
    wpe = f(inputs["W_pe"]); wpg = f(inputs["W_pg"])    # [128,1]
    wpeg = np.concatenate([wpe, wpg], axis=1)           # [128,2]
    u_pe = wint_bf.astype(np.float64) @ wpe.astype(np.float64)
    u_pg = wint_bf.astype(np.float64) @ wpg.astype(np.float64)
    upeg = 0.01 * np.concatenate([u_pe, u_pg], axis=1)  # [128,2]

    wcat = f(inputs["W_cat"]).copy()                    # [384,128]
    wgate = f(inputs["W_gate"]).copy()
    wcat[2 * HID:] /= float(NT)
    wgate[2 * HID:] /= float(NT)

    WB = np.zeros((128, NWB), dtype=np.float32)
    WB[:, OFF_WINT:OFF_WINT + 128] = wint_bf.astype(np.float32)
    WB[:, OFF_WTOK:OFF_WTOK + 256] = f(inputs["W_token"]).reshape(2, 128, HID).transpose(1, 0, 2).reshape(128, 256)
    WB[:, OFF_WPK:OFF_WPK + 256] = f(inputs["W_pocket"]).reshape(2, 128, HID).transpose(1, 0, 2).reshape(128, 256)
    WB[:, OFF_WCAT:OFF_WCAT + 384] = wcat.reshape(3, 128, HID).transpose(1, 0, 2).reshape(128, 384)
    WB[:, OFF_WGATE:OFF_WGATE + 384] = wgate.reshape(3, 128, HID).transpose(1, 0, 2).reshape(128, 384)
    WB[:, OFF_WB1:OFF_WB1 + 256] = f(inputs["W_bias1"]).reshape(2, 128, HID).transpose(1, 0, 2).reshape(128, 256)
    WB[:, OFF_WB2:OFF_WB2 + 1] = f(inputs["W_bias2"])
    WB[:, OFF_WPEG:OFF_WPEG + 2] = wpeg
    WB[:, OFF_UPEG:OFF_UPEG + 2] = upeg
    WB_bf = WB.astype(ml_dtypes.bfloat16)

    # conv weights as [c, off*128 + o], scaled by 1/num_output_positions
    Wc0 = f(inputs["Wc0"])  # [128,64,3,3,3] applied to ms_feat_1
    Wc1 = f(inputs["Wc1"])  # [128,32,3,3,3] applied to ms_feat_0
    W0T = np.ascontiguousarray(Wc0.reshape(128, 64, 27).transpose(1, 2, 0)).reshape(64, 27 * 128) / 216.0
    W32 = np.ascontiguousarray(Wc1.reshape(128, 32, 27).transpose(1, 2, 0)).reshape(32, 27 * 128) / 2744.0

    W64 = np.zeros((64, 256), dtype=np.float32)
    W64[:, 0:128] = f(inputs["W_atom"])
    W64[:, 128:256] = f(inputs["W_graph"])

    col = lambda a: f(a).reshape(128, 1)
    BI = np.zeros((128, NBI), dtype=np.float32)
    BI[:, BI_TOK] = f(inputs["b_token"])
    BI[:, BI_ATOM] = f(inputs["b_atom"])
    BI[:, BI_INT] = f(inputs["b_int"])
    BI[:, BI_PK] = f(inputs["b_pocket"])
    BI[:, BI_CAT] = f(inputs["b_cat"])
    BI[:, BI_GH] = 0.5 * f(inputs["b_gate"])
    BI[:, BI_GR] = f(inputs["b_graph"])
    BI[:, BI_B1] = f(inputs["b_bias1"])
    BI[:, BI_C0] = f(inputs["bc0"])
    BI[:, BI_C1] = f(inputs["bc1"])
    BI[:, BI_WPEG:BI_WPEG + 2] = wpeg

    # window-membership masks: M[pos, off] = 1 iff pos-off in valid out range
    def win_mask(D, O):
        g = np.arange(D)
        z, y, x = np.meshgrid(g, g, g, indexing="ij")
        pos = np.stack([z.ravel(), y.ravel(), x.ravel()], 1)  # [D^3, 3]
        d = np.arange(3)
        dz, dy, dx = np.meshgrid(d, d, d, indexing="ij")
        off = np.stack([dz.ravel(), dy.ravel(), dx.ravel()], 1)  # [27, 3]
        r = pos[:, None, :] - off[None, :, :]
        return np.all((r >= 0) & (r < O), axis=2).astype(np.float32)  # [D^3, 27]

    M1 = win_mask(16, 14).reshape(32, 128, 27)
    M0 = win_mask(8, 6).reshape(4, 128, 27)

    bpe = float(np.asarray(inputs["b_pe"]).reshape(-1)[0])
    bpg = float(np.asarray(inputs["b_pg"]).reshape(-1)[0])
    bb2 = float(np.asarray(inputs["b_bias2"]).reshape(-1)[0])

    WE = np.zeros((128, 384), dtype=np.float32)
    WE[:, 0:128] = WB[:, OFF_WINT:OFF_WINT + 128]
    WE[:, 128:384] = WB[:, OFF_WTOK:OFF_WTOK + 256]
    shared = {
        "WB": WB_bf, "BI": BI,
        "WE": WE.astype(ml_dtypes.bfloat16),
        "W64": W64.astype(ml_dtypes.bfloat16),
        "W0T": W0T.astype(ml_dtypes.bfloat16),
        "W32": W32.astype(ml_dtypes.bfloat16),
        "M1m": M1.astype(ml_dtypes.bfloat16),
        "M0m": M0.astype(ml_dtypes.bfloat16),
    }

    in_maps = []
    for c in range(NCORES):
        n, h = c // 2, c % 2
        m = dict(shared)
        m["tfX"] = bf(tf[n].T.reshape(2, 128, 128))
        m["laT"] = bf(la[n, 512 * h:512 * (h + 1)].T)
        m["lgT"] = bf(lgr[n].T)
        m["m0T"] = bf(m0[n].reshape(32, 4096).T.reshape(32, 128, 32))
        m["m1T"] = bf(m1[n].reshape(64, 512).T.reshape(4, 128, 64))
        m["Sh"] = bf(S[512 * h:512 * (h + 1)].reshape(4, 128, NG))
        in_maps.append(m)

    bint_zero = bool(np.all(np.asarray(inputs["b_int"]) == 0.0))
    nc = build_program(bpe, bpg, bb2, bint_zero)
    r = run_bass_kernel_spmd(nc, in_maps, core_ids=list(range(NCORES)),
                             trace=TRACE, **(TRACE_KW if TRACE else {}))
    global LAST
    LAST = r
    res = r.results

    out = np.zeros((NI, NG), dtype=np.float32)
    for n in range(NI):
        out[n] = (res[2 * n]["res_out"][0, 0:NG] + res[2 * n + 1]["res_out"][0, 0:NG]
                  + res[2 * n]["res_out"][0, NG:2 * NG])
    return out


# revision 30
# speedup vs baseline: 2.1700x; 1.0142x over previous
import sys
import numpy as np
import ml_dtypes

sys.path.insert(0, "/opt/trn_rl_repo")

import concourse.bass as bass
import concourse.tile as tile
from concourse import mybir
from concourse.bass_utils import run_bass_kernel_spmd

F32 = mybir.dt.float32
BF16 = mybir.dt.bfloat16
AF = mybir.ActivationFunctionType
ALU = mybir.AluOpType

HID = 128
NT = 128       # tokens per image
NAH = 512      # atoms per core (half of 1024)
NG = 64        # ligand graphs
NI = 4         # images
NCORES = 8

# WB (128-partition weight concat, bf16) column offsets
OFF_WINT = 0
OFF_WTOK = 128
OFF_WPK = 384
OFF_WCAT = 640
OFF_WGATE = 1024
OFF_WB1 = 1408
OFF_WB2 = 1664
OFF_WPEG = 1665
OFF_UPEG = 1667
NWB = 1669

# BI (f32 bias concat) columns
BI_TOK, BI_ATOM, BI_INT, BI_PK, BI_CAT, BI_GH, BI_GR, BI_B1, BI_C0, BI_C1 = range(10)
BI_WPEG = 10   # cols 10:12 = [W_pe, W_pg] f32
NBI = 12

# lrelu unit assignment: 'A' = ACT Prelu, 'B' = DVE relu99 + linear-fold
N_A_UNITS = 32

TRACE = False
TRACE_KW = {}
LAST = None


_COMPUTE_INSTS = (
    "InstActivation", "InstTensorCopy", "InstTensorScalar", "InstTensorScalarPtr",
    "InstTensorTensor", "InstTensorTensorReduce", "InstTensorReduce", "InstMemSet",
    "InstMatmult", "InstScalarTensorTensor", "InstTensorTensorScan", "InstLdweights",
    "InstDMACopy", "InstDMATransposeAnt", "InstTriggeredCopy", "InstDrain",
    "InstEventSemaphoreOp", "InstSemaphoreOp", "InstCopy", "InstIota", "InstSelect",
)


def _legalize_waits(nc):
    # walrus in this toolchain accepts at most ONE sync wait on TPB compute
    # instructions; hoist extras into same-engine NoOps placed just before.
    k = 0
    for f in nc.m.functions:
        for blk in f.blocks:
            insts = blk.instructions
            out = []
            for ins in insts:
                si = getattr(ins, "sync_info", None)
                if (si is not None and len(si.on_wait) > 1
                        and type(ins).__name__ in _COMPUTE_INSTS):
                    waits = list(si.on_wait)
                    for w in waits[:-1]:
                        nop = mybir.InstNoOp(
                            name=f"WNOP-{k}", engine=ins.engine,
                            sync_info=mybir.SyncInfo(on_wait=[w], on_update=[]))
                        k += 1
                        out.append(nop)
                    ins.sync_info = mybir.SyncInfo(on_wait=[waits[-1]],
                                                   on_update=list(si.on_update))
                out.append(ins)
            blk.instructions = out
    return k


def _register_const(nc, val, dtype=F32):
    if (dtype, float(val)) in nc.const_aps.aps:
        return
    t = nc.alloc_sbuf_tensor(f"uconst-{dtype.name}-{val}", [128, 1], dtype)
    nc.gpsimd.memset(t.ap(), float(val))
    nc.const_aps.aps[(dtype, float(val))] = t.ap()


def _unit_engines():
    # interleave N_A_UNITS 'A' units among 64 as evenly as possible
    eng = []
    for u in range(64):
        if (u + 1) * N_A_UNITS // 64 > u * N_A_UNITS // 64:
            eng.append('A')
        else:
            eng.append('B')
    return eng


def build_program(bpe: float, bpg: float, bb2: float, bint_zero: bool = True,
                  sim_trace: bool = False) -> bass.Bass:
    nc = bass.Bass()
    _register_const(nc, 0.5 * bpg)
    _register_const(nc, bb2)
    nc.all_engine_barrier()

    # ---- DRAM inputs (per-core views; same names across SPMD cores) ----
    d_WB = nc.dram_tensor("WB", [128, NWB], BF16, kind="ExternalInput")
    d_BI = nc.dram_tensor("BI", [128, NBI], F32, kind="ExternalInput")
    d_EW = nc.dram_tensor("EW", [128, 640], BF16, kind="ExternalInput")
    d_LA6 = nc.dram_tensor("LA6", [64, 768], BF16, kind="ExternalInput")
    d_m0T = nc.dram_tensor("m0T", [32, 128, 32], BF16, kind="ExternalInput")
    d_M1 = nc.dram_tensor("M1m", [32, 128, 27], BF16, kind="ExternalInput")
    d_m1T = nc.dram_tensor("m1T", [4, 128, 64], BF16, kind="ExternalInput")
    d_M0 = nc.dram_tensor("M0m", [4, 128, 27], BF16, kind="ExternalInput")
    d_W0T = nc.dram_tensor("W0T", [64, 27 * 128], BF16, kind="ExternalInput")
    d_W32 = nc.dram_tensor("W32", [32, 27 * 128], BF16, kind="ExternalInput")
    d_lgT = nc.dram_tensor("lgT", [64, NG], BF16, kind="ExternalInput")
    d_Sh = nc.dram_tensor("Sh", [4, 128, NG], BF16, kind="ExternalInput")

    d_res = nc.dram_tensor("res_out", [1, 128], F32, kind="ExternalOutput")

    ENG = _unit_engines()
    if not bint_zero:
        ENG[:] = ['A'] * 64

    tc_ref = tile.TileContext(nc, trace_sim=sim_trace)
    with tc_ref as tc:
        with (
            tc.tile_pool(name="const", bufs=1) as cpool,
            tc.tile_pool(name="pre", bufs=1) as prepool,
            tc.tile_pool(name="x", bufs=12) as xpool,
            tc.tile_pool(name="u", bufs=8) as upool,
            tc.tile_pool(name="h", bufs=8) as hpool,
            tc.tile_pool(name="g", bufs=3) as gpool,
            tc.tile_pool(name="j", bufs=4) as jpool,
            tc.tile_pool(name="ps_y", bufs=3, space="PSUM") as psy,
            tc.tile_pool(name="ps_z", bufs=1, space="PSUM") as psz,
            tc.tile_pool(name="ps_p", bufs=1, space="PSUM") as pspre,
        ):
            # ---------- engine warmups (hide ACT table load + start PE pstate clock)
            warm = cpool.tile([128, 1], F32, tag="warm")
            nc.gpsimd.memset(warm[:], 0.0)
            warma = cpool.tile([128, 1], F32, tag="warma")
            nc.scalar.activation(warma[:], warm[:], AF.Silu)
            ps_warm = pspre.tile([1, 1], F32, tag="pre")
            nc.tensor.matmul(ps_warm[:], warm[:], warm[:], start=True, stop=True)
            warmb = cpool.tile([1, 1], F32, tag="warmb")
            nc.scalar.activation(warmb[:], ps_warm[:], AF.Copy)

            # ---------- input DMAs (order = DMA device service priority) ----
            EWsb = cpool.tile([128, 640], BF16, tag="EW")
            nc.sync.dma_start(EWsb[:], d_EW[:])
            BIsb = cpool.tile([128, NBI], F32, tag="BI")
            nc.sync.dma_start(BIsb[:], d_BI[:])
            LA6sb = cpool.tile([64, 768], BF16, tag="LA6")
            nc.sync.dma_start(LA6sb[:], d_LA6[:])
            tfx = EWsb[:, 0:256]
            WEsb = EWsb[:, 256:640]
            la = LA6sb[:, 256:768]
            W64sb = LA6sb[:, 0:256]
            WBsb = cpool.tile([128, NWB], BF16, tag="WB")
            nc.sync.dma_start(WBsb[:], d_WB[:])
            m0sb = cpool.tile([128, 1024], BF16, tag="m0")
            nc.sync.dma_start(m0sb[:, :].rearrange("p (u c) -> p u c", u=32),
                              d_m0T[:, :, :].rearrange("u p c -> p u c"))
            M1sb = cpool.tile([128, 864], BF16, tag="M1")
            nc.sync.dma_start(M1sb[:, :].rearrange("p (u o) -> p u o", u=32),
                              d_M1[:, :, :].rearrange("u p o -> p u o"))
            m1sb = cpool.tile([128, 256], BF16, tag="m1")
            nc.sync.dma_start(m1sb[:, :].rearrange("p (u c) -> p u c", u=4),
                              d_m1T[:, :, :].rearrange("u p c -> p u c"))
            M0sb = cpool.tile([128, 108], BF16, tag="M0")
            nc.sync.dma_start(M0sb[:, :].rearrange("p (u o) -> p u o", u=4),
                              d_M0[:, :, :].rearrange("u p o -> p u o"))
            W0Tsb = cpool.tile([64, 27 * 128], BF16, tag="W0T")
            nc.sync.dma_start(W0Tsb[:], d_W0T[:])
            W32sb = cpool.tile([32, 27 * 128], BF16, tag="W32")
            nc.sync.dma_start(W32sb[:], d_W32[:])
            lg = cpool.tile([64, NG], BF16, tag="lg")
            nc.sync.dma_start(lg[:], d_lgT[:])
            Stsb = cpool.tile([128, 4 * NG], BF16, tag="St")
            nc.sync.dma_start(Stsb[:, :].rearrange("p (q g) -> p q g", q=4),
                              d_Sh[:, :, :].rearrange("q p g -> p q g"))
            F32R = mybir.dt.float32r

            bias = lambda i: BIsb[:, i:i + 1]

            # ---------- preamble: tok / atoms (needed before main loop) -----
            tfr = prepool.tile([128, 256], BF16, tag="tfr")
            nc.scalar.activation(tfr[:], tfx, AF.Silu)
            ps_tok = pspre.tile([128, 128], F32, tag="pre")
            nc.tensor.matmul(ps_tok[:], WEsb[128:256] if False else EWsb[:, 384:512],
                             tfr[:, 0:128], start=True, stop=False)
            nc.tensor.matmul(ps_tok[:], EWsb[:, 512:640],
                             tfr[:, 128:256], start=False, stop=True)
            tokT = cpool.tile([128, NT], F32, tag="tokT")
            nc.scalar.activation(tokT[:], ps_tok[:], AF.Identity, bias=bias(BI_TOK))

            ps_at = pspre.tile([128, NAH], F32, tag="pre")
            nc.tensor.matmul(ps_at[:], W64sb[:, 0:128], la, start=True, stop=True)
            atomsT = cpool.tile([128, NAH], BF16, tag="atomsT")
            nc.vector.tensor_scalar(atomsT[:], ps_at[:], bias(BI_ATOM), 0.0,
                                    op0=ALU.add, op1=ALU.add)

            # ---------- deferred preamble tasks (interleaved into loop) ----
            state = {}

            def task_silu1():
                s0 = cpool.tile([128, 1024], BF16, tag="s0")
                nc.scalar.activation(s0[:], m0sb[:], AF.Silu)
                state["s0"] = s0

            def task_S1():
                S1 = pspre.tile([32, 27], F32, tag="pre")
                for u in range(32):
                    nc.tensor.matmul(S1[:], state["s0"][:, 32 * u:32 * u + 32],
                                     M1sb[:, 27 * u:27 * u + 27],
                                     start=(u == 0), stop=(u == 31))
                S1b = prepool.tile([32, 27], BF16, tag="S1b")
                nc.scalar.activation(S1b[:], S1[:], AF.Copy)
                state["S1b"] = S1b

            def task_p1():
                pp = pspre.tile([128, 1], F32, tag="pre")
                for o in range(27):
                    nc.tensor.matmul(pp[:], W32sb[:, 128 * o:128 * o + 128],
                                     state["S1b"][:, o:o + 1],
                                     start=(o == 0), stop=(o == 26))
                sp1 = prepool.tile([128, 1], BF16, tag="sp1")
                nc.scalar.activation(sp1[:], pp[:], AF.Silu, bias=bias(BI_C1))
                state["sp1"] = sp1

            def task_silu0():
                s1 = prepool.tile([128, 256], BF16, tag="s1")
                nc.scalar.activation(s1[:], m1sb[:], AF.Silu)
                state["s1"] = s1

            def task_S0():
                S0 = pspre.tile([64, 27], F32, tag="pre")
                for u in range(4):
                    nc.tensor.matmul(S0[:], state["s1"][:, 64 * u:64 * u + 64],
                                     M0sb[:, 27 * u:27 * u + 27],
                                     start=(u == 0), stop=(u == 3))
                S0b = prepool.tile([64, 27], BF16, tag="S0b")
                nc.scalar.activation(S0b[:], S0[:], AF.Copy)
                state["S0b"] = S0b

            def task_p0():
                pp = pspre.tile([128, 1], F32, tag="pre")
                for o in range(27):
                    nc.tensor.matmul(pp[:], W0Tsb[:, 128 * o:128 * o + 128],
                                     state["S0b"][:, o:o + 1],
                                     start=(o == 0), stop=(o == 26))
                sp0 = prepool.tile([128, 1], BF16, tag="sp0")
                nc.scalar.activation(sp0[:], pp[:], AF.Silu, bias=bias(BI_C0))
                state["sp0"] = sp0

            def task_pocket():
                ps_pk = pspre.tile([128, 1], F32, tag="pre")
                nc.tensor.matmul(ps_pk[:], WBsb[:, OFF_WPK:OFF_WPK + 128],
                                 state["sp0"][:], start=True, stop=False)
                nc.tensor.matmul(ps_pk[:], WBsb[:, OFF_WPK + 128:OFF_WPK + 256],
                                 state["sp1"][:], start=False, stop=True)
                pocket = prepool.tile([128, 1], BF16, tag="pocket")
                nc.scalar.activation(pocket[:], ps_pk[:], AF.Identity, bias=bias(BI_PK))
                state["pocket"] = pocket

            def task_pf():
                junkt = jpool.tile([128, NT], BF16, tag="junk")
                tok_sum = prepool.tile([128, 1], F32, tag="toksum")
                nc.scalar.activation(junkt[:], tokT[:], AF.Identity,
                                     accum_out=tok_sum[:])
                tok_sum_b = prepool.tile([128, 1], BF16, tag="toksumb")
                nc.scalar.activation(tok_sum_b[:], tok_sum[:], AF.Copy)
                ps_pf = pspre.tile([128, 2], F32, tag="pre")
                chunks = [state["pocket"], tok_sum_b, tok_sum_b]
                for q in range(3):
                    nc.tensor.matmul(ps_pf[:, 0:1],
                                     WBsb[:, OFF_WCAT + 128 * q:OFF_WCAT + 128 * (q + 1)],
                                     chunks[q][:], start=(q == 0), stop=(q == 2))
                for q in range(3):
                    nc.tensor.matmul(ps_pf[:, 1:2],
                                     WBsb[:, OFF_WGATE + 128 * q:OFF_WGATE + 128 * (q + 1)],
                                     chunks[q][:], start=(q == 0), stop=(q == 2))
                # sigmoid(z + bg) = 0.5 + 0.5*tanh(0.5z + 0.5bg)
                gt = prepool.tile([128, 1], F32, tag="gt")
                nc.scalar.activation(gt[:], ps_pf[:, 1:2], AF.Tanh,
                                     bias=bias(BI_GH), scale=0.5)
                pf_sig = prepool.tile([128, 1], F32, tag="pfsig")
                nc.gpsimd.tensor_scalar(pf_sig[:], gt[:], 0.5, 0.5, op0=ALU.mult, op1=ALU.add)
                pf_lin = prepool.tile([128, 1], F32, tag="pflin")
                nc.scalar.activation(pf_lin[:], ps_pf[:, 0:1], AF.Identity, bias=bias(BI_CAT))
                pf = prepool.tile([128, 1], BF16, tag="pf")
                nc.gpsimd.tensor_tensor(pf[:], pf_lin[:], pf_sig[:], op=ALU.mult)
                state["pf"] = pf

            def task_gf():
                ps_gf = pspre.tile([128, NG], F32, tag="pre")
                nc.tensor.matmul(ps_gf[:], W64sb[:, 128:256], lg[:], start=True, stop=True)
                gfT = prepool.tile([128, NG], BF16, tag="gfT")
                nc.scalar.activation(gfT[:], ps_gf[:], AF.Identity, bias=bias(BI_GR))
                state["gfT"] = gfT

            def task_bias1():
                ps_u = pspre.tile([128, 1], F32, tag="pre")
                nc.tensor.matmul(ps_u[:], WBsb[:, OFF_WB1:OFF_WB1 + 128],
                                 state["pf"][:], start=True, stop=True)
                ub = prepool.tile([128, 1], F32, tag="ub")
                nc.scalar.activation(ub[:], ps_u[:], AF.Identity, bias=bias(BI_B1))
                ps_hb = pspre.tile([128, NG], F32, tag="pre")
                nc.tensor.matmul(ps_hb[:], WBsb[:, OFF_WB1 + 128:OFF_WB1 + 256],
                                 state["gfT"][:], start=True, stop=True)
                hb = prepool.tile([128, NG], BF16, tag="hb")
                nc.scalar.activation(hb[:], ps_hb[:], AF.Prelu, bias=ub[:], alpha=0.01)
                state["hb"] = hb

            def task_bias2():
                ps_b2 = pspre.tile([1, NG], F32, tag="pre")
                nc.tensor.matmul(ps_b2[:], WBsb[:, OFF_WB2:OFF_WB2 + 1],
                                 state["hb"][:], start=True, stop=True)
                nc.scalar.activation(res[:, NG:2 * NG], ps_b2[:], AF.Identity, bias=bb2)

            pre_tasks = [task_silu1, task_S1, task_p1, task_silu0, task_S0,
                         task_p0, task_pocket, task_pf, task_gf, task_bias1,
                         task_bias2]
            TASK_AT = {12 + 4 * i: t for i, t in enumerate(pre_tasks)}

            res = cpool.tile([1, 128], F32, tag="res")

            # ---------- main loop ----------
            # 64 units u of 2 tokens; y2[o, 512v + a] for token j = 2u+v.
            # zq8 (per 64-token block) col layout: 8*(j%64) + 2*a_chunk + {pe,pg}
            wpegr = cpool.tile([128, 2], F32R, tag="wpegr")
            nc.scalar.activation(wpegr[:], BIsb[:, BI_WPEG:BI_WPEG + 2], AF.Copy)
            wpeg_ap = wpegr[:]
            upeg_ap = WBsb[:, OFF_UPEG:OFF_UPEG + 2]
            wint_ap = EWsb[:, 256:384]
            zq_tiles = [None, None]
            ae_parts = cpool.tile([128, 20], F32, tag="aeparts")
            pending = []

            def emit_unit(u):
                y2 = psy.tile([128, 1024], F32, tag="y")
                ujs = []
                for v in range(2):
                    j = 2 * u + v
                    Wj = xpool.tile([128, 128], BF16, tag="x")
                    nc.gpsimd.tensor_scalar_mul(Wj[:], wint_ap, tokT[:, j:j + 1])
                    nc.tensor.matmul(y2[:, 512 * v:512 * (v + 1)], Wj[:], atomsT[:],
                                     start=True, stop=True)
                    if ENG[u] == 'B':
                        uj = upool.tile([128, 2], BF16, tag="u")
                        nc.gpsimd.tensor_scalar_mul(uj[:], upeg_ap, tokT[:, j:j + 1])
                        ujs.append(uj)
                return (u, y2, ujs)

            def flush_unit(ent):
                u, y2, ujs = ent
                h = hpool.tile([128, 1024], F32R, tag="h")
                if ENG[u] == 'A':
                    nc.scalar.activation(h[:], y2[:], AF.Prelu, bias=bias(BI_INT),
                                         alpha=0.01)
                else:
                    # h = 0.99*relu(y); the 0.01*y linear part of lrelu is
                    # folded into the zq accumulation via upeg below
                    nc.vector.tensor_scalar(h[:], y2[:], 0.0, 0.99,
                                            op0=ALU.max, op1=ALU.mult)
                for v in range(2):
                    j = 2 * u + v
                    b, jj = j // 64, j % 64
                    if zq_tiles[b] is None:
                        zq_tiles[b] = psz.tile([128, 512], F32, tag="z", name=f"zq{b}")
                    zq = zq_tiles[b]
                    for a in range(4):
                        cols = zq[:, 8 * jj + 2 * a:8 * jj + 2 * a + 2]
                        if ENG[u] == 'A':
                            nc.tensor.matmul(cols, h[:, 512 * v + 128 * a:512 * v + 128 * (a + 1)],
                                             wpeg_ap, start=True, stop=True)
                        else:
                            nc.tensor.matmul(cols, h[:, 512 * v + 128 * a:512 * v + 128 * (a + 1)],
                                             wpeg_ap, start=True, stop=False)
                            nc.tensor.matmul(cols, atomsT[:, 128 * a:128 * (a + 1)],
                                             ujs[v][:], start=False, stop=True)

            def gates(b, c0, c1, slot):
                # process zq cols [c0:c1] -> ae_parts cols 4*slot : 4*slot+4
                zq = zq_tiles[b]
                n2 = (c1 - c0) // 2
                s = gpool.tile([128, 256], F32, tag="s")
                nc.scalar.activation(s[:, 0:n2], zq[:, c0 + 1:c1:2], AF.Tanh,
                                     bias=0.5 * bpg, scale=0.5)
                w = gpool.tile([128, 256], F32, tag="w")
                nc.gpsimd.tensor_scalar(w[:, 0:n2], s[:, 0:n2], 0.5, 0.5,
                                        op0=ALU.mult, op1=ALU.add)
                t = gpool.tile([128, 256], F32, tag="t")
                nc.vector.scalar_tensor_tensor(t[:, 0:n2], zq[:, c0:c1:2], bpe, w[:, 0:n2],
                                               op0=ALU.add, op1=ALU.mult)
                for a in range(4):
                    junka = jpool.tile([128, 64], F32, tag="junka")
                    nc.vector.tensor_scalar(junka[:, 0:n2 // 4], t[:, a:n2:4], 1.0, 0.0,
                                            op0=ALU.mult, op1=ALU.add,
                                            accum_out=ae_parts[:, 4 * slot + a:
                                                              4 * slot + a + 1])

            for u in range(64):
                pending.append(emit_unit(u))
                if len(pending) > 1:
                    flush_unit(pending.pop(0))
                fu = u - 1  # unit just flushed
                if fu == 15:
                    gates(0, 0)
                elif fu == 31:
                    gates(0, 1)
                elif fu == 47:
                    gates(1, 0)
                if fu in TASK_AT:
                    _old_pri = tc.cur_priority
                    tc.cur_priority = _old_pri + 100000
                    TASK_AT[fu]()
                    tc.cur_priority = _old_pri

            flush_unit(pending.pop(0))
            gates(1, 1)

            # atom_e reduce -> seg matmul -> out
            ae8 = prepool.tile([128, 8], F32, tag="ae8")
            nc.gpsimd.tensor_tensor(ae8[:], ae_parts[:, 0:8], ae_parts[:, 8:16], op=ALU.add)
            ae4f = prepool.tile([128, 4], F32, tag="ae4f")
            nc.gpsimd.tensor_tensor(ae4f[:], ae8[:, 0:4], ae8[:, 4:8], op=ALU.add)
            ae4b = prepool.tile([128, 4], BF16, tag="ae4b")
            nc.gpsimd.tensor_tensor(ae4b[:], ae4f[:], ae_parts[:, 16:20], op=ALU.add)
            ps_seg = pspre.tile([1, NG], F32, tag="pre")
            for q in range(4):
                nc.tensor.matmul(ps_seg[:], ae4b[:, q:q + 1], Stsb[:, q * NG:(q + 1) * NG],
                                 start=(q == 0), stop=(q == 3))
            nc.vector.tensor_scalar(res[:, 0:NG], ps_seg[:], 1.0, 0.0,
                                    op0=ALU.mult, op1=ALU.add)
            nc.sync.dma_start(d_res[:], res[:])

    _legalize_waits(nc)
    nc._tile_ctx = tc_ref
    return nc


def kernel(**inputs) -> np.ndarray:
    f = lambda a: np.ascontiguousarray(np.asarray(a), dtype=np.float32)
    bf = lambda a: np.ascontiguousarray(np.asarray(a, dtype=np.float32)).astype(ml_dtypes.bfloat16)
    tf = f(inputs["token_features"])
    la = f(inputs["lig_atom"])
    lgr = f(inputs["lig_graph"])
    m0 = f(inputs["ms_feat_0"])
    m1 = f(inputs["ms_feat_1"])
    lb = np.asarray(inputs["ligand_batch"])
    S = (lb[:, None] == np.arange(NG)[None, :]).astype(np.float32)

    # ---- weight prep (host-side layout/scale transforms only) ----
    wint_bf = bf(inputs["W_int"])                       # [128,128]
    wpe = f(inputs["W_pe"]); wpg = f(inputs["W_pg"])    # [128,1]
    wpeg = np.concatenate([wpe, wpg], axis=1)           # [128,2]
    u_pe = wint_bf.astype(np.float64) @ wpe.astype(np.float64)
    u_pg = wint_bf.astype(np.float64) @ wpg.astype(np.float64)
    upeg = 0.01 * np.concatenate([u_pe, u_pg], axis=1)  # [128,2]

    wcat = f(inputs["W_cat"]).copy()                    # [384,128]
    wgate = f(inputs["W_gate"]).copy()
    wcat[2 * HID:] /= float(NT)
    wgate[2 * HID:] /= float(NT)

    WB = np.zeros((128, NWB), dtype=np.float32)
    WB[:, OFF_WINT:OFF_WINT + 128] = wint_bf.astype(np.float32)
    WB[:, OFF_WTOK:OFF_WTOK + 256] = f(inputs["W_token"]).reshape(2, 128, HID).transpose(1, 0, 2).reshape(128, 256)
    WB[:, OFF_WPK:OFF_WPK + 256] = f(inputs["W_pocket"]).reshape(2, 128, HID).transpose(1, 0, 2).reshape(128, 256)
    WB[:, OFF_WCAT:OFF_WCAT + 384] = wcat.reshape(3, 128, HID).transpose(1, 0, 2).reshape(128, 384)
    WB[:, OFF_WGATE:OFF_WGATE + 384] = wgate.reshape(3, 128, HID).transpose(1, 0, 2).reshape(128, 384)
    WB[:, OFF_WB1:OFF_WB1 + 256] = f(inputs["W_bias1"]).reshape(2, 128, HID).transpose(1, 0, 2).reshape(128, 256)
    WB[:, OFF_WB2:OFF_WB2 + 1] = f(inputs["W_bias2"])
    WB[:, OFF_WPEG:OFF_WPEG + 2] = wpeg
    WB[:, OFF_UPEG:OFF_UPEG + 2] = upeg
    WB_bf = WB.astype(ml_dtypes.bfloat16)

    # conv weights as [c, off*128 + o], scaled by 1/num_output_positions
    Wc0 = f(inputs["Wc0"])  # [128,64,3,3,3] applied to ms_feat_1
    Wc1 = f(inputs["Wc1"])  # [128,32,3,3,3] applied to ms_feat_0
    W0T = np.ascontiguousarray(Wc0.reshape(128, 64, 27).transpose(1, 2, 0)).reshape(64, 27 * 128) / 216.0
    W32 = np.ascontiguousarray(Wc1.reshape(128, 32, 27).transpose(1, 2, 0)).reshape(32, 27 * 128) / 2744.0

    W64 = np.zeros((64, 256), dtype=np.float32)
    W64[:, 0:128] = f(inputs["W_atom"])
    W64[:, 128:256] = f(inputs["W_graph"])

    col = lambda a: f(a).reshape(128, 1)
    BI = np.zeros((128, NBI), dtype=np.float32)
    BI[:, BI_TOK] = f(inputs["b_token"])
    BI[:, BI_ATOM] = f(inputs["b_atom"])
    BI[:, BI_INT] = f(inputs["b_int"])
    BI[:, BI_PK] = f(inputs["b_pocket"])
    BI[:, BI_CAT] = f(inputs["b_cat"])
    BI[:, BI_GH] = 0.5 * f(inputs["b_gate"])
    BI[:, BI_GR] = f(inputs["b_graph"])
    BI[:, BI_B1] = f(inputs["b_bias1"])
    BI[:, BI_C0] = f(inputs["bc0"])
    BI[:, BI_C1] = f(inputs["bc1"])
    BI[:, BI_WPEG:BI_WPEG + 2] = wpeg

    # window-membership masks: M[pos, off] = 1 iff pos-off in valid out range
    def win_mask(D, O):
        g = np.arange(D)
        z, y, x = np.meshgrid(g, g, g, indexing="ij")
        pos = np.stack([z.ravel(), y.ravel(), x.ravel()], 1)  # [D^3, 3]
        d = np.arange(3)
        dz, dy, dx = np.meshgrid(d, d, d, indexing="ij")
        off = np.stack([dz.ravel(), dy.ravel(), dx.ravel()], 1)  # [27, 3]
        r = pos[:, None, :] - off[None, :, :]
        return np.all((r >= 0) & (r < O), axis=2).astype(np.float32)  # [D^3, 27]

    M1 = win_mask(16, 14).reshape(32, 128, 27)
    M0 = win_mask(8, 6).reshape(4, 128, 27)

    bpe = float(np.asarray(inputs["b_pe"]).reshape(-1)[0])
    bpg = float(np.asarray(inputs["b_pg"]).reshape(-1)[0])
    bb2 = float(np.asarray(inputs["b_bias2"]).reshape(-1)[0])

    LA6w = np.zeros((64, 256), dtype=np.float32)
    LA6w[:, 0:128] = f(inputs["W_atom"])
    LA6w[:, 128:256] = f(inputs["W_graph"])
    shared = {
        "WB": WB_bf, "BI": BI,
        "W0T": W0T.astype(ml_dtypes.bfloat16),
        "W32": W32.astype(ml_dtypes.bfloat16),
        "M1m": M1.astype(ml_dtypes.bfloat16),
        "M0m": M0.astype(ml_dtypes.bfloat16),
    }

    in_maps = []
    for c in range(NCORES):
        n, h = c // 2, c % 2
        m = dict(shared)
        EW = np.zeros((128, 640), dtype=np.float32)
        EW[:, 0:256] = tf[n].T.reshape(2, 128, 128).transpose(1, 0, 2).reshape(128, 256)
        EW[:, 256:384] = WB[:, OFF_WINT:OFF_WINT + 128]
        EW[:, 384:640] = WB[:, OFF_WTOK:OFF_WTOK + 256]
        m["EW"] = EW.astype(ml_dtypes.bfloat16)
        LA6 = np.zeros((64, 768), dtype=np.float32)
        LA6[:, 0:256] = LA6w
        LA6[:, 256:768] = la[n, 512 * h:512 * (h + 1)].T
        m["LA6"] = LA6.astype(ml_dtypes.bfloat16)
        m["lgT"] = bf(lgr[n].T)
        m["m0T"] = bf(m0[n].reshape(32, 4096).T.reshape(32, 128, 32))
        m["m1T"] = bf(m1[n].reshape(64, 512).T.reshape(4, 128, 64))
        m["Sh"] = bf(S[512 * h:512 * (h + 1)].reshape(4, 128, NG))
        in_maps.append(m)

    bint_zero = bool(np.all(np.asarray(inputs["b_int"]) == 0.0))
    nc = build_program(bpe, bpg, bb2, bint_zero)
    r = run_bass_kernel_spmd(nc, in_maps, core_ids=list(range(NCORES)),
                             trace=TRACE, **(TRACE_KW if TRACE else {}))
    global LAST
    LAST = r
    res = r.results

    out = np.zeros((NI, NG), dtype=np.float32)
    for n in range(NI):
        out[n] = (res[2 * n]["res_out"][0, 0:NG] + res[2 * n + 1]["res_out"][0, 0:NG]
                  + res[2 * n]["res_out"][0, NG:2 * NG])
    return out
